# revision 1
# baseline (speedup 1.0000x reference)
"""Trainium2 Bass kernel for a 2-layer edge-gated GCN (DiffGNNPlacement).

Math (reference, per layer):
    ew   = 0.5 + sigmoid(edge_logits)                  # [E]
    deg  = segsum(ew -> col) + 1                       # [N]
    dis  = deg^-1/2
    norm = dis[row] * ew * dis[col]                    # [E]
    out  = segsum(norm * (h@W)[row] -> col) + (h@W)*dis^2 + b

Key transform: aggregation commutes with the (linear) feature transform, so
    out = (segsum(norm * h[row] -> col) + h*dis^2) @ W + b
and the self-loop term becomes an extra "edge" (n -> n, weight dis[n]^2).
Layer 1 therefore needs no inter-device exchange at all (x is replicated);
only one all-gather of h1 is needed between the layers (done host-side,
since each core runs its own specialized program).

Device algorithm (per core, nodes sharded 12500/core):
  - edges partitioned by target shard, plus self-edges, sorted by target col,
    grouped by source chunk of 25000 rows (dma_gather indices are int16),
    packed into 128-slot tiles spanning <=32 target cols.
  - per tile: dma_gather 128 rows (256B each) of the feature table ->
    SBUF [128, 64]; a host-built one-hot-times-norm matrix S [128, 32];
    PE matmul psum[64, off:off+w] += gathered^T @ S accumulates the
    aggregation z^T for a 512-col PSUM window; windows flush to SBUF.
  - dense: h = relu(z @ W + b) chunk-wise (row-major for the gather table
    of the next layer), plus a transposed pass feeding the classifier head.

The same compiled program serves both layers (weights/tables are inputs);
it is launched twice per core with a host concat of h1 shards in between.
"""

import os
import sys
import math
import numpy as np
from contextlib import ExitStack

for _p in ("/opt/trn_rl_repo", "/root/.axon_site/_ro/trn_rl_repo"):
    if os.path.isdir(_p) and _p not in sys.path:
        sys.path.insert(0, _p)


# ----------------------------------------------------------------- config ---
class Cfg:
    def __init__(self, N=100000, E=1600000, C=64, H2=32, P=8,
                 SRC_CHUNK=25000, W=32, WIN=512, TCH=32, HBATCH=16):
        self.N, self.E, self.C, self.H2, self.P = N, E, C, H2, P
        self.NLOC = N // P
        self.SRC_CHUNK = SRC_CHUNK
        self.NGRP = (N + SRC_CHUNK - 1) // SRC_CHUNK
        self.W = W            # S tile width (target-col window per tile)
        self.WIN = WIN        # PSUM accumulation window (cols)
        self.TCH = TCH        # tiles per gather chunk
        self.HBATCH = HBATCH  # dense row-chunks per h_out DMA batch
        self.NWIN = (self.NLOC + WIN - 1) // WIN
        assert SRC_CHUNK <= 32767
        assert C * 4 == 256  # dma_gather elem constraint (256B rows)


FULL = Cfg()


# --------------------------------------------------------- host preprocess ---
def _sigmoid(x):
    return 0.5 * (np.tanh(0.5 * x) + 1.0)


def preprocess(x, edge_index, edge_logits, cfg=FULL):
    """Compute norms and per-device tile plans (pure numpy)."""
    N, NLOC, G = cfg.N, cfg.NLOC, cfg.NGRP
    row = np.asarray(edge_index[0], dtype=np.int64)
    col = np.asarray(edge_index[1], dtype=np.int64)
    ew = (0.5 + _sigmoid(np.asarray(edge_logits, dtype=np.float32))).astype(np.float32)
    deg = np.bincount(col, weights=ew.astype(np.float64), minlength=N).astype(np.float32) + 1.0
    dis = deg ** -0.5
    norm = (dis[row] * ew * dis[col]).astype(np.float32)

    # self-loop term (dis^2 * h) is folded in host-side via the sxT input
    a_row, a_col, a_val = row, col, norm
    dev = a_col // NLOC
    grp = a_row // cfg.SRC_CHUNK
    order = np.lexsort((a_col, grp, dev))
    a_row, a_col, a_val = a_row[order], a_col[order], a_val[order]
    dev, grp = dev[order], grp[order]

    # segment boundaries per (dev, grp)
    key = dev * G + grp
    bounds = np.searchsorted(key, np.arange(cfg.P * G + 1))
    plans = []
    for d in range(cfg.P):
        gplans = []
        for g in range(G):
            a, b = bounds[d * G + g], bounds[d * G + g + 1]
            gplans.append(_plan_group(
                (a_row[a:b] - g * cfg.SRC_CHUNK).astype(np.int16),
                (a_col[a:b] - d * NLOC).astype(np.int32),
                a_val[a:b], cfg))
        plans.append(gplans)
    return plans, dis


def _plan_group(rows, cols, vals, cfg):
    """Tile a sorted-by-col edge list: 128-slot tiles, <=W col span, not
    crossing WIN window boundaries. Returns packed gather/S arrays."""
    m = len(cols)
    starts, c0s = [], []
    i = 0
    while i < m:
        c0 = int(cols[i])
        lim = min(c0 + cfg.W, ((c0 // cfg.WIN) + 1) * cfg.WIN)
        jmax = min(i + 128, m)
        j = i + int(np.searchsorted(cols[i:jmax], lim, side="left"))
        starts.append(i)
        c0s.append(c0)
        i = j
    T = len(c0s)
    starts_a = np.array(starts + [m], dtype=np.int64)
    c0s = np.array(c0s, dtype=np.int32)

    tile_of = np.repeat(np.arange(T), np.diff(starts_a))
    slot = np.arange(m) - starts_a[tile_of]
    idx16 = np.zeros((T, 128), np.int16)
    idx16[tile_of, slot] = rows
    S = np.zeros((T, 128, cfg.W), np.float32)
    S[tile_of, slot, cols - c0s[tile_of]] = vals

    # chunk packing
    TCH = cfg.TCH
    nch = max(1, (T + TCH - 1) // TCH)
    Tp = nch * TCH
    flat = np.zeros(Tp * 128, np.int16)
    flat[: T * 128] = idx16.reshape(-1)
    # wrap: idx i -> [i % 16, i // 16], replicated across 8 groups of 16 partitions
    wrapped = flat.reshape(nch, TCH * 128 // 16, 16).transpose(0, 2, 1)  # [nch,16,TCH*8]
    idx_w = np.ascontiguousarray(np.tile(wrapped, (1, 8, 1)))            # [nch,128,TCH*8]
    Sp = np.zeros((Tp, 128, cfg.W), np.float32)
    Sp[:T] = S
    S_pk = np.ascontiguousarray(
        Sp.reshape(nch, TCH, 128, cfg.W).transpose(0, 2, 1, 3))          # [nch,128,TCH,W]
    nids = [min(TCH, T - ch * TCH) * 128 for ch in range(nch)]

    win = c0s // cfg.WIN
    off = c0s - win * cfg.WIN
    return dict(T=T, nch=nch, idx=idx_w, S=S_pk, nids=nids, win=win, off=off)


# ---------------------------------------------------------- program builder ---
def build_program(plan_d, cfg=FULL, name="gnn"):
    import concourse.bass as bass
    import concourse.mybir as mybir
    from concourse import bacc
    from concourse.tile import TileContext

    f32, i16 = mybir.dt.float32, mybir.dt.int16
    C, W, WIN, TCH, NLOC = cfg.C, cfg.W, cfg.WIN, cfg.TCH, cfg.NLOC
    G = cfg.NGRP

    nc = bacc.Bacc("TRN2", enable_partition_id=False,
                   target_bir_lowering=False, name=name)

    table = nc.dram_tensor("table", [cfg.N, C], f32, kind="ExternalInput")
    sxT_dr = nc.dram_tensor("sxT", [C, NLOC], f32, kind="ExternalInput")
    Wd = nc.dram_tensor("Wd", [C, C], f32, kind="ExternalInput")
    bb_dr = nc.dram_tensor("bb", [128, C], f32, kind="ExternalInput")
    bdc = nc.dram_tensor("bdc", [C, 1], f32, kind="ExternalInput")
    lw = nc.dram_tensor("lw", [C, 1], f32, kind="ExternalInput")
    lb = nc.dram_tensor("lb", [1, 1], f32, kind="ExternalInput")
    idx_dr, S_dr = [], []
    for g in range(G):
        p = plan_d[g]
        idx_dr.append(nc.dram_tensor(f"idx{g}", list(p["idx"].shape), i16,
                                     kind="ExternalInput"))
        S_dr.append(nc.dram_tensor(f"S{g}", list(p["S"].shape), f32,
                                   kind="ExternalInput"))
    h_out = nc.dram_tensor("h_out", [NLOC, C], f32, kind="ExternalOutput")
    outT = nc.dram_tensor("outT", [2, NLOC], f32, kind="ExternalOutput")

    # per-window tile lists: (g, t, off, weff)
    win_tiles = [[] for _ in range(cfg.NWIN)]
    for g in range(G):
        p = plan_d[g]
        for t in range(p["T"]):
            w = int(p["win"][t])
            off = int(p["off"][t])
            wlen = min(WIN, NLOC - w * WIN)
            weff = min(W, wlen - off)
            win_tiles[w].append((g, t, off, weff))

    with TileContext(nc) as tc, ExitStack() as ex:
        cpool = ex.enter_context(tc.tile_pool(name="consts", bufs=1))
        zpool = ex.enter_context(tc.tile_pool(name="z", bufs=1))
        gpools = [ex.enter_context(tc.tile_pool(name=f"gat{g}", bufs=2)) for g in range(G)]
        ipools = [ex.enter_context(tc.tile_pool(name=f"idx{g}", bufs=4)) for g in range(G)]
        spools = [ex.enter_context(tc.tile_pool(name=f"s{g}", bufs=2)) for g in range(G)]
        ppool = ex.enter_context(tc.tile_pool(name="psagg", bufs=2, space="PSUM"))
        pdpool = ex.enter_context(tc.tile_pool(name="psd", bufs=2, space="PSUM"))
        ptpool = ex.enter_context(tc.tile_pool(name="pst", bufs=2, space="PSUM"))
        plpool = ex.enter_context(tc.tile_pool(name="psl", bufs=2, space="PSUM"))
        hpool = ex.enter_context(tc.tile_pool(name="hrows", bufs=2))
        htpool = ex.enter_context(tc.tile_pool(name="ht", bufs=2))
        opool = ex.enter_context(tc.tile_pool(name="ot", bufs=2))

        # ---- constants
        zrow = cpool.tile([1, WIN], f32)
        nc.vector.memset(zrow[:, :], 0.0)
        Wd_sb = cpool.tile([C, C], f32)
        nc.sync.dma_start(out=Wd_sb[:, :], in_=Wd[:, :])
        bb = cpool.tile([128, C], f32)
        nc.sync.dma_start(out=bb[:, :], in_=bb_dr[:, :])
        bd_col = cpool.tile([C, 1], f32)
        nc.sync.dma_start(out=bd_col[:, :], in_=bdc[:, :])
        lw_sb = cpool.tile([C, 1], f32)
        nc.sync.dma_start(out=lw_sb[:, :], in_=lw[:, :])
        lb_sb = cpool.tile([1, 1], f32)
        nc.sync.dma_start(out=lb_sb[:, :], in_=lb[:, :])
        nlb = cpool.tile([1, 1], f32)
        nc.scalar.mul(nlb[:, :], lb_sb[:, :], -1.0)

        zT = zpool.tile([C, NLOC], f32)  # aggregation result, transposed
        nc.sync.dma_start(out=zT[:, :], in_=sxT_dr[:, :])  # self-loop term

        # ---- aggregation
        cur = [dict(ch=-1, gb=None, sb=None) for _ in range(G)]

        def ensure_chunk(g, ch):
            st = cur[g]
            if st["ch"] == ch:
                return st
            p = plan_d[g]
            ntl = min(TCH, p["T"] - ch * TCH)
            nid = p["nids"][ch]
            ib = ipools[g].tile([128, TCH * 8], i16, tag="idx")
            nc.sync.dma_start(out=ib[:, : ntl * 8], in_=idx_dr[g][ch, :, : ntl * 8])
            sb = spools[g].tile([128, TCH, W], f32, tag="s")
            nc.scalar.dma_start(out=sb[:, :ntl, :], in_=S_dr[g][ch, :, :ntl, :])
            gb = gpools[g].tile([128, TCH, C], f32, tag="g")
            nc.gpsimd.dma_gather(
                gb[:, :ntl, :],
                table[g * cfg.SRC_CHUNK:(g + 1) * cfg.SRC_CHUNK, :],
                ib[:, : ntl * 8],
                nid, nid, C,
                single_packet=False,
            )
            st.update(ch=ch, gb=gb, sb=sb)
            return st

        for w in range(cfg.NWIN):
            wlen = min(WIN, NLOC - w * WIN)
            ps = ppool.tile([C, WIN], f32)
            nc.tensor.matmul(ps[:, :wlen], lhsT=zrow[:, :C], rhs=zrow[:, :wlen],
                             start=True, stop=False)
            tl = win_tiles[w]
            for g, t, off, weff in tl:
                st = ensure_chunk(g, t // TCH)
                tp = t % TCH
                nc.tensor.matmul(
                    ps[:, off:off + weff],
                    lhsT=st["gb"][:, tp, :],
                    rhs=st["sb"][:, tp, :weff],
                    start=False, stop=False,
                    skip_group_check=True,
                )
            nc.tensor.matmul(ps[:, :wlen], lhsT=zrow[:, :C], rhs=zrow[:, :wlen],
                             start=False, stop=True)
            zw = zT[:, w * WIN:w * WIN + wlen]
            nc.vector.tensor_tensor(out=zw, in0=ps[:, :wlen], in1=zw,
                                    op=mybir.AluOpType.add)

        _stage = os.environ.get("GNN_STAGE", "all")

        # ---- dense, row-major (next layer's gather table)
        nck = (NLOC + 127) // 128 if _stage in ("all", "dense") else 0
        hb = None
        for k in range(nck):
            mrow = min(128, NLOC - k * 128)
            kk = k % cfg.HBATCH
            if kk == 0:
                nb = min(cfg.HBATCH, nck - k)
                hb = hpool.tile([128, cfg.HBATCH, C], f32, tag="h")
            psd = pdpool.tile([128, C], f32)
            nc.tensor.matmul(psd[:mrow, :], lhsT=zT[:, k * 128:k * 128 + mrow],
                             rhs=Wd_sb[:, :], start=True, stop=True)
            nc.vector.tensor_tensor(out=hb[:mrow, kk, :], in0=psd[:mrow, :],
                                    in1=bb[:mrow, :], op=mybir.AluOpType.add)
            nc.scalar.activation(hb[:mrow, kk, :], hb[:mrow, kk, :],
                                 mybir.ActivationFunctionType.Relu)
            if kk == nb - 1:
                k0 = k - kk
                r0, r1 = k0 * 128, min(NLOC, (k + 1) * 128)
                nfull = (r1 - r0) // 128
                if nfull:
                    dst = h_out[r0:r0 + nfull * 128, :].rearrange(
                        "(t p) c -> p t c", p=128)
                    nc.sync.dma_start(out=dst, in_=hb[:, :nfull, :])
                rem = (r1 - r0) - nfull * 128
                if rem:
                    nc.sync.dma_start(out=h_out[r0 + nfull * 128:r1, :],
                                      in_=hb[:rem, nfull, :])

        # ---- dense, transposed + head
        for q in range(cfg.NWIN if _stage in ("all", "head") else 0):
            wlen = min(WIN, NLOC - q * WIN)
            pst = ptpool.tile([C, WIN], f32)
            nc.tensor.matmul(pst[:, :wlen], lhsT=Wd_sb[:, :],
                             rhs=zT[:, q * WIN:q * WIN + wlen],
                             start=True, stop=True)
            ht = htpool.tile([C, WIN], f32, tag="ht")
            nc.scalar.activation(ht[:, :wlen], pst[:, :wlen],
                                 mybir.ActivationFunctionType.Relu, bias=bd_col[:, :])
            psl = plpool.tile([1, WIN], f32)
            nc.tensor.matmul(psl[:, :wlen], lhsT=lw_sb[:, :], rhs=ht[:, :wlen],
                             start=True, stop=True)
            otn = opool.tile([1, WIN], f32, tag="otn")
            otp = opool.tile([1, WIN], f32, tag="otp")
            nc.scalar.activation(otn[:, :wlen], psl[:, :wlen],
                                 mybir.ActivationFunctionType.Identity,
                                 bias=nlb[:, :], scale=-1.0)
            nc.scalar.activation(otp[:, :wlen], psl[:, :wlen],
                                 mybir.ActivationFunctionType.Identity,
                                 bias=lb_sb[:, :], scale=1.0)
            nc.sync.dma_start(out=outT[0:1, q * WIN:q * WIN + wlen], in_=otn[:, :wlen])
            nc.sync.dma_start(out=outT[1:2, q * WIN:q * WIN + wlen], in_=otp[:, :wlen])

    nc.compile()
    return nc


# ------------------------------------------------------------------ runner ---
def make_runner(nc, device):
    """Single-core jit runner pinned to one device, reusable across calls."""
    import jax
    import concourse.mybir as mybir
    from concourse import bass2jax

    bass2jax.install_neuronx_cc_hook()

    in_names, out_names, out_avals, zero_shapes = [], [], [], []
    for alloc in nc.m.functions[0].allocations:
        if not isinstance(alloc, mybir.MemoryLocationSet):
            continue
        nm = alloc.memorylocations[0].name
        if alloc.kind == "ExternalInput":
            in_names.append(nm)
        elif alloc.kind == "ExternalOutput":
            shape = tuple(alloc.tensor_shape)
            dtype = mybir.dt.np(alloc.dtype)
            out_names.append(nm)
            out_avals.append(jax.core.ShapedArray(shape, dtype))
            zero_shapes.append((shape, dtype))
    n_params = len(in_names)
    all_in_names = in_names + out_names
    donate = tuple(range(n_params, n_params + len(out_names)))

    def _body(*args):
        outs = bass2jax._bass_exec_p.bind(
            *args,
            out_avals=tuple(out_avals),
            in_names=tuple(all_in_names),
            out_names=tuple(out_names),
            lowering_input_output_aliases=(),
            sim_require_finite=True,
            sim_require_nnan=True,
            nc=nc,
        )
        return tuple(outs)

    jitted = jax.jit(_body, donate_argnums=donate, keep_unused=True)

    def run(in_map):
        args = [jax.device_put(np.asarray(in_map[nm]), device) for nm in in_names]
        zeros = [jax.device_put(np.zeros(s, d), device) for s, d in zero_shapes]
        outs = jitted(*args, *zeros)
        return {nm: outs[i] for i, nm in enumerate(out_names)}

    return run


# ---------------------------------------------------------------- kernel() ---
_CACHE = {}


def _get_runners(plans, cfg):
    import jax
    key = "runners"
    if key in _CACHE:
        return _CACHE[key]
    devices = jax.devices()[:cfg.P]
    ncs = [build_program(plans[d], cfg, name=f"gnn_d{d}") for d in range(cfg.P)]
    runners = [make_runner(ncs[d], devices[d]) for d in range(cfg.P)]
    _CACHE[key] = runners
    return runners


def run_two_phase(inputs, cfg=FULL):
    import jax
    from concurrent.futures import ThreadPoolExecutor

    x = np.asarray(inputs["x"], np.float32)
    W1 = np.asarray(inputs["W1"], np.float32)
    b1 = np.asarray(inputs["b1"], np.float32)
    W2 = np.asarray(inputs["W2"], np.float32)
    b2 = np.asarray(inputs["b2"], np.float32)
    lin_w = np.asarray(inputs["lin_w"], np.float32)
    lin_b = np.asarray(inputs["lin_b"], np.float32)
    C, H2 = cfg.C, cfg.H2

    plans, dis = preprocess(x, inputs["edge_index"], inputs["edge_logits"], cfg)
    dis2 = (dis * dis).astype(np.float32)
    runners = _get_runners(plans, cfg)

    W2p = np.zeros((C, C), np.float32)
    W2p[:, :H2] = W2
    b2p = np.zeros(C, np.float32)
    b2p[:H2] = b2
    lwp = np.zeros((C, 1), np.float32)
    lwp[:H2, 0] = lin_w[:, 0]
    lbp = lin_b.reshape(1, 1)
    zconst = np.zeros((C, 1), np.float32)

    def phase_inputs(d, table, Wd, bdv, lwv, lbv):
        p = plans[d]
        sh = slice(d * cfg.NLOC, (d + 1) * cfg.NLOC)
        sxT = np.ascontiguousarray((table[sh] * dis2[sh, None]).T)
        m = dict(table=table, sxT=sxT, Wd=Wd, bb=np.tile(bdv, (128, 1)),
                 bdc=bdv.reshape(C, 1), lw=lwv, lb=lbv)
        for g in range(cfg.NGRP):
            m[f"idx{g}"] = p[g]["idx"]
            m[f"S{g}"] = p[g]["S"]
        return m

    # phase A: table=x, dense=W1/b1 (head inputs zeroed; outT ignored)
    with ThreadPoolExecutor(cfg.P) as exe:
        resA = list(exe.map(
            lambda d: runners[d](phase_inputs(d, x, W1, b1, zconst,
                                              np.zeros((1, 1), np.float32))),
            range(cfg.P)))
    h1 = np.concatenate([np.asarray(r["h_out"]) for r in resA], axis=0)

    # phase B: table=h1, dense=padded W2/b2, head=lin
    with ThreadPoolExecutor(cfg.P) as exe:
        resB = list(exe.map(
            lambda d: runners[d](phase_inputs(d, h1, W2p, b2p, lwp, lbp)),
            range(cfg.P)))
    out = np.concatenate([np.asarray(r["outT"]).T for r in resB], axis=0)
    return out.astype(np.float32)


def kernel(x, edge_index, edge_logits, W1, b1, W2, b2, lin_w, lin_b):
    inputs = dict(x=x, edge_index=edge_index, edge_logits=edge_logits,
                  W1=W1, b1=b1, W2=W2, b2=b2, lin_w=lin_w, lin_b=lin_b)
    return run_two_phase(inputs, FULL)



# revision 2
# speedup vs baseline: 2.8147x; 2.8147x over previous
"""Trainium2 Bass kernel for a 2-layer edge-gated GCN (DiffGNNPlacement).

Math (reference, per layer):
    ew   = 0.5 + sigmoid(edge_logits)                  # [E]
    deg  = segsum(ew -> col) + 1                       # [N]
    dis  = deg^-1/2
    norm = dis[row] * ew * dis[col]                    # [E]
    out  = segsum(norm * (h@W)[row] -> col) + (h@W)*dis^2 + b

Key transform: aggregation commutes with the (linear) feature transform, so
    out = (segsum(norm * h[row] -> col) + h*dis^2) @ W + b
and the self-loop term is folded in host-side via the sxT input.

Device algorithm (per core, nodes sharded 12500/core):
  - node feature table stored bf16, padded to 256B rows ([N, 128] bf16) to
    satisfy the dma_gather 256B-elem constraint.
  - edges partitioned by target shard, sorted by target col, grouped by
    source chunk of 25000 rows (dma_gather indices are int16), packed into
    128-slot tiles spanning <=32 target cols.
  - per tile: dma_gather 128 rows -> SBUF [128, 128] bf16; a host-built
    one-hot-times-norm matrix S [128, 32] bf16; PE matmul
    psum[64, off:off+w] += gathered[:, :64]^T @ S accumulates the
    aggregation z^T for a 512-col PSUM window; windows flush to SBUF (f32).
  - SWDGE descriptor generation is the bottleneck engine: gathers are
    spread over all 4 SWDGE queues (queue q runs on Q7 cores 2q/2q+1), with
    queue_num = issue_index % 4 so the Tile DMASW lane (issue % 8) is always
    fed by a single queue (completions stay FIFO per lane -> race-free).
  - dense: h = relu(z @ W + b) chunk-wise in bf16 (row-major padded table
    layout for the next layer's gather), plus a transposed pass feeding the
    classifier head.

The same compiled program serves both layers; it is launched twice per core
with a host concat of h1 shards in between.
"""

import os
import sys
import math
import numpy as np
import ml_dtypes
from contextlib import ExitStack

for _p in ("/opt/trn_rl_repo", "/root/.axon_site/_ro/trn_rl_repo"):
    if os.path.isdir(_p) and _p not in sys.path:
        sys.path.insert(0, _p)

BF16 = ml_dtypes.bfloat16


# ----------------------------------------------------------------- config ---
class Cfg:
    def __init__(self, N=100000, E=1600000, C=64, H2=32, P=8,
                 SRC_CHUNK=25000, W=32, WIN=512, TCH=32, HBATCH=16, TBLW=128):
        self.N, self.E, self.C, self.H2, self.P = N, E, C, H2, P
        self.NLOC = N // P
        self.SRC_CHUNK = SRC_CHUNK
        self.NGRP = (N + SRC_CHUNK - 1) // SRC_CHUNK
        self.W = W            # S tile width (target-col window per tile)
        self.WIN = WIN        # PSUM accumulation window (cols)
        self.TCH = TCH        # tiles per gather chunk
        self.HBATCH = HBATCH  # dense row-chunks per h_out DMA batch
        self.TBLW = TBLW      # padded table row width (bf16 elems; 256B rows)
        self.NWIN = (self.NLOC + WIN - 1) // WIN
        assert SRC_CHUNK <= 32767
        assert TBLW * 2 == 256  # dma_gather elem constraint (256B rows)


FULL = Cfg()


# --------------------------------------------------------- host preprocess ---
def _sigmoid(x):
    return 0.5 * (np.tanh(0.5 * x) + 1.0)


def preprocess(edge_index, edge_logits, cfg=FULL):
    """Compute norms and per-device tile plans (pure numpy)."""
    N, NLOC, G = cfg.N, cfg.NLOC, cfg.NGRP
    row = np.asarray(edge_index[0], dtype=np.int64)
    col = np.asarray(edge_index[1], dtype=np.int64)
    ew = (0.5 + _sigmoid(np.asarray(edge_logits, dtype=np.float32))).astype(np.float32)
    deg = np.bincount(col, weights=ew.astype(np.float64), minlength=N).astype(np.float32) + 1.0
    dis = deg ** -0.5
    norm = (dis[row] * ew * dis[col]).astype(np.float32)

    # self-loop term (dis^2 * h) is folded in host-side via the sxT input
    a_row, a_col, a_val = row, col, norm
    dev = a_col // NLOC
    grp = a_row // cfg.SRC_CHUNK
    order = np.lexsort((a_col, grp, dev))
    a_row, a_col, a_val = a_row[order], a_col[order], a_val[order]
    dev, grp = dev[order], grp[order]

    # segment boundaries per (dev, grp)
    key = dev * G + grp
    bounds = np.searchsorted(key, np.arange(cfg.P * G + 1))
    plans = []
    for d in range(cfg.P):
        gplans = []
        for g in range(G):
            a, b = bounds[d * G + g], bounds[d * G + g + 1]
            gplans.append(_plan_group(
                (a_row[a:b] - g * cfg.SRC_CHUNK).astype(np.int16),
                (a_col[a:b] - d * NLOC).astype(np.int32),
                a_val[a:b], cfg))
        plans.append(gplans)
    return plans, dis


def _plan_group(rows, cols, vals, cfg):
    """Tile a sorted-by-col edge list: 128-slot tiles, <=W col span, not
    crossing WIN window boundaries. Returns packed gather/S arrays."""
    m = len(cols)
    starts, c0s = [], []
    i = 0
    while i < m:
        c0 = int(cols[i])
        lim = min(c0 + cfg.W, ((c0 // cfg.WIN) + 1) * cfg.WIN)
        jmax = min(i + 128, m)
        j = i + int(np.searchsorted(cols[i:jmax], lim, side="left"))
        starts.append(i)
        c0s.append(c0)
        i = j
    T = len(c0s)
    starts_a = np.array(starts + [m], dtype=np.int64)
    c0s = np.array(c0s, dtype=np.int32)

    tile_of = np.repeat(np.arange(T), np.diff(starts_a))
    slot = np.arange(m) - starts_a[tile_of]
    idx16 = np.zeros((T, 128), np.int16)
    idx16[tile_of, slot] = rows
    S = np.zeros((T, 128, cfg.W), np.float32)
    S[tile_of, slot, cols - c0s[tile_of]] = vals

    # chunk packing
    TCH = cfg.TCH
    nch = max(1, (T + TCH - 1) // TCH)
    Tp = nch * TCH
    flat = np.zeros(Tp * 128, np.int16)
    flat[: T * 128] = idx16.reshape(-1)
    # wrap: idx i -> [i % 16, i // 16], replicated across 8 groups of 16 partitions
    wrapped = flat.reshape(nch, TCH * 128 // 16, 16).transpose(0, 2, 1)  # [nch,16,TCH*8]
    idx_w = np.ascontiguousarray(np.tile(wrapped, (1, 8, 1)))            # [nch,128,TCH*8]
    Sp = np.zeros((Tp, 128, cfg.W), np.float32)
    Sp[:T] = S
    S_pk = np.ascontiguousarray(
        Sp.reshape(nch, TCH, 128, cfg.W).transpose(0, 2, 1, 3)).astype(BF16)  # [nch,128,TCH,W]
    nids = [min(TCH, T - ch * TCH) * 128 for ch in range(nch)]

    win = c0s // cfg.WIN
    off = c0s - win * cfg.WIN
    return dict(T=T, nch=nch, idx=idx_w, S=S_pk, nids=nids, win=win, off=off)


# ---------------------------------------------------------- program builder ---
def build_program(plan_d, cfg=FULL, name="gnn"):
    import concourse.bass as bass
    import concourse.mybir as mybir
    from concourse import bacc
    from concourse.tile import TileContext

    f32, i16, bf16 = mybir.dt.float32, mybir.dt.int16, mybir.dt.bfloat16
    C, W, WIN, TCH, NLOC = cfg.C, cfg.W, cfg.WIN, cfg.TCH, cfg.NLOC
    TBLW = cfg.TBLW
    G = cfg.NGRP
    NQ = 4  # SWDGE queues

    nc = bacc.Bacc("TRN2", enable_partition_id=False,
                   target_bir_lowering=False, name=name,
                   num_swdge_queues=NQ)

    table = nc.dram_tensor("table", [cfg.N, TBLW], bf16, kind="ExternalInput")
    sxT_dr = nc.dram_tensor("sxT", [C, NLOC], f32, kind="ExternalInput")
    Wd = nc.dram_tensor("Wd", [C, C], bf16, kind="ExternalInput")
    bb_dr = nc.dram_tensor("bb", [128, C], f32, kind="ExternalInput")
    bdc = nc.dram_tensor("bdc", [C, 1], f32, kind="ExternalInput")
    lw = nc.dram_tensor("lw", [C, 1], bf16, kind="ExternalInput")
    lb = nc.dram_tensor("lb", [1, 1], f32, kind="ExternalInput")
    idx_dr, S_dr = [], []
    for g in range(G):
        p = plan_d[g]
        idx_dr.append(nc.dram_tensor(f"idx{g}", list(p["idx"].shape), i16,
                                     kind="ExternalInput"))
        S_dr.append(nc.dram_tensor(f"S{g}", list(p["S"].shape), bf16,
                                   kind="ExternalInput"))
    h_out = nc.dram_tensor("h_out", [NLOC, TBLW], bf16, kind="ExternalOutput")
    outT = nc.dram_tensor("outT", [2, NLOC], f32, kind="ExternalOutput")

    # per-window tile lists: (g, t, off, weff)
    win_tiles = [[] for _ in range(cfg.NWIN)]
    for g in range(G):
        p = plan_d[g]
        for t in range(p["T"]):
            w = int(p["win"][t])
            off = int(p["off"][t])
            wlen = min(WIN, NLOC - w * WIN)
            weff = min(W, wlen - off)
            win_tiles[w].append((g, t, off, weff))

    with TileContext(nc) as tc, ExitStack() as ex:
        cpool = ex.enter_context(tc.tile_pool(name="consts", bufs=1))
        zpool = ex.enter_context(tc.tile_pool(name="z", bufs=1))
        gpools = [ex.enter_context(tc.tile_pool(name=f"gat{g}", bufs=2)) for g in range(G)]
        ipools = [ex.enter_context(tc.tile_pool(name=f"idx{g}", bufs=4)) for g in range(G)]
        spools = [ex.enter_context(tc.tile_pool(name=f"s{g}", bufs=2)) for g in range(G)]
        ppool = ex.enter_context(tc.tile_pool(name="psagg", bufs=2, space="PSUM"))
        pdpool = ex.enter_context(tc.tile_pool(name="psd", bufs=2, space="PSUM"))
        ptpool = ex.enter_context(tc.tile_pool(name="pst", bufs=2, space="PSUM"))
        plpool = ex.enter_context(tc.tile_pool(name="psl", bufs=2, space="PSUM"))
        hpool = ex.enter_context(tc.tile_pool(name="hrows", bufs=2))
        htpool = ex.enter_context(tc.tile_pool(name="ht", bufs=2))
        opool = ex.enter_context(tc.tile_pool(name="ot", bufs=2))

        # ---- constants
        zrow = cpool.tile([1, WIN], bf16)
        nc.vector.memset(zrow[:, :], 0.0)
        Wd_sb = cpool.tile([C, C], bf16)
        nc.sync.dma_start(out=Wd_sb[:, :], in_=Wd[:, :])
        bb = cpool.tile([128, C], f32)
        nc.sync.dma_start(out=bb[:, :], in_=bb_dr[:, :])
        bd_col = cpool.tile([C, 1], f32)
        nc.sync.dma_start(out=bd_col[:, :], in_=bdc[:, :])
        lw_sb = cpool.tile([C, 1], bf16)
        nc.sync.dma_start(out=lw_sb[:, :], in_=lw[:, :])
        lb_sb = cpool.tile([1, 1], f32)
        nc.sync.dma_start(out=lb_sb[:, :], in_=lb[:, :])
        nlb = cpool.tile([1, 1], f32)
        nc.scalar.mul(nlb[:, :], lb_sb[:, :], -1.0)

        zT = zpool.tile([C, NLOC], f32)  # aggregation result, transposed
        nc.sync.dma_start(out=zT[:, :], in_=sxT_dr[:, :])  # self-loop term
        zb = zpool.tile([C, NLOC], bf16)  # bf16 copy for the dense phase

        # ---- aggregation
        cur = [dict(ch=-1, gb=None, sb=None) for _ in range(G)]
        q_counter = [0]

        def ensure_chunk(g, ch):
            st = cur[g]
            if st["ch"] == ch:
                return st
            p = plan_d[g]
            ntl = min(TCH, p["T"] - ch * TCH)
            nid = p["nids"][ch]
            ib = ipools[g].tile([128, TCH * 8], i16, tag="idx")
            nc.sync.dma_start(out=ib[:, : ntl * 8], in_=idx_dr[g][ch, :, : ntl * 8])
            sb = spools[g].tile([128, TCH, W], bf16, tag="s")
            nc.scalar.dma_start(out=sb[:, :ntl, :], in_=S_dr[g][ch, :, :ntl, :])
            gb = gpools[g].tile([128, TCH, TBLW], bf16, tag="g")
            nc.gpsimd.dma_gather(
                gb[:, :ntl, :],
                table[g * cfg.SRC_CHUNK:(g + 1) * cfg.SRC_CHUNK, :],
                ib[:, : ntl * 8],
                nid, nid, TBLW,
                single_packet=False,
                queue_num=q_counter[0] % NQ,
            )
            q_counter[0] += 1
            st.update(ch=ch, gb=gb, sb=sb)
            return st

        for w in range(cfg.NWIN):
            wlen = min(WIN, NLOC - w * WIN)
            ps = ppool.tile([C, WIN], f32)
            nc.tensor.matmul(ps[:, :wlen], lhsT=zrow[:, :C], rhs=zrow[:, :wlen],
                             start=True, stop=False)
            tl = win_tiles[w]
            for g, t, off, weff in tl:
                st = ensure_chunk(g, t // TCH)
                tp = t % TCH
                nc.tensor.matmul(
                    ps[:, off:off + weff],
                    lhsT=st["gb"][:, tp, :C],
                    rhs=st["sb"][:, tp, :weff],
                    start=False, stop=False,
                    skip_group_check=True,
                )
            nc.tensor.matmul(ps[:, :wlen], lhsT=zrow[:, :C], rhs=zrow[:, :wlen],
                             start=False, stop=True)
            zw = zT[:, w * WIN:w * WIN + wlen]
            nc.vector.tensor_tensor(out=zw, in0=ps[:, :wlen], in1=zw,
                                    op=mybir.AluOpType.add)
            # bf16 copy for the dense/head matmuls
            nc.scalar.copy(out=zb[:, w * WIN:w * WIN + wlen], in_=zw)

        _stage = os.environ.get("GNN_STAGE", "all")

        # ---- dense, row-major (next layer's gather table)
        nck = (NLOC + 127) // 128 if _stage in ("all", "dense") else 0
        hb = None
        for k in range(nck):
            mrow = min(128, NLOC - k * 128)
            kk = k % cfg.HBATCH
            if kk == 0:
                nb = min(cfg.HBATCH, nck - k)
                hb = hpool.tile([128, cfg.HBATCH, TBLW], bf16, tag="h")
                nc.vector.memset(hb[:, :, C:], 0.0)  # zero the pad half
            psd = pdpool.tile([128, C], f32)
            nc.tensor.matmul(psd[:mrow, :], lhsT=zb[:, k * 128:k * 128 + mrow],
                             rhs=Wd_sb[:, :], start=True, stop=True)
            nc.vector.tensor_tensor(out=hb[:mrow, kk, :C], in0=psd[:mrow, :],
                                    in1=bb[:mrow, :], op=mybir.AluOpType.add)
            nc.scalar.activation(hb[:mrow, kk, :C], hb[:mrow, kk, :C],
                                 mybir.ActivationFunctionType.Relu)
            if kk == nb - 1:
                k0 = k - kk
                r0, r1 = k0 * 128, min(NLOC, (k + 1) * 128)
                nfull = (r1 - r0) // 128
                if nfull:
                    dst = h_out[r0:r0 + nfull * 128, :].rearrange(
                        "(t p) c -> p t c", p=128)
                    nc.sync.dma_start(out=dst, in_=hb[:, :nfull, :])
                rem = (r1 - r0) - nfull * 128
                if rem:
                    nc.sync.dma_start(out=h_out[r0 + nfull * 128:r1, :],
                                      in_=hb[:rem, nfull, :])

        # ---- dense, transposed + head
        for q in range(cfg.NWIN if _stage in ("all", "head") else 0):
            wlen = min(WIN, NLOC - q * WIN)
            pst = ptpool.tile([C, WIN], f32)
            nc.tensor.matmul(pst[:, :wlen], lhsT=Wd_sb[:, :],
                             rhs=zb[:, q * WIN:q * WIN + wlen],
                             start=True, stop=True)
            ht = htpool.tile([C, WIN], bf16, tag="ht")
            nc.scalar.activation(ht[:, :wlen], pst[:, :wlen],
                                 mybir.ActivationFunctionType.Relu, bias=bd_col[:, :])
            psl = plpool.tile([1, WIN], f32)
            nc.tensor.matmul(psl[:, :wlen], lhsT=lw_sb[:, :], rhs=ht[:, :wlen],
                             start=True, stop=True)
            otn = opool.tile([1, WIN], f32, tag="otn")
            otp = opool.tile([1, WIN], f32, tag="otp")
            nc.scalar.activation(otn[:, :wlen], psl[:, :wlen],
                                 mybir.ActivationFunctionType.Identity,
                                 bias=nlb[:, :], scale=-1.0)
            nc.scalar.activation(otp[:, :wlen], psl[:, :wlen],
                                 mybir.ActivationFunctionType.Identity,
                                 bias=lb_sb[:, :], scale=1.0)
            nc.sync.dma_start(out=outT[0:1, q * WIN:q * WIN + wlen], in_=otn[:, :wlen])
            nc.sync.dma_start(out=outT[1:2, q * WIN:q * WIN + wlen], in_=otp[:, :wlen])

    nc.compile()
    return nc


# ------------------------------------------------------------------ runner ---
def make_runner(nc, device):
    """Single-core jit runner pinned to one device, reusable across calls."""
    import jax
    import concourse.mybir as mybir
    from concourse import bass2jax

    bass2jax.install_neuronx_cc_hook()

    in_names, out_names, out_avals, zero_shapes = [], [], [], []
    for alloc in nc.m.functions[0].allocations:
        if not isinstance(alloc, mybir.MemoryLocationSet):
            continue
        nm = alloc.memorylocations[0].name
        if alloc.kind == "ExternalInput":
            in_names.append(nm)
        elif alloc.kind == "ExternalOutput":
            shape = tuple(alloc.tensor_shape)
            dtype = mybir.dt.np(alloc.dtype)
            out_names.append(nm)
            out_avals.append(jax.core.ShapedArray(shape, dtype))
            zero_shapes.append((shape, dtype))
    n_params = len(in_names)
    all_in_names = in_names + out_names
    donate = tuple(range(n_params, n_params + len(out_names)))

    def _body(*args):
        outs = bass2jax._bass_exec_p.bind(
            *args,
            out_avals=tuple(out_avals),
            in_names=tuple(all_in_names),
            out_names=tuple(out_names),
            lowering_input_output_aliases=(),
            sim_require_finite=True,
            sim_require_nnan=True,
            nc=nc,
        )
        return tuple(outs)

    jitted = jax.jit(_body, donate_argnums=donate, keep_unused=True)

    def run(in_map):
        args = [jax.device_put(np.asarray(in_map[nm]), device) for nm in in_names]
        zeros = [jax.device_put(np.zeros(s, d), device) for s, d in zero_shapes]
        outs = jitted(*args, *zeros)
        return {nm: outs[i] for i, nm in enumerate(out_names)}

    return run


# ---------------------------------------------------------------- kernel() ---
_CACHE = {}


def _get_runners(plans, cfg):
    import jax
    key = "runners"
    if key in _CACHE:
        return _CACHE[key]
    devices = jax.devices()[:cfg.P]
    ncs = [build_program(plans[d], cfg, name=f"gnn_d{d}") for d in range(cfg.P)]
    runners = [make_runner(ncs[d], devices[d]) for d in range(cfg.P)]
    _CACHE[key] = runners
    return runners


def _pad_table(x32, cfg):
    """f32 [N, 64] -> padded bf16 [N, 128] (256B rows)."""
    t = np.zeros((x32.shape[0], cfg.TBLW), BF16)
    t[:, :cfg.C] = x32.astype(BF16)
    return t


def run_two_phase(inputs, cfg=FULL):
    import jax
    from concurrent.futures import ThreadPoolExecutor

    x = np.asarray(inputs["x"], np.float32)
    W1 = np.asarray(inputs["W1"], np.float32)
    b1 = np.asarray(inputs["b1"], np.float32)
    W2 = np.asarray(inputs["W2"], np.float32)
    b2 = np.asarray(inputs["b2"], np.float32)
    lin_w = np.asarray(inputs["lin_w"], np.float32)
    lin_b = np.asarray(inputs["lin_b"], np.float32)
    C, H2 = cfg.C, cfg.H2

    plans, dis = preprocess(inputs["edge_index"], inputs["edge_logits"], cfg)
    dis2 = (dis * dis).astype(np.float32)
    runners = _get_runners(plans, cfg)

    x_tbl = _pad_table(x, cfg)
    W2p = np.zeros((C, C), np.float32)
    W2p[:, :H2] = W2
    b2p = np.zeros(C, np.float32)
    b2p[:H2] = b2
    lwp = np.zeros((C, 1), np.float32)
    lwp[:H2, 0] = lin_w[:, 0]
    lbp = lin_b.reshape(1, 1)
    zconst = np.zeros((C, 1), BF16)

    def phase_inputs(d, table, sx32, Wdv, bdv, lwv, lbv):
        p = plans[d]
        sh = slice(d * cfg.NLOC, (d + 1) * cfg.NLOC)
        sxT = np.ascontiguousarray((sx32[sh] * dis2[sh, None]).T)
        m = dict(table=table, sxT=sxT, Wd=Wdv.astype(BF16),
                 bb=np.tile(bdv, (128, 1)).astype(np.float32),
                 bdc=bdv.reshape(C, 1).astype(np.float32),
                 lw=lwv.astype(BF16), lb=lbv.astype(np.float32))
        for g in range(cfg.NGRP):
            m[f"idx{g}"] = p[g]["idx"]
            m[f"S{g}"] = p[g]["S"]
        return m

    # phase A: table=x, dense=W1/b1 (head inputs zeroed; outT ignored)
    with ThreadPoolExecutor(cfg.P) as exe:
        resA = list(exe.map(
            lambda d: runners[d](phase_inputs(d, x_tbl, x, W1, b1, zconst,
                                              np.zeros((1, 1), np.float32))),
            range(cfg.P)))
    h_shards = [np.asarray(r["h_out"]) for r in resA]
    h1_tbl = np.concatenate(h_shards, axis=0)          # already padded bf16
    h1_f32 = h1_tbl[:, :C].astype(np.float32)

    # phase B: table=h1, dense=padded W2/b2, head=lin
    with ThreadPoolExecutor(cfg.P) as exe:
        resB = list(exe.map(
            lambda d: runners[d](phase_inputs(d, h1_tbl, h1_f32, W2p, b2p, lwp, lbp)),
            range(cfg.P)))
    out = np.concatenate([np.asarray(r["outT"]).T for r in resB], axis=0)
    return out.astype(np.float32)


def kernel(x, edge_index, edge_logits, W1, b1, W2, b2, lin_w, lin_b):
    inputs = dict(x=x, edge_index=edge_index, edge_logits=edge_logits,
                  W1=W1, b1=b1, W2=W2, b2=b2, lin_w=lin_w, lin_b=lin_b)
    return run_two_phase(inputs, FULL)


# revision 3
# speedup vs baseline: 3.3312x; 1.1835x over previous
"""Trainium2 Bass kernel for a 2-layer edge-gated GCN (DiffGNNPlacement).

Math (reference, per layer):
    ew   = 0.5 + sigmoid(edge_logits)                  # [E]
    deg  = segsum(ew -> col) + 1                       # [N]
    dis  = deg^-1/2
    norm = dis[row] * ew * dis[col]                    # [E]
    out  = segsum(norm * (h@W)[row] -> col) + (h@W)*dis^2 + b

Key transform: aggregation commutes with the (linear) feature transform, so
    out = (segsum(norm * h[row] -> col) + h*dis^2) @ W + b
and the self-loop term is folded in host-side via the sxT input.

Device algorithm (per core, nodes sharded 12500/core):
  - node feature table stored bf16, padded to 256B rows ([N, 128] bf16) to
    satisfy the dma_gather 256B-elem constraint.
  - edges partitioned by target shard, sorted by target col, grouped by
    source chunk of 25000 rows (dma_gather indices are int16), packed into
    128-slot tiles spanning <=32 target cols.
  - per tile: dma_gather 128 rows -> SBUF [128, 128] bf16; a host-built
    one-hot-times-norm matrix S [128, 32] bf16; PE matmul
    psum[64, off:off+w] += gathered[:, :64]^T @ S accumulates the
    aggregation z^T for a 512-col PSUM window; windows flush to SBUF (f32).
  - SWDGE descriptor generation is the bottleneck engine (~9.5ns/descriptor,
    ring-write bound): gathers are spread over all 4 SWDGE queues (queue q
    runs on Q7 cores 2q/2q+1), with queue_num = issue_index % 4 so the Tile
    DMASW lane (issue % 8) is always fed by a single queue (completions stay
    FIFO per lane -> race-free). Early chunks are small so the PE pipeline
    ramps quickly.
  - dense h = relu(z @ W + b) and the classifier head are interleaved into
    the window loop (each 512-col window's dense work runs in PE stall gaps
    while the next gather round generates descriptors).

The same compiled program serves both layers; it is launched twice per core
with a host concat of h1 shards in between.
"""

import os
import sys
import math
import numpy as np
import ml_dtypes
from contextlib import ExitStack

for _p in ("/opt/trn_rl_repo", "/root/.axon_site/_ro/trn_rl_repo"):
    if os.path.isdir(_p) and _p not in sys.path:
        sys.path.insert(0, _p)

BF16 = ml_dtypes.bfloat16


# ----------------------------------------------------------------- config ---
class Cfg:
    def __init__(self, N=100000, E=1600000, C=64, H2=32, P=8,
                 SRC_CHUNK=25000, W=32, WIN=512, TCH=32, TBLW=128):
        self.N, self.E, self.C, self.H2, self.P = N, E, C, H2, P
        self.NLOC = N // P
        self.SRC_CHUNK = SRC_CHUNK
        self.NGRP = (N + SRC_CHUNK - 1) // SRC_CHUNK
        self.W = W            # S tile width (target-col window per tile)
        self.WIN = WIN        # PSUM accumulation window (cols)
        self.TCH = TCH        # steady-state tiles per gather chunk
        self.RAMP = (6, 12, 24)  # tile counts for the first chunks (fast ramp)
        self.TBLW = TBLW      # padded table row width (bf16 elems; 256B rows)
        self.NWIN = (self.NLOC + WIN - 1) // WIN
        assert SRC_CHUNK <= 32767
        assert TBLW * 2 == 256  # dma_gather elem constraint (256B rows)


FULL = Cfg()


# --------------------------------------------------------- host preprocess ---
def _sigmoid(x):
    return 0.5 * (np.tanh(0.5 * x) + 1.0)


def preprocess(edge_index, edge_logits, cfg=FULL):
    """Compute norms and per-device tile plans (pure numpy)."""
    N, NLOC, G = cfg.N, cfg.NLOC, cfg.NGRP
    row = np.asarray(edge_index[0], dtype=np.int64)
    col = np.asarray(edge_index[1], dtype=np.int64)
    ew = (0.5 + _sigmoid(np.asarray(edge_logits, dtype=np.float32))).astype(np.float32)
    deg = np.bincount(col, weights=ew.astype(np.float64), minlength=N).astype(np.float32) + 1.0
    dis = deg ** -0.5
    norm = (dis[row] * ew * dis[col]).astype(np.float32)

    # self-loop term (dis^2 * h) is folded in host-side via the sxT input
    a_row, a_col, a_val = row, col, norm
    dev = a_col // NLOC
    grp = a_row // cfg.SRC_CHUNK
    order = np.lexsort((a_col, grp, dev))
    a_row, a_col, a_val = a_row[order], a_col[order], a_val[order]
    dev, grp = dev[order], grp[order]

    # segment boundaries per (dev, grp)
    key = dev * G + grp
    bounds = np.searchsorted(key, np.arange(cfg.P * G + 1))
    plans = []
    for d in range(cfg.P):
        gplans = []
        for g in range(G):
            a, b = bounds[d * G + g], bounds[d * G + g + 1]
            gplans.append(_plan_group(
                (a_row[a:b] - g * cfg.SRC_CHUNK).astype(np.int16),
                (a_col[a:b] - d * NLOC).astype(np.int32),
                a_val[a:b], cfg))
        plans.append(gplans)
    return plans, dis


def _chunk_sizes(T, cfg):
    """Variable chunk sizes: small first chunks for fast pipeline ramp."""
    sizes = []
    for s in cfg.RAMP:
        if T - sum(sizes) <= 0:
            break
        sizes.append(min(s, T - sum(sizes)))
    rem = T - sum(sizes)
    while rem > 0:
        s = min(cfg.TCH, rem)
        sizes.append(s)
        rem -= s
    return sizes or [0]


def _plan_group(rows, cols, vals, cfg):
    """Tile a sorted-by-col edge list: 128-slot tiles, <=W col span, not
    crossing WIN window boundaries. Returns tile-major packed gather/S
    arrays plus a variable-size chunk schedule."""
    m = len(cols)
    starts, c0s = [], []
    i = 0
    while i < m:
        c0 = int(cols[i])
        lim = min(c0 + cfg.W, ((c0 // cfg.WIN) + 1) * cfg.WIN)
        jmax = min(i + 128, m)
        j = i + int(np.searchsorted(cols[i:jmax], lim, side="left"))
        starts.append(i)
        c0s.append(c0)
        i = j
    T = len(c0s)
    starts_a = np.array(starts + [m], dtype=np.int64)
    c0s = np.array(c0s, dtype=np.int32)

    tile_of = np.repeat(np.arange(T), np.diff(starts_a))
    slot = np.arange(m) - starts_a[tile_of]
    idx16 = np.zeros((max(T, 1), 128), np.int16)
    S = np.zeros((max(T, 1), 128, cfg.W), np.float32)
    if m:
        idx16[tile_of, slot] = rows
        S[tile_of, slot, cols - c0s[tile_of]] = vals

    # tile-major packing:
    # idx: per tile, 128 int16 wrapped [16, 8] and replicated across the 8
    # groups of 16 partitions -> [128, 8] per tile -> [128, T*8] overall.
    wrapped = idx16.reshape(max(T, 1), 8, 16).transpose(2, 0, 1)   # [16, T, 8]
    idx_w = np.ascontiguousarray(
        np.tile(wrapped.reshape(16, max(T, 1) * 8), (8, 1)))       # [128, T*8]
    S_pk = np.ascontiguousarray(S.transpose(1, 0, 2)).astype(BF16)  # [128, T, W]

    chunks = []   # (t0, ntl)
    t0 = 0
    for s in _chunk_sizes(T, cfg):
        chunks.append((t0, s))
        t0 += s

    win = c0s // cfg.WIN
    off = c0s - win * cfg.WIN
    return dict(T=T, idx=idx_w, S=S_pk, chunks=chunks, win=win, off=off)


# ---------------------------------------------------------- program builder ---
def build_program(plan_d, cfg=FULL, name="gnn"):
    import concourse.bass as bass
    import concourse.mybir as mybir
    from concourse import bacc
    from concourse.tile import TileContext

    f32, i16, bf16 = mybir.dt.float32, mybir.dt.int16, mybir.dt.bfloat16
    C, W, WIN, NLOC = cfg.C, cfg.W, cfg.WIN, cfg.NLOC
    TBLW = cfg.TBLW
    G = cfg.NGRP
    NQ = 4  # SWDGE queues
    TCH = cfg.TCH  # max chunk size (buffer allocation)

    nc = bacc.Bacc("TRN2", enable_partition_id=False,
                   target_bir_lowering=False, name=name,
                   num_swdge_queues=NQ)

    table = nc.dram_tensor("table", [cfg.N, TBLW], bf16, kind="ExternalInput")
    sxT_dr = nc.dram_tensor("sxT", [C, NLOC], f32, kind="ExternalInput")
    Wd = nc.dram_tensor("Wd", [C, C], bf16, kind="ExternalInput")
    bb_dr = nc.dram_tensor("bb", [128, C], f32, kind="ExternalInput")
    bdc = nc.dram_tensor("bdc", [C, 1], f32, kind="ExternalInput")
    lw = nc.dram_tensor("lw", [C, 1], bf16, kind="ExternalInput")
    lb = nc.dram_tensor("lb", [1, 1], f32, kind="ExternalInput")
    idx_dr, S_dr = [], []
    for g in range(G):
        p = plan_d[g]
        idx_dr.append(nc.dram_tensor(f"idx{g}", list(p["idx"].shape), i16,
                                     kind="ExternalInput"))
        S_dr.append(nc.dram_tensor(f"S{g}", list(p["S"].shape), bf16,
                                   kind="ExternalInput"))
    h_out = nc.dram_tensor("h_out", [NLOC, TBLW], bf16, kind="ExternalOutput")
    outT = nc.dram_tensor("outT", [2, NLOC], f32, kind="ExternalOutput")

    # per-window tile lists: (g, t, off, weff)
    win_tiles = [[] for _ in range(cfg.NWIN)]
    for g in range(G):
        p = plan_d[g]
        for t in range(p["T"]):
            w = int(p["win"][t])
            off = int(p["off"][t])
            wlen = min(WIN, NLOC - w * WIN)
            weff = min(W, wlen - off)
            win_tiles[w].append((g, t, off, weff))

    # tile index -> chunk index per group
    tile_chunk = []
    for g in range(G):
        p = plan_d[g]
        tc_map = np.zeros(max(p["T"], 1), np.int64)
        for ci, (t0, ntl) in enumerate(p["chunks"]):
            tc_map[t0:t0 + ntl] = ci
        tile_chunk.append(tc_map)

    _stage = os.environ.get("GNN_STAGE", "all")

    with TileContext(nc) as tc, ExitStack() as ex:
        cpool = ex.enter_context(tc.tile_pool(name="consts", bufs=1))
        zpool = ex.enter_context(tc.tile_pool(name="z", bufs=1))
        gpools = [ex.enter_context(tc.tile_pool(name=f"gat{g}", bufs=2)) for g in range(G)]
        ipools = [ex.enter_context(tc.tile_pool(name=f"idx{g}", bufs=4)) for g in range(G)]
        spools = [ex.enter_context(tc.tile_pool(name=f"s{g}", bufs=2)) for g in range(G)]
        ppool = ex.enter_context(tc.tile_pool(name="psagg", bufs=2, space="PSUM"))
        pdpool = ex.enter_context(tc.tile_pool(name="psd", bufs=2, space="PSUM"))
        ptpool = ex.enter_context(tc.tile_pool(name="pst", bufs=2, space="PSUM"))
        plpool = ex.enter_context(tc.tile_pool(name="psl", bufs=2, space="PSUM"))
        hpool = ex.enter_context(tc.tile_pool(name="hrows", bufs=2))
        htpool = ex.enter_context(tc.tile_pool(name="ht", bufs=2))
        opool = ex.enter_context(tc.tile_pool(name="ot", bufs=2))

        # ---- aggregation chunk loader (shared by pre-warm + window loop)
        cur = [dict(ch=-1, gb=None, sb=None) for _ in range(G)]
        q_counter = [0]

        def ensure_chunk(g, ch):
            st = cur[g]
            if st["ch"] == ch:
                return st
            p = plan_d[g]
            t0, ntl = p["chunks"][ch]
            nid = ntl * 128
            ib = ipools[g].tile([128, TCH * 8], i16, tag="idx")
            nc.sync.dma_start(out=ib[:, : ntl * 8], in_=idx_dr[g][:, t0 * 8:(t0 + ntl) * 8])
            sb = spools[g].tile([128, TCH, W], bf16, tag="s")
            nc.scalar.dma_start(out=sb[:, :ntl, :], in_=S_dr[g][:, t0:t0 + ntl, :])
            gb = gpools[g].tile([128, TCH, TBLW], bf16, tag="g")
            nc.gpsimd.dma_gather(
                gb[:, :ntl, :],
                table[g * cfg.SRC_CHUNK:(g + 1) * cfg.SRC_CHUNK, :],
                ib[:, : ntl * 8],
                nid, nid, TBLW,
                single_packet=False,
                queue_num=q_counter[0] % NQ,
            )
            q_counter[0] += 1
            st.update(ch=ch, gb=gb, sb=sb, t0=t0)
            return st

        # pre-warm the first gather round before the big sxT DMA queues up
        for g in range(G):
            if plan_d[g]["T"]:
                ensure_chunk(g, 0)

        # ---- constants
        zrow = cpool.tile([1, WIN], bf16)
        nc.vector.memset(zrow[:, :], 0.0)
        Wd_sb = cpool.tile([C, C], bf16)
        nc.sync.dma_start(out=Wd_sb[:, :], in_=Wd[:, :])
        bb = cpool.tile([128, C], f32)
        nc.sync.dma_start(out=bb[:, :], in_=bb_dr[:, :])
        bd_col = cpool.tile([C, 1], f32)
        nc.sync.dma_start(out=bd_col[:, :], in_=bdc[:, :])
        lw_sb = cpool.tile([C, 1], bf16)
        nc.sync.dma_start(out=lw_sb[:, :], in_=lw[:, :])
        lb_sb = cpool.tile([1, 1], f32)
        nc.sync.dma_start(out=lb_sb[:, :], in_=lb[:, :])
        nlb = cpool.tile([1, 1], f32)
        nc.scalar.mul(nlb[:, :], lb_sb[:, :], -1.0)

        zT = zpool.tile([C, NLOC], f32)  # aggregation result, transposed
        nc.sync.dma_start(out=zT[:, :], in_=sxT_dr[:, :])  # self-loop term
        zb = zpool.tile([C, NLOC], bf16)  # bf16 copy for the dense phase

        # ---- fused window loop: aggregation + dense + head per 512-col window
        HB = WIN // 128  # dense row-chunks per window (4)

        def dense_for_window(w):
            wlen = min(WIN, NLOC - w * WIN)
            nchunks = (wlen + 127) // 128
            hb = hpool.tile([128, HB, TBLW], bf16, tag="h")
            nc.vector.memset(hb[:, :, C:], 0.0)  # zero the pad half
            for kk in range(nchunks):
                k = w * HB + kk
                mrow = min(128, NLOC - k * 128)
                psd = pdpool.tile([128, C], f32)
                nc.tensor.matmul(psd[:mrow, :], lhsT=zb[:, k * 128:k * 128 + mrow],
                                 rhs=Wd_sb[:, :], start=True, stop=True)
                nc.vector.tensor_tensor(out=hb[:mrow, kk, :C], in0=psd[:mrow, :],
                                        in1=bb[:mrow, :], op=mybir.AluOpType.add)
                nc.scalar.activation(hb[:mrow, kk, :C], hb[:mrow, kk, :C],
                                     mybir.ActivationFunctionType.Relu)
            r0, r1 = w * WIN, w * WIN + wlen
            nfull = (r1 - r0) // 128
            if nfull:
                dst = h_out[r0:r0 + nfull * 128, :].rearrange(
                    "(t p) c -> p t c", p=128)
                nc.sync.dma_start(out=dst, in_=hb[:, :nfull, :])
            rem = (r1 - r0) - nfull * 128
            if rem:
                nc.sync.dma_start(out=h_out[r0 + nfull * 128:r1, :],
                                  in_=hb[:rem, nfull, :])

        def head_for_window(w):
            wlen = min(WIN, NLOC - w * WIN)
            pst = ptpool.tile([C, WIN], f32)
            nc.tensor.matmul(pst[:, :wlen], lhsT=Wd_sb[:, :],
                             rhs=zb[:, w * WIN:w * WIN + wlen],
                             start=True, stop=True)
            ht = htpool.tile([C, WIN], bf16, tag="ht")
            nc.scalar.activation(ht[:, :wlen], pst[:, :wlen],
                                 mybir.ActivationFunctionType.Relu, bias=bd_col[:, :])
            psl = plpool.tile([1, WIN], f32)
            nc.tensor.matmul(psl[:, :wlen], lhsT=lw_sb[:, :], rhs=ht[:, :wlen],
                             start=True, stop=True)
            otn = opool.tile([1, WIN], f32, tag="otn")
            otp = opool.tile([1, WIN], f32, tag="otp")
            nc.scalar.activation(otn[:, :wlen], psl[:, :wlen],
                                 mybir.ActivationFunctionType.Identity,
                                 bias=nlb[:, :], scale=-1.0)
            nc.scalar.activation(otp[:, :wlen], psl[:, :wlen],
                                 mybir.ActivationFunctionType.Identity,
                                 bias=lb_sb[:, :], scale=1.0)
            nc.sync.dma_start(out=outT[0:1, w * WIN:w * WIN + wlen], in_=otn[:, :wlen])
            nc.sync.dma_start(out=outT[1:2, w * WIN:w * WIN + wlen], in_=otp[:, :wlen])

        for w in range(cfg.NWIN):
            wlen = min(WIN, NLOC - w * WIN)
            ps = ppool.tile([C, WIN], f32)
            nc.tensor.matmul(ps[:, :wlen], lhsT=zrow[:, :C], rhs=zrow[:, :wlen],
                             start=True, stop=False)
            for g, t, off, weff in win_tiles[w]:
                st = ensure_chunk(g, int(tile_chunk[g][t]))
                tp = t - st["t0"]
                nc.tensor.matmul(
                    ps[:, off:off + weff],
                    lhsT=st["gb"][:, tp, :C],
                    rhs=st["sb"][:, tp, :weff],
                    start=False, stop=False,
                    skip_group_check=True,
                )
            nc.tensor.matmul(ps[:, :wlen], lhsT=zrow[:, :C], rhs=zrow[:, :wlen],
                             start=False, stop=True)
            zw = zT[:, w * WIN:w * WIN + wlen]
            nc.vector.tensor_tensor(out=zw, in0=ps[:, :wlen], in1=zw,
                                    op=mybir.AluOpType.add)
            # bf16 copy for the dense/head matmuls
            nc.scalar.copy(out=zb[:, w * WIN:w * WIN + wlen], in_=zw)
            if _stage in ("all", "dense"):
                dense_for_window(w)
            if _stage in ("all", "head"):
                head_for_window(w)

    nc.compile()
    return nc


# ------------------------------------------------------------------ runner ---
def make_runner(nc, device):
    """Single-core jit runner pinned to one device, reusable across calls."""
    import jax
    import concourse.mybir as mybir
    from concourse import bass2jax

    bass2jax.install_neuronx_cc_hook()

    in_names, out_names, out_avals, zero_shapes = [], [], [], []
    for alloc in nc.m.functions[0].allocations:
        if not isinstance(alloc, mybir.MemoryLocationSet):
            continue
        nm = alloc.memorylocations[0].name
        if alloc.kind == "ExternalInput":
            in_names.append(nm)
        elif alloc.kind == "ExternalOutput":
            shape = tuple(alloc.tensor_shape)
            dtype = mybir.dt.np(alloc.dtype)
            out_names.append(nm)
            out_avals.append(jax.core.ShapedArray(shape, dtype))
            zero_shapes.append((shape, dtype))
    n_params = len(in_names)
    all_in_names = in_names + out_names
    donate = tuple(range(n_params, n_params + len(out_names)))

    def _body(*args):
        outs = bass2jax._bass_exec_p.bind(
            *args,
            out_avals=tuple(out_avals),
            in_names=tuple(all_in_names),
            out_names=tuple(out_names),
            lowering_input_output_aliases=(),
            sim_require_finite=True,
            sim_require_nnan=True,
            nc=nc,
        )
        return tuple(outs)

    jitted = jax.jit(_body, donate_argnums=donate, keep_unused=True)

    def run(in_map):
        args = [jax.device_put(np.asarray(in_map[nm]), device) for nm in in_names]
        zeros = [jax.device_put(np.zeros(s, d), device) for s, d in zero_shapes]
        outs = jitted(*args, *zeros)
        return {nm: outs[i] for i, nm in enumerate(out_names)}

    return run


# ---------------------------------------------------------------- kernel() ---
_CACHE = {}


def _get_runners(plans, cfg):
    import jax
    key = "runners"
    if key in _CACHE:
        return _CACHE[key]
    devices = jax.devices()[:cfg.P]
    ncs = [build_program(plans[d], cfg, name=f"gnn_d{d}") for d in range(cfg.P)]
    runners = [make_runner(ncs[d], devices[d]) for d in range(cfg.P)]
    _CACHE[key] = runners
    return runners


def _pad_table(x32, cfg):
    """f32 [N, 64] -> padded bf16 [N, 128] (256B rows)."""
    t = np.zeros((x32.shape[0], cfg.TBLW), BF16)
    t[:, :cfg.C] = x32.astype(BF16)
    return t


def run_two_phase(inputs, cfg=FULL):
    import jax
    from concurrent.futures import ThreadPoolExecutor

    x = np.asarray(inputs["x"], np.float32)
    W1 = np.asarray(inputs["W1"], np.float32)
    b1 = np.asarray(inputs["b1"], np.float32)
    W2 = np.asarray(inputs["W2"], np.float32)
    b2 = np.asarray(inputs["b2"], np.float32)
    lin_w = np.asarray(inputs["lin_w"], np.float32)
    lin_b = np.asarray(inputs["lin_b"], np.float32)
    C, H2 = cfg.C, cfg.H2

    plans, dis = preprocess(inputs["edge_index"], inputs["edge_logits"], cfg)
    dis2 = (dis * dis).astype(np.float32)
    runners = _get_runners(plans, cfg)

    x_tbl = _pad_table(x, cfg)
    W2p = np.zeros((C, C), np.float32)
    W2p[:, :H2] = W2
    b2p = np.zeros(C, np.float32)
    b2p[:H2] = b2
    lwp = np.zeros((C, 1), np.float32)
    lwp[:H2, 0] = lin_w[:, 0]
    lbp = lin_b.reshape(1, 1)
    zconst = np.zeros((C, 1), BF16)

    def phase_inputs(d, table, sx32, Wdv, bdv, lwv, lbv):
        p = plans[d]
        sh = slice(d * cfg.NLOC, (d + 1) * cfg.NLOC)
        sxT = np.ascontiguousarray((sx32[sh] * dis2[sh, None]).T)
        m = dict(table=table, sxT=sxT, Wd=Wdv.astype(BF16),
                 bb=np.tile(bdv, (128, 1)).astype(np.float32),
                 bdc=bdv.reshape(C, 1).astype(np.float32),
                 lw=lwv.astype(BF16), lb=lbv.astype(np.float32))
        for g in range(cfg.NGRP):
            m[f"idx{g}"] = p[g]["idx"]
            m[f"S{g}"] = p[g]["S"]
        return m

    # phase A: table=x, dense=W1/b1 (head inputs zeroed; outT ignored)
    with ThreadPoolExecutor(cfg.P) as exe:
        resA = list(exe.map(
            lambda d: runners[d](phase_inputs(d, x_tbl, x, W1, b1, zconst,
                                              np.zeros((1, 1), np.float32))),
            range(cfg.P)))
    h_shards = [np.asarray(r["h_out"]) for r in resA]
    h1_tbl = np.concatenate(h_shards, axis=0)          # already padded bf16
    h1_f32 = h1_tbl[:, :C].astype(np.float32)

    # phase B: table=h1, dense=padded W2/b2, head=lin
    with ThreadPoolExecutor(cfg.P) as exe:
        resB = list(exe.map(
            lambda d: runners[d](phase_inputs(d, h1_tbl, h1_f32, W2p, b2p, lwp, lbp)),
            range(cfg.P)))
    out = np.concatenate([np.asarray(r["outT"]).T for r in resB], axis=0)
    return out.astype(np.float32)


def kernel(x, edge_index, edge_logits, W1, b1, W2, b2, lin_w, lin_b):
    inputs = dict(x=x, edge_index=edge_index, edge_logits=edge_logits,
                  W1=W1, b1=b1, W2=W2, b2=b2, lin_w=lin_w, lin_b=lin_b)
    return run_two_phase(inputs, FULL)


# revision 5
# speedup vs baseline: 3.3757x; 1.0134x over previous
"""Trainium2 Bass kernel for a 2-layer edge-gated GCN (DiffGNNPlacement).

Math (reference, per layer):
    ew   = 0.5 + sigmoid(edge_logits)                  # [E]
    deg  = segsum(ew -> col) + 1                       # [N]
    dis  = deg^-1/2
    norm = dis[row] * ew * dis[col]                    # [E]
    out  = segsum(norm * (h@W)[row] -> col) + (h@W)*dis^2 + b

Key transform: aggregation commutes with the (linear) feature transform, so
    out = (segsum(norm * h[row] -> col) + h*dis^2) @ W + b
and the self-loop term is folded in host-side via the sxT input.

Device algorithm (per core, nodes sharded 12500/core):
  - node feature table stored bf16, padded to 256B rows ([N, 128] bf16) to
    satisfy the dma_gather 256B-elem constraint.
  - edges partitioned by target shard, sorted by target col, grouped by
    source chunk of 25000 rows (dma_gather indices are int16), packed into
    128-slot tiles spanning <=32 target cols.
  - per tile: dma_gather 128 rows -> SBUF [128, 128] bf16; a host-built
    one-hot-times-norm matrix S [128, 32] bf16; PE matmul
    psum[64, off:off+w] += gathered[:, :64]^T @ S accumulates the
    aggregation z^T for a 512-col PSUM window; windows flush to SBUF (f32).
  - SWDGE descriptor generation is the bottleneck engine (~9.5ns/descriptor,
    ring-write bound): gathers are spread over all 4 SWDGE queues (queue q
    runs on Q7 cores 2q/2q+1), with queue_num = issue_index % 4 so the Tile
    DMASW lane (issue % 8) is always fed by a single queue (completions stay
    FIFO per lane -> race-free). Early chunks are small so the PE pipeline
    ramps quickly.
  - dense h = relu(z @ W + b) and the classifier head are interleaved into
    the window loop (each 512-col window's dense work runs in PE stall gaps
    while the next gather round generates descriptors).

The same compiled program serves both layers; it is launched twice per core
with a host concat of h1 shards in between.
"""

import os
import sys
import math
import numpy as np
import ml_dtypes
from contextlib import ExitStack

for _p in ("/opt/trn_rl_repo", "/root/.axon_site/_ro/trn_rl_repo"):
    if os.path.isdir(_p) and _p not in sys.path:
        sys.path.insert(0, _p)

BF16 = ml_dtypes.bfloat16


# ----------------------------------------------------------------- config ---
class Cfg:
    def __init__(self, N=100000, E=1600000, C=64, H2=32, P=8,
                 SRC_CHUNK=25000, W=32, WIN=512, TCH=32, TBLW=128):
        self.N, self.E, self.C, self.H2, self.P = N, E, C, H2, P
        self.NLOC = N // P
        self.SRC_CHUNK = SRC_CHUNK
        self.NGRP = (N + SRC_CHUNK - 1) // SRC_CHUNK
        self.W = W            # S tile width (target-col window per tile)
        self.WIN = WIN        # PSUM accumulation window (cols)
        self.TCH = TCH        # steady-state tiles per gather chunk
        self.RAMP = (6, 12, 24)  # tile counts for the first chunks (fast ramp)
        self.TBLW = TBLW      # padded table row width (bf16 elems; 256B rows)
        self.NWIN = (self.NLOC + WIN - 1) // WIN
        assert SRC_CHUNK <= 32767
        assert TBLW * 2 == 256  # dma_gather elem constraint (256B rows)


FULL = Cfg()


# --------------------------------------------------------- host preprocess ---
def _sigmoid(x):
    return 0.5 * (np.tanh(0.5 * x) + 1.0)


def preprocess(edge_index, edge_logits, cfg=FULL):
    """Compute norms and per-device tile plans (pure numpy)."""
    N, NLOC, G = cfg.N, cfg.NLOC, cfg.NGRP
    row = np.asarray(edge_index[0], dtype=np.int64)
    col = np.asarray(edge_index[1], dtype=np.int64)
    ew = (0.5 + _sigmoid(np.asarray(edge_logits, dtype=np.float32))).astype(np.float32)
    deg = np.bincount(col, weights=ew.astype(np.float64), minlength=N).astype(np.float32) + 1.0
    dis = deg ** -0.5
    norm = (dis[row] * ew * dis[col]).astype(np.float32)

    # self-loop term (dis^2 * h) is folded in host-side via the sxT input
    a_row, a_col, a_val = row, col, norm
    dev = a_col // NLOC
    grp = a_row // cfg.SRC_CHUNK
    order = np.lexsort((a_col, grp, dev))
    a_row, a_col, a_val = a_row[order], a_col[order], a_val[order]
    dev, grp = dev[order], grp[order]

    # segment boundaries per (dev, grp)
    key = dev * G + grp
    bounds = np.searchsorted(key, np.arange(cfg.P * G + 1))
    plans = []
    for d in range(cfg.P):
        gplans = []
        for g in range(G):
            a, b = bounds[d * G + g], bounds[d * G + g + 1]
            gplans.append(_plan_group(
                (a_row[a:b] - g * cfg.SRC_CHUNK).astype(np.int16),
                (a_col[a:b] - d * NLOC).astype(np.int32),
                a_val[a:b], cfg))
        plans.append(gplans)
    return plans, dis


def _chunk_sizes(T, cfg):
    """Variable chunk sizes: small first chunks for fast pipeline ramp."""
    sizes = []
    for s in cfg.RAMP:
        if T - sum(sizes) <= 0:
            break
        sizes.append(min(s, T - sum(sizes)))
    rem = T - sum(sizes)
    while rem > 0:
        s = min(cfg.TCH, rem)
        sizes.append(s)
        rem -= s
    return sizes or [0]


def _plan_group(rows, cols, vals, cfg):
    """Tile a sorted-by-col edge list: 128-slot tiles, <=W col span, not
    crossing WIN window boundaries. Returns tile-major packed gather/S
    arrays plus a variable-size chunk schedule."""
    m = len(cols)
    starts, c0s = [], []
    i = 0
    while i < m:
        c0 = int(cols[i])
        lim = min(c0 + cfg.W, ((c0 // cfg.WIN) + 1) * cfg.WIN)
        jmax = min(i + 128, m)
        j = i + int(np.searchsorted(cols[i:jmax], lim, side="left"))
        starts.append(i)
        c0s.append(c0)
        i = j
    T = len(c0s)
    starts_a = np.array(starts + [m], dtype=np.int64)
    c0s = np.array(c0s, dtype=np.int32)

    tile_of = np.repeat(np.arange(T), np.diff(starts_a))
    slot = np.arange(m) - starts_a[tile_of]
    idx16 = np.zeros((max(T, 1), 128), np.int16)
    S = np.zeros((max(T, 1), 128, cfg.W), np.float32)
    if m:
        idx16[tile_of, slot] = rows
        S[tile_of, slot, cols - c0s[tile_of]] = vals

    # tile-major packing:
    # idx: per tile, 128 int16 wrapped [16, 8] and replicated across the 8
    # groups of 16 partitions -> [128, 8] per tile -> [128, T*8] overall.
    wrapped = idx16.reshape(max(T, 1), 8, 16).transpose(2, 0, 1)   # [16, T, 8]
    idx_w = np.ascontiguousarray(
        np.tile(wrapped.reshape(16, max(T, 1) * 8), (8, 1)))       # [128, T*8]
    S_pk = np.ascontiguousarray(S.transpose(1, 0, 2)).astype(BF16)  # [128, T, W]

    chunks = []   # (t0, ntl)
    t0 = 0
    for s in _chunk_sizes(T, cfg):
        chunks.append((t0, s))
        t0 += s

    win = c0s // cfg.WIN
    off = c0s - win * cfg.WIN
    return dict(T=T, idx=idx_w, S=S_pk, chunks=chunks, win=win, off=off)


# ---------------------------------------------------------- program builder ---
def build_program(plan_d, cfg=FULL, name="gnn"):
    import concourse.bass as bass
    import concourse.mybir as mybir
    from concourse import bacc
    from concourse.tile import TileContext

    f32, i16, bf16 = mybir.dt.float32, mybir.dt.int16, mybir.dt.bfloat16
    C, W, WIN, NLOC = cfg.C, cfg.W, cfg.WIN, cfg.NLOC
    TBLW = cfg.TBLW
    G = cfg.NGRP
    NQ = 4  # SWDGE queues
    TCH = cfg.TCH  # max chunk size (buffer allocation)

    nc = bacc.Bacc("TRN2", enable_partition_id=False,
                   target_bir_lowering=False, name=name,
                   num_swdge_queues=NQ)

    table = nc.dram_tensor("table", [cfg.N, TBLW], bf16, kind="ExternalInput")
    sxT_dr = nc.dram_tensor("sxT", [C, NLOC], f32, kind="ExternalInput")
    Wd = nc.dram_tensor("Wd", [C, C], bf16, kind="ExternalInput")
    bb_dr = nc.dram_tensor("bb", [128, C], f32, kind="ExternalInput")
    bdc = nc.dram_tensor("bdc", [C, 1], f32, kind="ExternalInput")
    lw = nc.dram_tensor("lw", [C, 1], bf16, kind="ExternalInput")
    lb = nc.dram_tensor("lb", [1, 1], f32, kind="ExternalInput")
    idx_dr, S_dr = [], []
    for g in range(G):
        p = plan_d[g]
        idx_dr.append(nc.dram_tensor(f"idx{g}", list(p["idx"].shape), i16,
                                     kind="ExternalInput"))
        S_dr.append(nc.dram_tensor(f"S{g}", list(p["S"].shape), bf16,
                                   kind="ExternalInput"))
    h_out = nc.dram_tensor("h_out", [NLOC, TBLW], bf16, kind="ExternalOutput")
    outT = nc.dram_tensor("outT", [2, NLOC], f32, kind="ExternalOutput")

    # per-window tile lists: (g, t, off, weff)
    win_tiles = [[] for _ in range(cfg.NWIN)]
    for g in range(G):
        p = plan_d[g]
        for t in range(p["T"]):
            w = int(p["win"][t])
            off = int(p["off"][t])
            wlen = min(WIN, NLOC - w * WIN)
            weff = min(W, wlen - off)
            win_tiles[w].append((g, t, off, weff))

    # tile index -> chunk index per group
    tile_chunk = []
    for g in range(G):
        p = plan_d[g]
        tc_map = np.zeros(max(p["T"], 1), np.int64)
        for ci, (t0, ntl) in enumerate(p["chunks"]):
            tc_map[t0:t0 + ntl] = ci
        tile_chunk.append(tc_map)

    _stage = os.environ.get("GNN_STAGE", "all")

    with TileContext(nc) as tc, ExitStack() as ex:
        cpool = ex.enter_context(tc.tile_pool(name="consts", bufs=1))
        zpool = ex.enter_context(tc.tile_pool(name="z", bufs=1))
        gpools = [ex.enter_context(tc.tile_pool(name=f"gat{g}", bufs=2)) for g in range(G)]
        ipools = [ex.enter_context(tc.tile_pool(name=f"idx{g}", bufs=4)) for g in range(G)]
        spools = [ex.enter_context(tc.tile_pool(name=f"s{g}", bufs=2)) for g in range(G)]
        ppool = ex.enter_context(tc.tile_pool(name="psagg", bufs=2, space="PSUM"))
        pdpool = ex.enter_context(tc.tile_pool(name="psd", bufs=2, space="PSUM"))
        ptpool = ex.enter_context(tc.tile_pool(name="pst", bufs=2, space="PSUM"))
        plpool = ex.enter_context(tc.tile_pool(name="psl", bufs=2, space="PSUM"))
        hpool = ex.enter_context(tc.tile_pool(name="hrows", bufs=2))
        htpool = ex.enter_context(tc.tile_pool(name="ht", bufs=2))
        opool = ex.enter_context(tc.tile_pool(name="ot", bufs=2))

        # ---- aggregation chunk loader (shared by pre-warm + window loop)
        cur = [dict(ch=-1, gb=None, sb=None) for _ in range(G)]
        q_counter = [0]

        def ensure_chunk(g, ch):
            st = cur[g]
            if st["ch"] == ch:
                return st
            p = plan_d[g]
            t0, ntl = p["chunks"][ch]
            nid = ntl * 128
            ib = ipools[g].tile([128, TCH * 8], i16, tag="idx")
            nc.sync.dma_start(out=ib[:, : ntl * 8], in_=idx_dr[g][:, t0 * 8:(t0 + ntl) * 8])
            sb = spools[g].tile([128, TCH, W], bf16, tag="s")
            nc.scalar.dma_start(out=sb[:, :ntl, :], in_=S_dr[g][:, t0:t0 + ntl, :])
            gb = gpools[g].tile([128, TCH, TBLW], bf16, tag="g")
            nc.gpsimd.dma_gather(
                gb[:, :ntl, :],
                table[g * cfg.SRC_CHUNK:(g + 1) * cfg.SRC_CHUNK, :],
                ib[:, : ntl * 8],
                nid, nid, TBLW,
                single_packet=False,
                queue_num=q_counter[0] % NQ,
            )
            q_counter[0] += 1
            st.update(ch=ch, gb=gb, sb=sb, t0=t0)
            return st

        # pre-warm the first gather round before the big sxT DMA queues up
        for g in range(G):
            if plan_d[g]["T"]:
                ensure_chunk(g, 0)

        # ---- constants
        zrow = cpool.tile([1, WIN], bf16)
        nc.vector.memset(zrow[:, :], 0.0)
        Wd_sb = cpool.tile([C, C], bf16)
        nc.sync.dma_start(out=Wd_sb[:, :], in_=Wd[:, :])
        bb = cpool.tile([128, C], f32)
        nc.sync.dma_start(out=bb[:, :], in_=bb_dr[:, :])
        bd_col = cpool.tile([C, 1], f32)
        nc.sync.dma_start(out=bd_col[:, :], in_=bdc[:, :])
        lw_sb = cpool.tile([C, 1], bf16)
        nc.sync.dma_start(out=lw_sb[:, :], in_=lw[:, :])
        lb_sb = cpool.tile([1, 1], f32)
        nc.sync.dma_start(out=lb_sb[:, :], in_=lb[:, :])
        nlb = cpool.tile([1, 1], f32)
        nc.scalar.mul(nlb[:, :], lb_sb[:, :], -1.0)

        zT = zpool.tile([C, NLOC], f32)  # aggregation result, transposed
        nc.sync.dma_start(out=zT[:, :], in_=sxT_dr[:, :])  # self-loop term
        zb = zpool.tile([C, NLOC], bf16)  # bf16 copy for the dense phase

        # ---- fused window loop: aggregation + dense + head per 512-col window
        HB = WIN // 128  # dense row-chunks per window (4)

        def dense_for_window(w):
            wlen = min(WIN, NLOC - w * WIN)
            nchunks = (wlen + 127) // 128
            hb = hpool.tile([128, HB, TBLW], bf16, tag="h")
            nc.vector.memset(hb[:, :, C:], 0.0)  # zero the pad half
            for kk in range(nchunks):
                k = w * HB + kk
                mrow = min(128, NLOC - k * 128)
                psd = pdpool.tile([128, C], f32)
                nc.tensor.matmul(psd[:mrow, :], lhsT=zb[:, k * 128:k * 128 + mrow],
                                 rhs=Wd_sb[:, :], start=True, stop=True)
                nc.vector.tensor_tensor(out=hb[:mrow, kk, :C], in0=psd[:mrow, :],
                                        in1=bb[:mrow, :], op=mybir.AluOpType.add)
                nc.scalar.activation(hb[:mrow, kk, :C], hb[:mrow, kk, :C],
                                     mybir.ActivationFunctionType.Relu)
            r0, r1 = w * WIN, w * WIN + wlen
            nfull = (r1 - r0) // 128
            if nfull:
                dst = h_out[r0:r0 + nfull * 128, :].rearrange(
                    "(t p) c -> p t c", p=128)
                nc.sync.dma_start(out=dst, in_=hb[:, :nfull, :])
            rem = (r1 - r0) - nfull * 128
            if rem:
                nc.sync.dma_start(out=h_out[r0 + nfull * 128:r1, :],
                                  in_=hb[:rem, nfull, :])

        def head_for_window(w):
            wlen = min(WIN, NLOC - w * WIN)
            pst = ptpool.tile([C, WIN], f32)
            nc.tensor.matmul(pst[:, :wlen], lhsT=Wd_sb[:, :],
                             rhs=zb[:, w * WIN:w * WIN + wlen],
                             start=True, stop=True)
            ht = htpool.tile([C, WIN], bf16, tag="ht")
            nc.scalar.activation(ht[:, :wlen], pst[:, :wlen],
                                 mybir.ActivationFunctionType.Relu, bias=bd_col[:, :])
            psl = plpool.tile([1, WIN], f32)
            nc.tensor.matmul(psl[:, :wlen], lhsT=lw_sb[:, :], rhs=ht[:, :wlen],
                             start=True, stop=True)
            otn = opool.tile([1, WIN], f32, tag="otn")
            otp = opool.tile([1, WIN], f32, tag="otp")
            nc.scalar.activation(otn[:, :wlen], psl[:, :wlen],
                                 mybir.ActivationFunctionType.Identity,
                                 bias=nlb[:, :], scale=-1.0)
            nc.scalar.activation(otp[:, :wlen], psl[:, :wlen],
                                 mybir.ActivationFunctionType.Identity,
                                 bias=lb_sb[:, :], scale=1.0)
            nc.sync.dma_start(out=outT[0:1, w * WIN:w * WIN + wlen], in_=otn[:, :wlen])
            nc.sync.dma_start(out=outT[1:2, w * WIN:w * WIN + wlen], in_=otp[:, :wlen])

        for w in range(cfg.NWIN):
            wlen = min(WIN, NLOC - w * WIN)
            ps = ppool.tile([C, WIN], f32)
            nc.tensor.matmul(ps[:, :wlen], lhsT=zrow[:, :C], rhs=zrow[:, :wlen],
                             start=True, stop=False)
            for g, t, off, weff in win_tiles[w]:
                st = ensure_chunk(g, int(tile_chunk[g][t]))
                tp = t - st["t0"]
                nc.tensor.matmul(
                    ps[:, off:off + weff],
                    lhsT=st["gb"][:, tp, :C],
                    rhs=st["sb"][:, tp, :weff],
                    start=False, stop=False,
                    skip_group_check=True,
                )
            nc.tensor.matmul(ps[:, :wlen], lhsT=zrow[:, :C], rhs=zrow[:, :wlen],
                             start=False, stop=True)
            zw = zT[:, w * WIN:w * WIN + wlen]
            nc.vector.tensor_tensor(out=zw, in0=ps[:, :wlen], in1=zw,
                                    op=mybir.AluOpType.add)
            # bf16 copy for the dense/head matmuls
            nc.scalar.copy(out=zb[:, w * WIN:w * WIN + wlen], in_=zw)
            if _stage in ("all", "dense"):
                dense_for_window(w)
            if _stage in ("all", "head"):
                head_for_window(w)

    nc.compile()
    return nc


# ------------------------------------------------------------------ runner ---
def make_runner(nc, device):
    """Single-core jit runner pinned to one device, reusable across calls."""
    import jax
    import concourse.mybir as mybir
    from concourse import bass2jax

    bass2jax.install_neuronx_cc_hook()

    in_names, out_names, out_avals, zero_shapes = [], [], [], []
    for alloc in nc.m.functions[0].allocations:
        if not isinstance(alloc, mybir.MemoryLocationSet):
            continue
        nm = alloc.memorylocations[0].name
        if alloc.kind == "ExternalInput":
            in_names.append(nm)
        elif alloc.kind == "ExternalOutput":
            shape = tuple(alloc.tensor_shape)
            dtype = mybir.dt.np(alloc.dtype)
            out_names.append(nm)
            out_avals.append(jax.core.ShapedArray(shape, dtype))
            zero_shapes.append((shape, dtype))
    n_params = len(in_names)
    all_in_names = in_names + out_names
    donate = tuple(range(n_params, n_params + len(out_names)))

    def _body(*args):
        outs = bass2jax._bass_exec_p.bind(
            *args,
            out_avals=tuple(out_avals),
            in_names=tuple(all_in_names),
            out_names=tuple(out_names),
            lowering_input_output_aliases=(),
            sim_require_finite=True,
            sim_require_nnan=True,
            nc=nc,
        )
        return tuple(outs)

    jitted = jax.jit(_body, donate_argnums=donate, keep_unused=True)

    def run(in_map):
        args = [jax.device_put(np.asarray(in_map[nm]), device) for nm in in_names]
        zeros = [jax.device_put(np.zeros(s, d), device) for s, d in zero_shapes]
        outs = jitted(*args, *zeros)
        return {nm: outs[i] for i, nm in enumerate(out_names)}

    return run


# ---------------------------------------------------------------- kernel() ---
_CACHE = {}


def _get_runners(plans, cfg):
    import jax
    key = "runners"
    if key in _CACHE:
        return _CACHE[key]
    devices = jax.devices()[:cfg.P]
    ncs = [build_program(plans[d], cfg, name=f"gnn_d{d}") for d in range(cfg.P)]
    runners = [make_runner(ncs[d], devices[d]) for d in range(cfg.P)]
    _CACHE[key] = runners
    return runners


def _pad_table(x32, cfg):
    """f32 [N, 64] -> padded bf16 [N, 128] (256B rows)."""
    t = np.zeros((x32.shape[0], cfg.TBLW), BF16)
    t[:, :cfg.C] = x32.astype(BF16)
    return t


def run_two_phase(inputs, cfg=FULL):
    import jax
    from concurrent.futures import ThreadPoolExecutor

    x = np.asarray(inputs["x"], np.float32)
    W1 = np.asarray(inputs["W1"], np.float32)
    b1 = np.asarray(inputs["b1"], np.float32)
    W2 = np.asarray(inputs["W2"], np.float32)
    b2 = np.asarray(inputs["b2"], np.float32)
    lin_w = np.asarray(inputs["lin_w"], np.float32)
    lin_b = np.asarray(inputs["lin_b"], np.float32)
    C, H2 = cfg.C, cfg.H2

    plans, dis = preprocess(inputs["edge_index"], inputs["edge_logits"], cfg)
    dis2 = (dis * dis).astype(np.float32)
    runners = _get_runners(plans, cfg)

    x_tbl = _pad_table(x, cfg)
    W2p = np.zeros((C, C), np.float32)
    W2p[:, :H2] = W2
    b2p = np.zeros(C, np.float32)
    b2p[:H2] = b2
    lwp = np.zeros((C, 1), np.float32)
    lwp[:H2, 0] = lin_w[:, 0]
    lbp = lin_b.reshape(1, 1)
    zconst = np.zeros((C, 1), BF16)

    def phase_inputs(d, table, sx32, Wdv, bdv, lwv, lbv):
        p = plans[d]
        sh = slice(d * cfg.NLOC, (d + 1) * cfg.NLOC)
        sxT = np.ascontiguousarray((sx32[sh] * dis2[sh, None]).T)
        m = dict(table=table, sxT=sxT, Wd=Wdv.astype(BF16),
                 bb=np.tile(bdv, (128, 1)).astype(np.float32),
                 bdc=bdv.reshape(C, 1).astype(np.float32),
                 lw=lwv.astype(BF16), lb=lbv.astype(np.float32))
        for g in range(cfg.NGRP):
            m[f"idx{g}"] = p[g]["idx"]
            m[f"S{g}"] = p[g]["S"]
        return m

    # phase A: table=x, dense=W1/b1 (head inputs zeroed; outT ignored)
    with ThreadPoolExecutor(cfg.P) as exe:
        resA = list(exe.map(
            lambda d: runners[d](phase_inputs(d, x_tbl, x, W1, b1, zconst,
                                              np.zeros((1, 1), np.float32))),
            range(cfg.P)))
    h_shards = [np.asarray(r["h_out"]) for r in resA]
    h1_tbl = np.concatenate(h_shards, axis=0)          # already padded bf16
    h1_f32 = h1_tbl[:, :C].astype(np.float32)

    # phase B: table=h1, dense=padded W2/b2, head=lin
    with ThreadPoolExecutor(cfg.P) as exe:
        resB = list(exe.map(
            lambda d: runners[d](phase_inputs(d, h1_tbl, h1_f32, W2p, b2p, lwp, lbp)),
            range(cfg.P)))
    out = np.concatenate([np.asarray(r["outT"]).T for r in resB], axis=0)
    return out.astype(np.float32)


def kernel(x, edge_index, edge_logits, W1, b1, W2, b2, lin_w, lin_b):
    inputs = dict(x=x, edge_index=edge_index, edge_logits=edge_logits,
                  W1=W1, b1=b1, W2=W2, b2=b2, lin_w=lin_w, lin_b=lin_b)
    return run_two_phase(inputs, FULL)


# revision 7
# speedup vs baseline: 3.5420x; 1.0493x over previous
"""Trainium2 Bass kernel for a 2-layer edge-gated GCN (DiffGNNPlacement).

Math (reference, per layer):
    ew   = 0.5 + sigmoid(edge_logits)                  # [E]
    deg  = segsum(ew -> col) + 1                       # [N]
    dis  = deg^-1/2
    norm = dis[row] * ew * dis[col]                    # [E]
    out  = segsum(norm * (h@W)[row] -> col) + (h@W)*dis^2 + b

Key transform: aggregation commutes with the (linear) feature transform, so
    out = (segsum(norm * h[row] -> col) + h*dis^2) @ W + b
and the self-loop term is folded in host-side via the sxT input.

Device algorithm (per core, nodes sharded 12500/core):
  - SWDGE descriptor generation is the bottleneck (~9.5ns/descriptor): every
    gathered row costs one descriptor, so nodes are PAIRED per core (greedy
    matching of sources that co-occur in the same 16-target-col bucket) and
    the per-core feature table stores one 256B bf16 row per PAIR. One
    descriptor then feeds up to two edges.
  - edges partitioned by target shard, sorted by target col; slots are
    (pair, bucket) units; tiles pack 128 slots spanning <=32 target cols.
  - per tile: dma_gather 128 pair-rows -> SBUF [128, 128] bf16; host-built
    Su/Sv [128, 32] bf16 (norms of the u-half / v-half edges); two PE
    matmuls psum[64, off:off+w] += gathered[:, h*64:h*64+64]^T @ S_h
    accumulate the aggregation z^T for a 512-col PSUM window.
  - gathers spread over all 4 SWDGE queues (queue q = Q7 cores 2q/2q+1),
    queue_num = issue_index % 4 so the Tile DMASW lane (issue % 8) is fed by
    a single queue (completions stay FIFO per lane -> race-free). Early
    chunks are small for fast pipeline ramp.
  - dense h = relu(z @ W + b) and the classifier head are interleaved into
    the window loop (run in PE stall gaps while descriptors generate).

The same compiled program serves both layers; it is launched twice per core
with a host re-pairing of h1 shards in between.
"""

import os
import sys
import math
import numpy as np
import ml_dtypes
from contextlib import ExitStack

for _p in ("/opt/trn_rl_repo", "/root/.axon_site/_ro/trn_rl_repo"):
    if os.path.isdir(_p) and _p not in sys.path:
        sys.path.insert(0, _p)

BF16 = ml_dtypes.bfloat16


# ----------------------------------------------------------------- config ---
class Cfg:
    def __init__(self, N=100000, E=1600000, C=64, H2=32, P=8,
                 PAIR_CHUNK=25000, B=16, W=32, WIN=512, TCH=32, TBLW=128):
        self.N, self.E, self.C, self.H2, self.P = N, E, C, H2, P
        self.NLOC = N // P
        self.NPAIR = N // 2
        self.PAIR_CHUNK = PAIR_CHUNK          # pairs per gather group
        self.NGRP = (self.NPAIR + PAIR_CHUNK - 1) // PAIR_CHUNK
        self.B = B            # pairing bucket (target cols)
        self.W = W            # S tile width (target-col window per tile)
        self.WIN = WIN        # PSUM accumulation window (cols)
        self.TCH = TCH        # steady-state tiles per gather chunk
        self.RAMP = (6, 12, 24)  # tile counts for the first chunks
        self.TBLW = TBLW      # pair row width (bf16 elems; 256B rows)
        self.NWIN = (self.NLOC + WIN - 1) // WIN
        assert PAIR_CHUNK <= 32767
        assert TBLW * 2 == 256  # dma_gather elem constraint (256B rows)
        assert 2 * B <= W and WIN % B == 0


FULL = Cfg()


# --------------------------------------------------------- host preprocess ---
def _sigmoid(x):
    return 0.5 * (np.tanh(0.5 * x) + 1.0)


def _pair_nodes(r, c, cfg):
    """Greedy per-core pairing: sort sources by the first B-col bucket they
    appear in, pair adjacent. Returns pairs [NPAIR, 2] (a permutation of all
    nodes)."""
    N, B = cfg.N, cfg.B
    bucket = c // B
    o = np.lexsort((bucket, r))
    rs, bs = r[o], bucket[o]
    first = np.ones(len(rs), bool)
    if len(rs):
        first[1:] = rs[1:] != rs[:-1]
    src_f, buck_f = rs[first], bs[first]
    present = np.zeros(N, bool)
    present[src_f] = True
    absent = np.where(~present)[0]
    o3 = np.argsort(buck_f, kind="stable")
    allsrc = np.concatenate([src_f[o3], absent])
    pairs = allsrc.reshape(-1, 2)
    pairid = np.empty(N, np.int64)
    half = np.empty(N, np.int8)
    pairid[pairs[:, 0]] = np.arange(len(pairs))
    pairid[pairs[:, 1]] = np.arange(len(pairs))
    half[pairs[:, 0]] = 0
    half[pairs[:, 1]] = 1
    return pairs, pairid, half


def preprocess(edge_index, edge_logits, cfg=FULL):
    """Compute norms and per-device pairings + tile plans (pure numpy)."""
    N, NLOC = cfg.N, cfg.NLOC
    row = np.asarray(edge_index[0], dtype=np.int64)
    col = np.asarray(edge_index[1], dtype=np.int64)
    ew = (0.5 + _sigmoid(np.asarray(edge_logits, dtype=np.float32))).astype(np.float32)
    deg = np.bincount(col, weights=ew.astype(np.float64), minlength=N).astype(np.float32) + 1.0
    dis = deg ** -0.5
    norm = (dis[row] * ew * dis[col]).astype(np.float32)

    dev = col // NLOC
    order = np.argsort(dev, kind="stable")
    rs, cs, vs, ds = row[order], col[order] % NLOC, norm[order], dev[order]
    bounds = np.searchsorted(ds, np.arange(cfg.P + 1))
    plans = []
    for d in range(cfg.P):
        a, b = bounds[d], bounds[d + 1]
        plans.append(_plan_device(rs[a:b], cs[a:b], vs[a:b], cfg))
    return plans, dis


def _chunk_sizes(T, cfg):
    sizes = []
    for s in cfg.RAMP:
        if T - sum(sizes) <= 0:
            break
        sizes.append(min(s, T - sum(sizes)))
    rem = T - sum(sizes)
    while rem > 0:
        s = min(cfg.TCH, rem)
        sizes.append(s)
        rem -= s
    return sizes or [0]


def _plan_device(r, c, v, cfg):
    """Pair sources, build (pair, bucket) slots, pack 128-slot tiles."""
    B, W, WIN, G = cfg.B, cfg.W, cfg.WIN, cfg.NGRP
    pairs, pairid, half = _pair_nodes(r, c, cfg)
    perm = pairs.reshape(-1)                     # node order in the pair table

    p = pairid[r]
    h = half[r].astype(np.int64)
    grp = p // cfg.PAIR_CHUNK
    bucket = c // B

    # slots: distinct (grp, bucket, pair); edges sorted into slot-major order
    okey = ((grp * 800 + bucket) * np.int64(cfg.NPAIR)) + p
    o = np.argsort(okey, kind="stable")
    co, vo, ho, go, ko = c[o], v[o], h[o], grp[o], okey[o]
    po = p[o]
    m = len(ko)
    newslot = np.ones(max(m, 1), bool)
    if m:
        newslot[1:m] = ko[1:] != ko[:-1]
    slot_of_edge = np.cumsum(newslot[:m]) - 1 if m else np.zeros(0, np.int64)
    nslot = int(slot_of_edge[-1]) + 1 if m else 0
    slot_starts = np.where(newslot[:m])[0] if m else np.zeros(0, np.int64)
    slot_pair = po[slot_starts] if m else np.zeros(0, np.int64)
    slot_grp = go[slot_starts] if m else np.zeros(0, np.int64)
    slot_c0 = np.minimum.reduceat(co, slot_starts) if m else np.zeros(0, np.int64)
    slot_cmax = np.maximum.reduceat(co, slot_starts) if m else np.zeros(0, np.int64)

    # tiles: sequential pack per grp, <=128 slots, span < W, same WIN window
    tile_id = np.zeros(max(nslot, 1), np.int64)
    tile_c0s = []
    tile_grps = []
    t = -1
    cnt = 0
    cur_c0 = -10 ** 9
    cur_grp = -1
    for s in range(nslot):
        g = int(slot_grp[s])
        # align the tile base to the slot's bucket start: later slots in the
        # same bucket (ordered by pair id) may have smaller cols
        c0 = (int(slot_c0[s]) // B) * B
        cmax = int(slot_cmax[s])
        if (g != cur_grp or cnt >= 128 or cmax >= cur_c0 + W
                or (cmax // WIN) != (cur_c0 // WIN)):
            t += 1
            cnt = 0
            cur_c0 = c0
            cur_grp = g
            tile_c0s.append(c0)
            tile_grps.append(g)
        tile_id[s] = t
        cnt += 1
    T = t + 1 if nslot else 0
    tile_c0s = np.array(tile_c0s if T else [0], np.int32)
    tile_grp = np.array(tile_grps if T else [0], np.int64)
    tile_first_slot = np.searchsorted(tile_id[:max(nslot, 1)], np.arange(max(T, 1)))
    slot_in_tile = (np.arange(max(nslot, 1)) - tile_first_slot[tile_id]) if nslot else np.zeros(1, np.int64)

    Tm = max(T, 1)
    idx16 = np.zeros((Tm, 128), np.int16)
    if nslot:
        idx16[tile_id[:nslot], slot_in_tile[:nslot]] = (
            slot_pair - slot_grp * cfg.PAIR_CHUNK).astype(np.int16)
    S = np.zeros((Tm, 2, 128, cfg.W), np.float32)
    if m:
        e_tile = tile_id[slot_of_edge]
        e_slot = slot_in_tile[slot_of_edge]
        e_coff = co - tile_c0s[e_tile]
        np.add.at(S, (e_tile, ho, e_slot, e_coff), vo)

    # tile-major packing for the device
    wrapped = idx16.reshape(Tm, 8, 16).transpose(2, 0, 1)          # [16, T, 8]
    idx_w = np.ascontiguousarray(np.tile(wrapped.reshape(16, Tm * 8), (8, 1)))
    S_pk = np.ascontiguousarray(S.transpose(2, 0, 1, 3)).astype(BF16)  # [128, T, 2, W]

    # group tiles by grp for chunking (tiles are grp-ordered)
    gT = [int(np.sum(tile_grp[:T] == g)) for g in range(G)] if T else [0] * G
    gt0 = [int(np.searchsorted(tile_grp[:T], g)) for g in range(G)] if T else [0] * G
    gchunks = []
    for g in range(G):
        chunks = []
        t0 = gt0[g]
        for s in _chunk_sizes(gT[g], cfg):
            chunks.append((t0, s))
            t0 += s
        gchunks.append(chunks)

    win = tile_c0s // WIN
    off = tile_c0s - win * WIN
    return dict(T=T, idx=idx_w, S=S_pk, perm=perm, gchunks=gchunks,
                gT=gT, gt0=gt0, win=win, off=off, tile_grp=tile_grp)


# ---------------------------------------------------------- program builder ---
def build_program(plan, cfg=FULL, name="gnn"):
    import concourse.bass as bass
    import concourse.mybir as mybir
    from concourse import bacc
    from concourse.tile import TileContext

    f32, i16, bf16 = mybir.dt.float32, mybir.dt.int16, mybir.dt.bfloat16
    C, W, WIN, NLOC = cfg.C, cfg.W, cfg.WIN, cfg.NLOC
    TBLW = cfg.TBLW
    G = cfg.NGRP
    NQ = 4  # SWDGE queues
    TCH = cfg.TCH

    nc = bacc.Bacc("TRN2", enable_partition_id=False,
                   target_bir_lowering=False, name=name,
                   num_swdge_queues=NQ)

    table = nc.dram_tensor("table", [cfg.NPAIR, TBLW], bf16, kind="ExternalInput")
    sxT_dr = nc.dram_tensor("sxT", [C, NLOC], f32, kind="ExternalInput")
    Wd = nc.dram_tensor("Wd", [C, C], bf16, kind="ExternalInput")
    bb_dr = nc.dram_tensor("bb", [128, C], f32, kind="ExternalInput")
    bdc = nc.dram_tensor("bdc", [C, 1], f32, kind="ExternalInput")
    lw = nc.dram_tensor("lw", [C, 1], bf16, kind="ExternalInput")
    lb = nc.dram_tensor("lb", [1, 1], f32, kind="ExternalInput")
    idx_dr = nc.dram_tensor("idx", list(plan["idx"].shape), i16, kind="ExternalInput")
    S_dr = nc.dram_tensor("S", list(plan["S"].shape), bf16, kind="ExternalInput")
    h_out = nc.dram_tensor("h_out", [NLOC, C], bf16, kind="ExternalOutput")
    outT = nc.dram_tensor("outT", [2, NLOC], f32, kind="ExternalOutput")

    # per-window tile lists: (g, t, off, weff)
    win_tiles = [[] for _ in range(cfg.NWIN)]
    for t in range(plan["T"]):
        w = int(plan["win"][t])
        off = int(plan["off"][t])
        wlen = min(WIN, NLOC - w * WIN)
        weff = min(W, wlen - off)
        win_tiles[w].append((int(plan["tile_grp"][t]), t, off, weff))

    # tile -> chunk per group
    tile_chunk = np.zeros(max(plan["T"], 1), np.int64)
    for g in range(G):
        for ci, (t0, ntl) in enumerate(plan["gchunks"][g]):
            tile_chunk[t0:t0 + ntl] = ci

    _stage = os.environ.get("GNN_STAGE", "all")

    with TileContext(nc) as tc, ExitStack() as ex:
        cpool = ex.enter_context(tc.tile_pool(name="consts", bufs=1))
        zpool = ex.enter_context(tc.tile_pool(name="z", bufs=1))
        gpools = [ex.enter_context(tc.tile_pool(name=f"gat{g}", bufs=3)) for g in range(G)]
        ipools = [ex.enter_context(tc.tile_pool(name=f"idx{g}", bufs=4)) for g in range(G)]
        spools = [ex.enter_context(tc.tile_pool(name=f"s{g}", bufs=3)) for g in range(G)]
        ppool = ex.enter_context(tc.tile_pool(name="psagg", bufs=2, space="PSUM"))
        pdpool = ex.enter_context(tc.tile_pool(name="psd", bufs=2, space="PSUM"))
        ptpool = ex.enter_context(tc.tile_pool(name="pst", bufs=2, space="PSUM"))
        plpool = ex.enter_context(tc.tile_pool(name="psl", bufs=2, space="PSUM"))
        hpool = ex.enter_context(tc.tile_pool(name="hrows", bufs=2))
        htpool = ex.enter_context(tc.tile_pool(name="ht", bufs=2))
        opool = ex.enter_context(tc.tile_pool(name="ot", bufs=2))

        # ---- aggregation chunk loader
        cur = [dict(ch=-1, gb=None, sb=None, t0=0) for _ in range(G)]
        q_counter = [0]

        def ensure_chunk(g, ch):
            st = cur[g]
            if st["ch"] == ch:
                return st
            t0, ntl = plan["gchunks"][g][ch]
            nid = ntl * 128
            ib = ipools[g].tile([128, TCH * 8], i16, tag="idx")
            nc.sync.dma_start(out=ib[:, : ntl * 8], in_=idx_dr[:, t0 * 8:(t0 + ntl) * 8])
            sb = spools[g].tile([128, TCH, 2, W], bf16, tag="s")
            nc.scalar.dma_start(out=sb[:, :ntl, :, :], in_=S_dr[:, t0:t0 + ntl, :, :])
            gb = gpools[g].tile([128, TCH, TBLW], bf16, tag="g")
            nc.gpsimd.dma_gather(
                gb[:, :ntl, :],
                table[g * cfg.PAIR_CHUNK:(g + 1) * cfg.PAIR_CHUNK, :],
                ib[:, : ntl * 8],
                nid, nid, TBLW,
                single_packet=False,
                queue_num=q_counter[0] % NQ,
            )
            q_counter[0] += 1
            st.update(ch=ch, gb=gb, sb=sb, t0=t0)
            return st

        # pre-warm the first gather round before the big sxT DMA queues up
        for g in range(G):
            if plan["gT"][g]:
                ensure_chunk(g, 0)

        # ---- constants
        zrow = cpool.tile([1, WIN], bf16)
        nc.vector.memset(zrow[:, :], 0.0)
        Wd_sb = cpool.tile([C, C], bf16)
        nc.sync.dma_start(out=Wd_sb[:, :], in_=Wd[:, :])
        bb = cpool.tile([128, C], f32)
        nc.sync.dma_start(out=bb[:, :], in_=bb_dr[:, :])
        bd_col = cpool.tile([C, 1], f32)
        nc.sync.dma_start(out=bd_col[:, :], in_=bdc[:, :])
        lw_sb = cpool.tile([C, 1], bf16)
        nc.sync.dma_start(out=lw_sb[:, :], in_=lw[:, :])
        lb_sb = cpool.tile([1, 1], f32)
        nc.sync.dma_start(out=lb_sb[:, :], in_=lb[:, :])
        nlb = cpool.tile([1, 1], f32)
        nc.scalar.mul(nlb[:, :], lb_sb[:, :], -1.0)

        zT = zpool.tile([C, NLOC], f32)
        nc.sync.dma_start(out=zT[:, :], in_=sxT_dr[:, :])  # self-loop term
        zb = zpool.tile([C, NLOC], bf16)

        HB = WIN // 128

        def dense_for_window(w):
            wlen = min(WIN, NLOC - w * WIN)
            nchunks = (wlen + 127) // 128
            hb = hpool.tile([128, HB, C], bf16, tag="h")
            for kk in range(nchunks):
                k = w * HB + kk
                mrow = min(128, NLOC - k * 128)
                psd = pdpool.tile([128, C], f32)
                nc.tensor.matmul(psd[:mrow, :], lhsT=zb[:, k * 128:k * 128 + mrow],
                                 rhs=Wd_sb[:, :], start=True, stop=True)
                nc.vector.tensor_tensor(out=hb[:mrow, kk, :], in0=psd[:mrow, :],
                                        in1=bb[:mrow, :], op=mybir.AluOpType.add)
                nc.scalar.activation(hb[:mrow, kk, :], hb[:mrow, kk, :],
                                     mybir.ActivationFunctionType.Relu)
            r0, r1 = w * WIN, w * WIN + wlen
            nfull = (r1 - r0) // 128
            if nfull:
                dst = h_out[r0:r0 + nfull * 128, :].rearrange(
                    "(t p) c -> p t c", p=128)
                nc.sync.dma_start(out=dst, in_=hb[:, :nfull, :])
            rem = (r1 - r0) - nfull * 128
            if rem:
                nc.sync.dma_start(out=h_out[r0 + nfull * 128:r1, :],
                                  in_=hb[:rem, nfull, :])

        def head_for_window(w):
            wlen = min(WIN, NLOC - w * WIN)
            pst = ptpool.tile([C, WIN], f32)
            nc.tensor.matmul(pst[:, :wlen], lhsT=Wd_sb[:, :],
                             rhs=zb[:, w * WIN:w * WIN + wlen],
                             start=True, stop=True)
            ht = htpool.tile([C, WIN], bf16, tag="ht")
            nc.scalar.activation(ht[:, :wlen], pst[:, :wlen],
                                 mybir.ActivationFunctionType.Relu, bias=bd_col[:, :])
            psl = plpool.tile([1, WIN], f32)
            nc.tensor.matmul(psl[:, :wlen], lhsT=lw_sb[:, :], rhs=ht[:, :wlen],
                             start=True, stop=True)
            otn = opool.tile([1, WIN], f32, tag="otn")
            otp = opool.tile([1, WIN], f32, tag="otp")
            nc.scalar.activation(otn[:, :wlen], psl[:, :wlen],
                                 mybir.ActivationFunctionType.Identity,
                                 bias=nlb[:, :], scale=-1.0)
            nc.scalar.activation(otp[:, :wlen], psl[:, :wlen],
                                 mybir.ActivationFunctionType.Identity,
                                 bias=lb_sb[:, :], scale=1.0)
            nc.sync.dma_start(out=outT[0:1, w * WIN:w * WIN + wlen], in_=otn[:, :wlen])
            nc.sync.dma_start(out=outT[1:2, w * WIN:w * WIN + wlen], in_=otp[:, :wlen])

        for w in range(cfg.NWIN):
            wlen = min(WIN, NLOC - w * WIN)
            ps = ppool.tile([C, WIN], f32)
            nc.tensor.matmul(ps[:, :wlen], lhsT=zrow[:, :C], rhs=zrow[:, :wlen],
                             start=True, stop=False)
            for g, t, off, weff in win_tiles[w]:
                st = ensure_chunk(g, int(tile_chunk[t]))
                tp = t - st["t0"]
                for hh in range(2):
                    nc.tensor.matmul(
                        ps[:, off:off + weff],
                        lhsT=st["gb"][:, tp, hh * C:hh * C + C],
                        rhs=st["sb"][:, tp, hh, :weff],
                        start=False, stop=False,
                        skip_group_check=True,
                    )
            nc.tensor.matmul(ps[:, :wlen], lhsT=zrow[:, :C], rhs=zrow[:, :wlen],
                             start=False, stop=True)
            zw = zT[:, w * WIN:w * WIN + wlen]
            nc.vector.tensor_tensor(out=zw, in0=ps[:, :wlen], in1=zw,
                                    op=mybir.AluOpType.add)
            nc.scalar.copy(out=zb[:, w * WIN:w * WIN + wlen], in_=zw)
            if _stage in ("all", "dense"):
                dense_for_window(w)
            if _stage in ("all", "head"):
                head_for_window(w)

    nc.compile()
    return nc


# ------------------------------------------------------------------ runner ---
def make_runner(nc, device):
    """Single-core jit runner pinned to one device, reusable across calls."""
    import jax
    import concourse.mybir as mybir
    from concourse import bass2jax

    bass2jax.install_neuronx_cc_hook()

    in_names, out_names, out_avals, zero_shapes = [], [], [], []
    for alloc in nc.m.functions[0].allocations:
        if not isinstance(alloc, mybir.MemoryLocationSet):
            continue
        nm = alloc.memorylocations[0].name
        if alloc.kind == "ExternalInput":
            in_names.append(nm)
        elif alloc.kind == "ExternalOutput":
            shape = tuple(alloc.tensor_shape)
            dtype = mybir.dt.np(alloc.dtype)
            out_names.append(nm)
            out_avals.append(jax.core.ShapedArray(shape, dtype))
            zero_shapes.append((shape, dtype))
    n_params = len(in_names)
    all_in_names = in_names + out_names
    donate = tuple(range(n_params, n_params + len(out_names)))

    def _body(*args):
        outs = bass2jax._bass_exec_p.bind(
            *args,
            out_avals=tuple(out_avals),
            in_names=tuple(all_in_names),
            out_names=tuple(out_names),
            lowering_input_output_aliases=(),
            sim_require_finite=True,
            sim_require_nnan=True,
            nc=nc,
        )
        return tuple(outs)

    jitted = jax.jit(_body, donate_argnums=donate, keep_unused=True)

    def run(in_map):
        args = [jax.device_put(np.asarray(in_map[nm]), device) for nm in in_names]
        zeros = [jax.device_put(np.zeros(s, d), device) for s, d in zero_shapes]
        outs = jitted(*args, *zeros)
        return {nm: outs[i] for i, nm in enumerate(out_names)}

    return run


# ---------------------------------------------------------------- kernel() ---
_CACHE = {}


def _get_runners(plans, cfg):
    import jax
    key = "runners"
    if key in _CACHE:
        return _CACHE[key]
    devices = jax.devices()[:cfg.P]
    ncs = [build_program(plans[d], cfg, name=f"gnn_d{d}") for d in range(cfg.P)]
    runners = [make_runner(ncs[d], devices[d]) for d in range(cfg.P)]
    _CACHE[key] = runners
    return runners


def run_two_phase(inputs, cfg=FULL):
    import jax
    from concurrent.futures import ThreadPoolExecutor

    x = np.asarray(inputs["x"], np.float32)
    W1 = np.asarray(inputs["W1"], np.float32)
    b1 = np.asarray(inputs["b1"], np.float32)
    W2 = np.asarray(inputs["W2"], np.float32)
    b2 = np.asarray(inputs["b2"], np.float32)
    lin_w = np.asarray(inputs["lin_w"], np.float32)
    lin_b = np.asarray(inputs["lin_b"], np.float32)
    C, H2 = cfg.C, cfg.H2

    plans, dis = preprocess(inputs["edge_index"], inputs["edge_logits"], cfg)
    dis2 = (dis * dis).astype(np.float32)
    runners = _get_runners(plans, cfg)

    W2p = np.zeros((C, C), np.float32)
    W2p[:, :H2] = W2
    b2p = np.zeros(C, np.float32)
    b2p[:H2] = b2
    lwp = np.zeros((C, 1), np.float32)
    lwp[:H2, 0] = lin_w[:, 0]
    lbp = lin_b.reshape(1, 1)
    zconst = np.zeros((C, 1), BF16)

    def pair_table(d, f_bf16):
        return np.ascontiguousarray(
            f_bf16[plans[d]["perm"]].reshape(cfg.NPAIR, cfg.TBLW))

    def phase_inputs(d, tbl_bf16, sx32, Wdv, bdv, lwv, lbv):
        sh = slice(d * cfg.NLOC, (d + 1) * cfg.NLOC)
        sxT = np.ascontiguousarray((sx32[sh] * dis2[sh, None]).T)
        m = dict(table=pair_table(d, tbl_bf16), sxT=sxT, Wd=Wdv.astype(BF16),
                 bb=np.tile(bdv, (128, 1)).astype(np.float32),
                 bdc=bdv.reshape(C, 1).astype(np.float32),
                 lw=lwv.astype(BF16), lb=lbv.astype(np.float32),
                 idx=plans[d]["idx"], S=plans[d]["S"])
        return m

    x_bf16 = x.astype(BF16)

    # phase A: table=x pairs, dense=W1/b1 (head inputs zeroed)
    with ThreadPoolExecutor(cfg.P) as exe:
        resA = list(exe.map(
            lambda d: runners[d](phase_inputs(d, x_bf16, x, W1, b1, zconst,
                                              np.zeros((1, 1), np.float32))),
            range(cfg.P)))
    h_shards = [np.asarray(r["h_out"]) for r in resA]
    h1_bf16 = np.concatenate(h_shards, axis=0)          # [N, 64] bf16
    h1_f32 = h1_bf16.astype(np.float32)

    # phase B: table=h1 pairs, dense=padded W2/b2, head=lin
    with ThreadPoolExecutor(cfg.P) as exe:
        resB = list(exe.map(
            lambda d: runners[d](phase_inputs(d, h1_bf16, h1_f32, W2p, b2p, lwp, lbp)),
            range(cfg.P)))
    out = np.concatenate([np.asarray(r["outT"]).T for r in resB], axis=0)
    return out.astype(np.float32)


def kernel(x, edge_index, edge_logits, W1, b1, W2, b2, lin_w, lin_b):
    inputs = dict(x=x, edge_index=edge_index, edge_logits=edge_logits,
                  W1=W1, b1=b1, W2=W2, b2=b2, lin_w=lin_w, lin_b=lin_b)
    return run_two_phase(inputs, FULL)


# revision 8
# speedup vs baseline: 3.6137x; 1.0202x over previous
"""Trainium2 Bass kernel for a 2-layer edge-gated GCN (DiffGNNPlacement).

Math (reference, per layer):
    ew   = 0.5 + sigmoid(edge_logits)                  # [E]
    deg  = segsum(ew -> col) + 1                       # [N]
    dis  = deg^-1/2
    norm = dis[row] * ew * dis[col]                    # [E]
    out  = segsum(norm * (h@W)[row] -> col) + (h@W)*dis^2 + b

Key transform: aggregation commutes with the (linear) feature transform, so
    out = (segsum(norm * h[row] -> col) + h*dis^2) @ W + b
and the self-loop term is folded in host-side via the sxT input.

Device algorithm (per core, nodes sharded 12500/core):
  - SWDGE descriptor generation is the bottleneck (~9.5ns/descriptor): every
    gathered row costs one descriptor, so nodes are PAIRED per core (greedy
    matching of sources that co-occur in the same 16-target-col bucket) and
    the per-core feature table stores one 256B bf16 row per PAIR. One
    descriptor then feeds up to two edges.
  - edges partitioned by target shard, sorted by target col; slots are
    (pair, bucket) units; tiles pack 128 slots spanning <=32 target cols.
  - per tile: dma_gather 128 pair-rows -> SBUF [128, 128] bf16; host-built
    Su/Sv [128, 32] bf16 (norms of the u-half / v-half edges); two PE
    matmuls psum[64, off:off+w] += gathered[:, h*64:h*64+64]^T @ S_h
    accumulate the aggregation z^T for a 512-col PSUM window.
  - gathers spread over all 4 SWDGE queues (queue q = Q7 cores 2q/2q+1),
    queue_num = issue_index % 4 so the Tile DMASW lane (issue % 8) is fed by
    a single queue (completions stay FIFO per lane -> race-free). Early
    chunks are small for fast pipeline ramp.
  - dense h = relu(z @ W + b) and the classifier head are interleaved into
    the window loop (run in PE stall gaps while descriptors generate).

The same compiled program serves both layers; it is launched twice per core
with a host re-pairing of h1 shards in between.
"""

import os
import sys
import math
import numpy as np
import ml_dtypes
from contextlib import ExitStack

for _p in ("/opt/trn_rl_repo", "/root/.axon_site/_ro/trn_rl_repo"):
    if os.path.isdir(_p) and _p not in sys.path:
        sys.path.insert(0, _p)

BF16 = ml_dtypes.bfloat16


# ----------------------------------------------------------------- config ---
class Cfg:
    def __init__(self, N=100000, E=1600000, C=64, H2=32, P=8,
                 PAIR_CHUNK=25000, B=16, W=32, WIN=512, TCH=32, TBLW=128):
        self.N, self.E, self.C, self.H2, self.P = N, E, C, H2, P
        self.NLOC = N // P
        self.NPAIR = N // 2
        self.PAIR_CHUNK = PAIR_CHUNK          # pairs per gather group
        self.NGRP = (self.NPAIR + PAIR_CHUNK - 1) // PAIR_CHUNK
        self.B = B            # pairing bucket (target cols)
        self.W = W            # S tile width (target-col window per tile)
        self.WIN = WIN        # PSUM accumulation window (cols)
        self.TCH = TCH        # steady-state tiles per gather chunk
        self.RAMP = (6, 12, 24)  # tile counts for the first chunks
        self.TBLW = TBLW      # pair row width (bf16 elems; 256B rows)
        self.NWIN = (self.NLOC + WIN - 1) // WIN
        assert PAIR_CHUNK <= 32767
        assert TBLW * 2 == 256  # dma_gather elem constraint (256B rows)
        assert 2 * B <= W and WIN % B == 0


FULL = Cfg()


# --------------------------------------------------------- host preprocess ---
def _sigmoid(x):
    return 0.5 * (np.tanh(0.5 * x) + 1.0)


def _pair_nodes(r, c, cfg):
    """Greedy per-core pairing: sort sources by the first B-col bucket they
    appear in, pair adjacent. Returns pairs [NPAIR, 2] (a permutation of all
    nodes)."""
    N, B = cfg.N, cfg.B
    bucket = c // B
    o = np.lexsort((bucket, r))
    rs, bs = r[o], bucket[o]
    first = np.ones(len(rs), bool)
    if len(rs):
        first[1:] = rs[1:] != rs[:-1]
    src_f, buck_f = rs[first], bs[first]
    present = np.zeros(N, bool)
    present[src_f] = True
    absent = np.where(~present)[0]
    o3 = np.argsort(buck_f, kind="stable")
    allsrc = np.concatenate([src_f[o3], absent])
    pairs = allsrc.reshape(-1, 2)
    pairid = np.empty(N, np.int64)
    half = np.empty(N, np.int8)
    pairid[pairs[:, 0]] = np.arange(len(pairs))
    pairid[pairs[:, 1]] = np.arange(len(pairs))
    half[pairs[:, 0]] = 0
    half[pairs[:, 1]] = 1
    return pairs, pairid, half


def preprocess(edge_index, edge_logits, cfg=FULL):
    """Compute norms and per-device pairings + tile plans (pure numpy)."""
    N, NLOC = cfg.N, cfg.NLOC
    row = np.asarray(edge_index[0], dtype=np.int64)
    col = np.asarray(edge_index[1], dtype=np.int64)
    ew = (0.5 + _sigmoid(np.asarray(edge_logits, dtype=np.float32))).astype(np.float32)
    deg = np.bincount(col, weights=ew.astype(np.float64), minlength=N).astype(np.float32) + 1.0
    dis = deg ** -0.5
    norm = (dis[row] * ew * dis[col]).astype(np.float32)

    dev = col // NLOC
    order = np.argsort(dev, kind="stable")
    rs, cs, vs, ds = row[order], col[order] % NLOC, norm[order], dev[order]
    bounds = np.searchsorted(ds, np.arange(cfg.P + 1))
    plans = []
    for d in range(cfg.P):
        a, b = bounds[d], bounds[d + 1]
        plans.append(_plan_device(rs[a:b], cs[a:b], vs[a:b], cfg))
    return plans, dis


def _chunk_sizes(T, cfg):
    sizes = []
    for s in cfg.RAMP:
        if T - sum(sizes) <= 0:
            break
        sizes.append(min(s, T - sum(sizes)))
    rem = T - sum(sizes)
    while rem > 0:
        s = min(cfg.TCH, rem)
        sizes.append(s)
        rem -= s
    return sizes or [0]


def _plan_device(r, c, v, cfg):
    """Pair sources, build (pair, bucket) slots, pack 128-slot tiles."""
    B, W, WIN, G = cfg.B, cfg.W, cfg.WIN, cfg.NGRP
    pairs, pairid, half = _pair_nodes(r, c, cfg)
    perm = pairs.reshape(-1)                     # node order in the pair table

    p = pairid[r]
    h = half[r].astype(np.int64)
    grp = p // cfg.PAIR_CHUNK
    bucket = c // B

    # slots: distinct (grp, bucket, pair); edges sorted into slot-major order
    okey = ((grp * 800 + bucket) * np.int64(cfg.NPAIR)) + p
    o = np.argsort(okey, kind="stable")
    co, vo, ho, go, ko = c[o], v[o], h[o], grp[o], okey[o]
    po = p[o]
    m = len(ko)
    newslot = np.ones(max(m, 1), bool)
    if m:
        newslot[1:m] = ko[1:] != ko[:-1]
    slot_of_edge = np.cumsum(newslot[:m]) - 1 if m else np.zeros(0, np.int64)
    nslot = int(slot_of_edge[-1]) + 1 if m else 0
    slot_starts = np.where(newslot[:m])[0] if m else np.zeros(0, np.int64)
    slot_pair = po[slot_starts] if m else np.zeros(0, np.int64)
    slot_grp = go[slot_starts] if m else np.zeros(0, np.int64)
    slot_c0 = np.minimum.reduceat(co, slot_starts) if m else np.zeros(0, np.int64)
    slot_cmax = np.maximum.reduceat(co, slot_starts) if m else np.zeros(0, np.int64)

    # tiles: sequential pack per grp, <=128 slots, span < W, same WIN window
    tile_id = np.zeros(max(nslot, 1), np.int64)
    tile_c0s = []
    tile_grps = []
    t = -1
    cnt = 0
    cur_c0 = -10 ** 9
    cur_grp = -1
    for s in range(nslot):
        g = int(slot_grp[s])
        # align the tile base to the slot's bucket start: later slots in the
        # same bucket (ordered by pair id) may have smaller cols
        c0 = (int(slot_c0[s]) // B) * B
        cmax = int(slot_cmax[s])
        if (g != cur_grp or cnt >= 128 or cmax >= cur_c0 + W
                or (cmax // WIN) != (cur_c0 // WIN)):
            t += 1
            cnt = 0
            cur_c0 = c0
            cur_grp = g
            tile_c0s.append(c0)
            tile_grps.append(g)
        tile_id[s] = t
        cnt += 1
    T = t + 1 if nslot else 0
    tile_c0s = np.array(tile_c0s if T else [0], np.int32)
    tile_grp = np.array(tile_grps if T else [0], np.int64)
    tile_first_slot = np.searchsorted(tile_id[:max(nslot, 1)], np.arange(max(T, 1)))
    slot_in_tile = (np.arange(max(nslot, 1)) - tile_first_slot[tile_id]) if nslot else np.zeros(1, np.int64)

    Tm = max(T, 1)
    idx16 = np.zeros((Tm, 128), np.int16)
    if nslot:
        idx16[tile_id[:nslot], slot_in_tile[:nslot]] = (
            slot_pair - slot_grp * cfg.PAIR_CHUNK).astype(np.int16)
    S = np.zeros((Tm, 2, 128, cfg.W), np.float32)
    if m:
        e_tile = tile_id[slot_of_edge]
        e_slot = slot_in_tile[slot_of_edge]
        e_coff = co - tile_c0s[e_tile]
        np.add.at(S, (e_tile, ho, e_slot, e_coff), vo)

    # tile-major packing for the device
    wrapped = idx16.reshape(Tm, 8, 16).transpose(2, 0, 1)          # [16, T, 8]
    idx_w = np.ascontiguousarray(np.tile(wrapped.reshape(16, Tm * 8), (8, 1)))
    S_pk = np.ascontiguousarray(S.transpose(2, 0, 1, 3)).astype(BF16)  # [128, T, 2, W]

    # group tiles by grp for chunking (tiles are grp-ordered)
    gT = [int(np.sum(tile_grp[:T] == g)) for g in range(G)] if T else [0] * G
    gt0 = [int(np.searchsorted(tile_grp[:T], g)) for g in range(G)] if T else [0] * G
    gchunks = []
    for g in range(G):
        chunks = []
        t0 = gt0[g]
        for s in _chunk_sizes(gT[g], cfg):
            chunks.append((t0, s))
            t0 += s
        gchunks.append(chunks)

    win = tile_c0s // WIN
    off = tile_c0s - win * WIN
    return dict(T=T, idx=idx_w, S=S_pk, perm=perm, gchunks=gchunks,
                gT=gT, gt0=gt0, win=win, off=off, tile_grp=tile_grp)


# ---------------------------------------------------------- program builder ---
def build_program(plan, cfg=FULL, name="gnn"):
    import concourse.bass as bass
    import concourse.mybir as mybir
    from concourse import bacc
    from concourse.tile import TileContext

    f32, i16, bf16 = mybir.dt.float32, mybir.dt.int16, mybir.dt.bfloat16
    C, W, WIN, NLOC = cfg.C, cfg.W, cfg.WIN, cfg.NLOC
    TBLW = cfg.TBLW
    G = cfg.NGRP
    NQ = 4  # SWDGE queues
    TCH = cfg.TCH

    nc = bacc.Bacc("TRN2", enable_partition_id=False,
                   target_bir_lowering=False, name=name,
                   num_swdge_queues=NQ)

    table = nc.dram_tensor("table", [cfg.NPAIR, TBLW], bf16, kind="ExternalInput")
    sxT_dr = nc.dram_tensor("sxT", [C, NLOC], f32, kind="ExternalInput")
    Wd = nc.dram_tensor("Wd", [C, C], bf16, kind="ExternalInput")
    bb_dr = nc.dram_tensor("bb", [128, C], f32, kind="ExternalInput")
    bdc = nc.dram_tensor("bdc", [C, 1], f32, kind="ExternalInput")
    lw = nc.dram_tensor("lw", [C, 1], bf16, kind="ExternalInput")
    lb = nc.dram_tensor("lb", [1, 1], f32, kind="ExternalInput")
    idx_dr = nc.dram_tensor("idx", list(plan["idx"].shape), i16, kind="ExternalInput")
    S_dr = nc.dram_tensor("S", list(plan["S"].shape), bf16, kind="ExternalInput")
    h_out = nc.dram_tensor("h_out", [NLOC, C], bf16, kind="ExternalOutput")
    outT = nc.dram_tensor("outT", [2, NLOC], f32, kind="ExternalOutput")

    # per-window tile lists: (g, t, off, weff)
    win_tiles = [[] for _ in range(cfg.NWIN)]
    for t in range(plan["T"]):
        w = int(plan["win"][t])
        off = int(plan["off"][t])
        wlen = min(WIN, NLOC - w * WIN)
        weff = min(W, wlen - off)
        win_tiles[w].append((int(plan["tile_grp"][t]), t, off, weff))

    # tile -> chunk per group
    tile_chunk = np.zeros(max(plan["T"], 1), np.int64)
    for g in range(G):
        for ci, (t0, ntl) in enumerate(plan["gchunks"][g]):
            tile_chunk[t0:t0 + ntl] = ci

    _stage = os.environ.get("GNN_STAGE", "all")

    with TileContext(nc) as tc, ExitStack() as ex:
        cpool = ex.enter_context(tc.tile_pool(name="consts", bufs=1))
        zpool = ex.enter_context(tc.tile_pool(name="z", bufs=1))
        gpools = [ex.enter_context(tc.tile_pool(name=f"gat{g}", bufs=4)) for g in range(G)]
        ipools = [ex.enter_context(tc.tile_pool(name=f"idx{g}", bufs=4)) for g in range(G)]
        spools = [ex.enter_context(tc.tile_pool(name=f"s{g}", bufs=4)) for g in range(G)]
        ppool = ex.enter_context(tc.tile_pool(name="psagg", bufs=2, space="PSUM"))
        pdpool = ex.enter_context(tc.tile_pool(name="psd", bufs=2, space="PSUM"))
        ptpool = ex.enter_context(tc.tile_pool(name="pst", bufs=2, space="PSUM"))
        plpool = ex.enter_context(tc.tile_pool(name="psl", bufs=2, space="PSUM"))
        hpool = ex.enter_context(tc.tile_pool(name="hrows", bufs=2))
        htpool = ex.enter_context(tc.tile_pool(name="ht", bufs=2))
        opool = ex.enter_context(tc.tile_pool(name="ot", bufs=2))

        # ---- aggregation chunk loader
        cur = [dict(ch=-1, gb=None, sb=None, t0=0) for _ in range(G)]
        q_counter = [0]

        def ensure_chunk(g, ch):
            st = cur[g]
            if st["ch"] == ch:
                return st
            t0, ntl = plan["gchunks"][g][ch]
            nid = ntl * 128
            ib = ipools[g].tile([128, TCH * 8], i16, tag="idx")
            nc.sync.dma_start(out=ib[:, : ntl * 8], in_=idx_dr[:, t0 * 8:(t0 + ntl) * 8])
            sb = spools[g].tile([128, TCH, 2, W], bf16, tag="s")
            nc.scalar.dma_start(out=sb[:, :ntl, :, :], in_=S_dr[:, t0:t0 + ntl, :, :])
            gb = gpools[g].tile([128, TCH, TBLW], bf16, tag="g")
            nc.gpsimd.dma_gather(
                gb[:, :ntl, :],
                table[g * cfg.PAIR_CHUNK:(g + 1) * cfg.PAIR_CHUNK, :],
                ib[:, : ntl * 8],
                nid, nid, TBLW,
                single_packet=False,
                queue_num=q_counter[0] % NQ,
            )
            q_counter[0] += 1
            st.update(ch=ch, gb=gb, sb=sb, t0=t0)
            return st

        # pre-warm the first gather round before the big sxT DMA queues up
        for g in range(G):
            if plan["gT"][g]:
                ensure_chunk(g, 0)

        # ---- constants
        zrow = cpool.tile([1, WIN], bf16)
        nc.vector.memset(zrow[:, :], 0.0)
        Wd_sb = cpool.tile([C, C], bf16)
        nc.sync.dma_start(out=Wd_sb[:, :], in_=Wd[:, :])
        bb = cpool.tile([128, C], f32)
        nc.sync.dma_start(out=bb[:, :], in_=bb_dr[:, :])
        bd_col = cpool.tile([C, 1], f32)
        nc.sync.dma_start(out=bd_col[:, :], in_=bdc[:, :])
        lw_sb = cpool.tile([C, 1], bf16)
        nc.sync.dma_start(out=lw_sb[:, :], in_=lw[:, :])
        lb_sb = cpool.tile([1, 1], f32)
        nc.sync.dma_start(out=lb_sb[:, :], in_=lb[:, :])
        nlb = cpool.tile([1, 1], f32)
        nc.scalar.mul(nlb[:, :], lb_sb[:, :], -1.0)

        zT = zpool.tile([C, NLOC], f32)
        nc.sync.dma_start(out=zT[:, :], in_=sxT_dr[:, :])  # self-loop term
        zb = zpool.tile([C, NLOC], bf16)

        HB = WIN // 128

        def dense_for_window(w):
            wlen = min(WIN, NLOC - w * WIN)
            nchunks = (wlen + 127) // 128
            hb = hpool.tile([128, HB, C], bf16, tag="h")
            for kk in range(nchunks):
                k = w * HB + kk
                mrow = min(128, NLOC - k * 128)
                psd = pdpool.tile([128, C], f32)
                nc.tensor.matmul(psd[:mrow, :], lhsT=zb[:, k * 128:k * 128 + mrow],
                                 rhs=Wd_sb[:, :], start=True, stop=True)
                nc.vector.tensor_tensor(out=hb[:mrow, kk, :], in0=psd[:mrow, :],
                                        in1=bb[:mrow, :], op=mybir.AluOpType.add)
                nc.scalar.activation(hb[:mrow, kk, :], hb[:mrow, kk, :],
                                     mybir.ActivationFunctionType.Relu)
            r0, r1 = w * WIN, w * WIN + wlen
            nfull = (r1 - r0) // 128
            if nfull:
                dst = h_out[r0:r0 + nfull * 128, :].rearrange(
                    "(t p) c -> p t c", p=128)
                nc.sync.dma_start(out=dst, in_=hb[:, :nfull, :])
            rem = (r1 - r0) - nfull * 128
            if rem:
                nc.sync.dma_start(out=h_out[r0 + nfull * 128:r1, :],
                                  in_=hb[:rem, nfull, :])

        def head_for_window(w):
            wlen = min(WIN, NLOC - w * WIN)
            pst = ptpool.tile([C, WIN], f32)
            nc.tensor.matmul(pst[:, :wlen], lhsT=Wd_sb[:, :],
                             rhs=zb[:, w * WIN:w * WIN + wlen],
                             start=True, stop=True)
            ht = htpool.tile([C, WIN], bf16, tag="ht")
            nc.scalar.activation(ht[:, :wlen], pst[:, :wlen],
                                 mybir.ActivationFunctionType.Relu, bias=bd_col[:, :])
            psl = plpool.tile([1, WIN], f32)
            nc.tensor.matmul(psl[:, :wlen], lhsT=lw_sb[:, :], rhs=ht[:, :wlen],
                             start=True, stop=True)
            otn = opool.tile([1, WIN], f32, tag="otn")
            otp = opool.tile([1, WIN], f32, tag="otp")
            nc.scalar.activation(otn[:, :wlen], psl[:, :wlen],
                                 mybir.ActivationFunctionType.Identity,
                                 bias=nlb[:, :], scale=-1.0)
            nc.scalar.activation(otp[:, :wlen], psl[:, :wlen],
                                 mybir.ActivationFunctionType.Identity,
                                 bias=lb_sb[:, :], scale=1.0)
            nc.sync.dma_start(out=outT[0:1, w * WIN:w * WIN + wlen], in_=otn[:, :wlen])
            nc.sync.dma_start(out=outT[1:2, w * WIN:w * WIN + wlen], in_=otp[:, :wlen])

        for w in range(cfg.NWIN):
            wlen = min(WIN, NLOC - w * WIN)
            ps = ppool.tile([C, WIN], f32)
            nc.tensor.matmul(ps[:, :wlen], lhsT=zrow[:, :C], rhs=zrow[:, :wlen],
                             start=True, stop=False)
            for g, t, off, weff in win_tiles[w]:
                st = ensure_chunk(g, int(tile_chunk[t]))
                tp = t - st["t0"]
                for hh in range(2):
                    nc.tensor.matmul(
                        ps[:, off:off + weff],
                        lhsT=st["gb"][:, tp, hh * C:hh * C + C],
                        rhs=st["sb"][:, tp, hh, :weff],
                        start=False, stop=False,
                        skip_group_check=True,
                    )
            nc.tensor.matmul(ps[:, :wlen], lhsT=zrow[:, :C], rhs=zrow[:, :wlen],
                             start=False, stop=True)
            zw = zT[:, w * WIN:w * WIN + wlen]
            nc.vector.tensor_tensor(out=zw, in0=ps[:, :wlen], in1=zw,
                                    op=mybir.AluOpType.add)
            nc.scalar.copy(out=zb[:, w * WIN:w * WIN + wlen], in_=zw)
            if _stage in ("all", "dense"):
                dense_for_window(w)
            if _stage in ("all", "head"):
                head_for_window(w)

    nc.compile()
    return nc


# ------------------------------------------------------------------ runner ---
def make_runner(nc, device):
    """Single-core jit runner pinned to one device, reusable across calls."""
    import jax
    import concourse.mybir as mybir
    from concourse import bass2jax

    bass2jax.install_neuronx_cc_hook()

    in_names, out_names, out_avals, zero_shapes = [], [], [], []
    for alloc in nc.m.functions[0].allocations:
        if not isinstance(alloc, mybir.MemoryLocationSet):
            continue
        nm = alloc.memorylocations[0].name
        if alloc.kind == "ExternalInput":
            in_names.append(nm)
        elif alloc.kind == "ExternalOutput":
            shape = tuple(alloc.tensor_shape)
            dtype = mybir.dt.np(alloc.dtype)
            out_names.append(nm)
            out_avals.append(jax.core.ShapedArray(shape, dtype))
            zero_shapes.append((shape, dtype))
    n_params = len(in_names)
    all_in_names = in_names + out_names
    donate = tuple(range(n_params, n_params + len(out_names)))

    def _body(*args):
        outs = bass2jax._bass_exec_p.bind(
            *args,
            out_avals=tuple(out_avals),
            in_names=tuple(all_in_names),
            out_names=tuple(out_names),
            lowering_input_output_aliases=(),
            sim_require_finite=True,
            sim_require_nnan=True,
            nc=nc,
        )
        return tuple(outs)

    jitted = jax.jit(_body, donate_argnums=donate, keep_unused=True)

    def run(in_map):
        args = [jax.device_put(np.asarray(in_map[nm]), device) for nm in in_names]
        zeros = [jax.device_put(np.zeros(s, d), device) for s, d in zero_shapes]
        outs = jitted(*args, *zeros)
        return {nm: outs[i] for i, nm in enumerate(out_names)}

    return run


# ---------------------------------------------------------------- kernel() ---
_CACHE = {}


def _get_runners(plans, cfg):
    import jax
    key = "runners"
    if key in _CACHE:
        return _CACHE[key]
    devices = jax.devices()[:cfg.P]
    ncs = [build_program(plans[d], cfg, name=f"gnn_d{d}") for d in range(cfg.P)]
    runners = [make_runner(ncs[d], devices[d]) for d in range(cfg.P)]
    _CACHE[key] = runners
    return runners


def run_two_phase(inputs, cfg=FULL):
    import jax
    from concurrent.futures import ThreadPoolExecutor

    x = np.asarray(inputs["x"], np.float32)
    W1 = np.asarray(inputs["W1"], np.float32)
    b1 = np.asarray(inputs["b1"], np.float32)
    W2 = np.asarray(inputs["W2"], np.float32)
    b2 = np.asarray(inputs["b2"], np.float32)
    lin_w = np.asarray(inputs["lin_w"], np.float32)
    lin_b = np.asarray(inputs["lin_b"], np.float32)
    C, H2 = cfg.C, cfg.H2

    plans, dis = preprocess(inputs["edge_index"], inputs["edge_logits"], cfg)
    dis2 = (dis * dis).astype(np.float32)
    runners = _get_runners(plans, cfg)

    W2p = np.zeros((C, C), np.float32)
    W2p[:, :H2] = W2
    b2p = np.zeros(C, np.float32)
    b2p[:H2] = b2
    lwp = np.zeros((C, 1), np.float32)
    lwp[:H2, 0] = lin_w[:, 0]
    lbp = lin_b.reshape(1, 1)
    zconst = np.zeros((C, 1), BF16)

    def pair_table(d, f_bf16):
        return np.ascontiguousarray(
            f_bf16[plans[d]["perm"]].reshape(cfg.NPAIR, cfg.TBLW))

    def phase_inputs(d, tbl_bf16, sx32, Wdv, bdv, lwv, lbv):
        sh = slice(d * cfg.NLOC, (d + 1) * cfg.NLOC)
        sxT = np.ascontiguousarray((sx32[sh] * dis2[sh, None]).T)
        m = dict(table=pair_table(d, tbl_bf16), sxT=sxT, Wd=Wdv.astype(BF16),
                 bb=np.tile(bdv, (128, 1)).astype(np.float32),
                 bdc=bdv.reshape(C, 1).astype(np.float32),
                 lw=lwv.astype(BF16), lb=lbv.astype(np.float32),
                 idx=plans[d]["idx"], S=plans[d]["S"])
        return m

    x_bf16 = x.astype(BF16)

    # phase A: table=x pairs, dense=W1/b1 (head inputs zeroed)
    with ThreadPoolExecutor(cfg.P) as exe:
        resA = list(exe.map(
            lambda d: runners[d](phase_inputs(d, x_bf16, x, W1, b1, zconst,
                                              np.zeros((1, 1), np.float32))),
            range(cfg.P)))
    h_shards = [np.asarray(r["h_out"]) for r in resA]
    h1_bf16 = np.concatenate(h_shards, axis=0)          # [N, 64] bf16
    h1_f32 = h1_bf16.astype(np.float32)

    # phase B: table=h1 pairs, dense=padded W2/b2, head=lin
    with ThreadPoolExecutor(cfg.P) as exe:
        resB = list(exe.map(
            lambda d: runners[d](phase_inputs(d, h1_bf16, h1_f32, W2p, b2p, lwp, lbp)),
            range(cfg.P)))
    out = np.concatenate([np.asarray(r["outT"]).T for r in resB], axis=0)
    return out.astype(np.float32)


def kernel(x, edge_index, edge_logits, W1, b1, W2, b2, lin_w, lin_b):
    inputs = dict(x=x, edge_index=edge_index, edge_logits=edge_logits,
                  W1=W1, b1=b1, W2=W2, b2=b2, lin_w=lin_w, lin_b=lin_b)
    return run_two_phase(inputs, FULL)


# revision 9
# speedup vs baseline: 3.6179x; 1.0012x over previous
"""Trainium2 Bass kernel for a 2-layer edge-gated GCN (DiffGNNPlacement).

Math (reference, per layer):
    ew   = 0.5 + sigmoid(edge_logits)                  # [E]
    deg  = segsum(ew -> col) + 1                       # [N]
    dis  = deg^-1/2
    norm = dis[row] * ew * dis[col]                    # [E]
    out  = segsum(norm * (h@W)[row] -> col) + (h@W)*dis^2 + b

Key transform: aggregation commutes with the (linear) feature transform, so
    out = (segsum(norm * h[row] -> col) + h*dis^2) @ W + b
and the self-loop term is folded in host-side via the sxT input.

Device algorithm (per core, nodes sharded 12500/core):
  - SWDGE descriptor generation is the bottleneck (~9.5ns/descriptor): every
    gathered row costs one descriptor, so nodes are PAIRED per core (greedy
    matching of sources that co-occur in the same 16-target-col bucket) and
    the per-core feature table stores one 256B bf16 row per PAIR. One
    descriptor then feeds up to two edges.
  - edges partitioned by target shard, sorted by target col; slots are
    (pair, bucket) units; tiles pack 128 slots spanning <=32 target cols.
  - per tile: dma_gather 128 pair-rows -> SBUF [128, 128] bf16; host-built
    Su/Sv [128, 32] bf16 (norms of the u-half / v-half edges); two PE
    matmuls psum[64, off:off+w] += gathered[:, h*64:h*64+64]^T @ S_h
    accumulate the aggregation z^T for a 512-col PSUM window.
  - gathers spread over all 4 SWDGE queues (queue q = Q7 cores 2q/2q+1),
    queue_num = issue_index % 4 so the Tile DMASW lane (issue % 8) is fed by
    a single queue (completions stay FIFO per lane -> race-free). Early
    chunks are small for fast pipeline ramp.
  - dense h = relu(z @ W + b) and the classifier head are interleaved into
    the window loop (run in PE stall gaps while descriptors generate).

The same compiled program serves both layers; it is launched twice per core
with a host re-pairing of h1 shards in between.
"""

import os
import sys
import math
import numpy as np
import ml_dtypes
from contextlib import ExitStack

for _p in ("/opt/trn_rl_repo", "/root/.axon_site/_ro/trn_rl_repo"):
    if os.path.isdir(_p) and _p not in sys.path:
        sys.path.insert(0, _p)

BF16 = ml_dtypes.bfloat16


# ----------------------------------------------------------------- config ---
class Cfg:
    def __init__(self, N=100000, E=1600000, C=64, H2=32, P=8,
                 PAIR_CHUNK=25000, B=16, W=32, WIN=512, TCH=32, TBLW=128):
        self.N, self.E, self.C, self.H2, self.P = N, E, C, H2, P
        self.NLOC = N // P
        self.NPAIR = N // 2
        self.PAIR_CHUNK = PAIR_CHUNK          # pairs per gather group
        self.NGRP = (self.NPAIR + PAIR_CHUNK - 1) // PAIR_CHUNK
        self.B = B            # pairing bucket (target cols)
        self.W = W            # S tile width (target-col window per tile)
        self.WIN = WIN        # PSUM accumulation window (cols)
        self.TCH = TCH        # steady-state tiles per gather chunk
        self.RAMP = (6, 12, 24)  # tile counts for the first chunks
        self.TBLW = TBLW      # pair row width (bf16 elems; 256B rows)
        self.NWIN = (self.NLOC + WIN - 1) // WIN
        assert PAIR_CHUNK <= 32767
        assert TBLW * 2 == 256  # dma_gather elem constraint (256B rows)
        assert 2 * B <= W and WIN % B == 0


FULL = Cfg()


# --------------------------------------------------------- host preprocess ---
def _sigmoid(x):
    return 0.5 * (np.tanh(0.5 * x) + 1.0)


def _pair_nodes(r, c, cfg):
    """Greedy per-core pairing: sort sources by the first B-col bucket they
    appear in, pair adjacent. Returns pairs [NPAIR, 2] (a permutation of all
    nodes)."""
    N, B = cfg.N, cfg.B
    bucket = c // B
    o = np.lexsort((bucket, r))
    rs, bs = r[o], bucket[o]
    first = np.ones(len(rs), bool)
    if len(rs):
        first[1:] = rs[1:] != rs[:-1]
    src_f, buck_f = rs[first], bs[first]
    present = np.zeros(N, bool)
    present[src_f] = True
    absent = np.where(~present)[0]
    o3 = np.argsort(buck_f, kind="stable")
    allsrc = np.concatenate([src_f[o3], absent])
    pairs = allsrc.reshape(-1, 2)
    pairid = np.empty(N, np.int64)
    half = np.empty(N, np.int8)
    pairid[pairs[:, 0]] = np.arange(len(pairs))
    pairid[pairs[:, 1]] = np.arange(len(pairs))
    half[pairs[:, 0]] = 0
    half[pairs[:, 1]] = 1
    return pairs, pairid, half


def preprocess(edge_index, edge_logits, cfg=FULL):
    """Compute norms and per-device pairings + tile plans (pure numpy)."""
    N, NLOC = cfg.N, cfg.NLOC
    row = np.asarray(edge_index[0], dtype=np.int64)
    col = np.asarray(edge_index[1], dtype=np.int64)
    ew = (0.5 + _sigmoid(np.asarray(edge_logits, dtype=np.float32))).astype(np.float32)
    deg = np.bincount(col, weights=ew.astype(np.float64), minlength=N).astype(np.float32) + 1.0
    dis = deg ** -0.5
    norm = (dis[row] * ew * dis[col]).astype(np.float32)

    dev = col // NLOC
    order = np.argsort(dev, kind="stable")
    rs, cs, vs, ds = row[order], col[order] % NLOC, norm[order], dev[order]
    bounds = np.searchsorted(ds, np.arange(cfg.P + 1))
    plans = []
    for d in range(cfg.P):
        a, b = bounds[d], bounds[d + 1]
        plans.append(_plan_device(rs[a:b], cs[a:b], vs[a:b], cfg))
    return plans, dis


def _chunk_sizes(T, cfg):
    sizes = []
    for s in cfg.RAMP:
        if T - sum(sizes) <= 0:
            break
        sizes.append(min(s, T - sum(sizes)))
    rem = T - sum(sizes)
    while rem > 0:
        s = min(cfg.TCH, rem)
        sizes.append(s)
        rem -= s
    return sizes or [0]


def _plan_device(r, c, v, cfg):
    """Pair sources, build (pair, bucket) slots, pack 128-slot tiles."""
    B, W, WIN, G = cfg.B, cfg.W, cfg.WIN, cfg.NGRP
    pairs, pairid, half = _pair_nodes(r, c, cfg)
    perm = pairs.reshape(-1)                     # node order in the pair table

    p = pairid[r]
    h = half[r].astype(np.int64)
    grp = p // cfg.PAIR_CHUNK
    bucket = c // B

    # slots: distinct (grp, bucket, pair); edges sorted into slot-major order
    okey = ((grp * 800 + bucket) * np.int64(cfg.NPAIR)) + p
    o = np.argsort(okey, kind="stable")
    co, vo, ho, go, ko = c[o], v[o], h[o], grp[o], okey[o]
    po = p[o]
    m = len(ko)
    newslot = np.ones(max(m, 1), bool)
    if m:
        newslot[1:m] = ko[1:] != ko[:-1]
    slot_of_edge = np.cumsum(newslot[:m]) - 1 if m else np.zeros(0, np.int64)
    nslot = int(slot_of_edge[-1]) + 1 if m else 0
    slot_starts = np.where(newslot[:m])[0] if m else np.zeros(0, np.int64)
    slot_pair = po[slot_starts] if m else np.zeros(0, np.int64)
    slot_grp = go[slot_starts] if m else np.zeros(0, np.int64)
    slot_c0 = np.minimum.reduceat(co, slot_starts) if m else np.zeros(0, np.int64)
    slot_cmax = np.maximum.reduceat(co, slot_starts) if m else np.zeros(0, np.int64)

    # tiles: sequential pack per grp, <=128 slots, span < W, same WIN window
    tile_id = np.zeros(max(nslot, 1), np.int64)
    tile_c0s = []
    tile_grps = []
    t = -1
    cnt = 0
    cur_c0 = -10 ** 9
    cur_grp = -1
    for s in range(nslot):
        g = int(slot_grp[s])
        # align the tile base to the slot's bucket start: later slots in the
        # same bucket (ordered by pair id) may have smaller cols
        c0 = (int(slot_c0[s]) // B) * B
        cmax = int(slot_cmax[s])
        if (g != cur_grp or cnt >= 128 or cmax >= cur_c0 + W
                or (cmax // WIN) != (cur_c0 // WIN)):
            t += 1
            cnt = 0
            cur_c0 = c0
            cur_grp = g
            tile_c0s.append(c0)
            tile_grps.append(g)
        tile_id[s] = t
        cnt += 1
    T = t + 1 if nslot else 0
    tile_c0s = np.array(tile_c0s if T else [0], np.int32)
    tile_grp = np.array(tile_grps if T else [0], np.int64)
    tile_first_slot = np.searchsorted(tile_id[:max(nslot, 1)], np.arange(max(T, 1)))
    slot_in_tile = (np.arange(max(nslot, 1)) - tile_first_slot[tile_id]) if nslot else np.zeros(1, np.int64)

    Tm = max(T, 1)
    idx16 = np.zeros((Tm, 128), np.int16)
    if nslot:
        idx16[tile_id[:nslot], slot_in_tile[:nslot]] = (
            slot_pair - slot_grp * cfg.PAIR_CHUNK).astype(np.int16)
    S = np.zeros((Tm, 2, 128, cfg.W), np.float32)
    if m:
        e_tile = tile_id[slot_of_edge]
        e_slot = slot_in_tile[slot_of_edge]
        e_coff = co - tile_c0s[e_tile]
        np.add.at(S, (e_tile, ho, e_slot, e_coff), vo)

    # tile-major packing for the device
    wrapped = idx16.reshape(Tm, 8, 16).transpose(2, 0, 1)          # [16, T, 8]
    idx_w = np.ascontiguousarray(np.tile(wrapped.reshape(16, Tm * 8), (8, 1)))
    S_pk = np.ascontiguousarray(S.transpose(2, 0, 1, 3)).astype(BF16)  # [128, T, 2, W]

    # group tiles by grp for chunking (tiles are grp-ordered)
    gT = [int(np.sum(tile_grp[:T] == g)) for g in range(G)] if T else [0] * G
    gt0 = [int(np.searchsorted(tile_grp[:T], g)) for g in range(G)] if T else [0] * G
    gchunks = []
    for g in range(G):
        chunks = []
        t0 = gt0[g]
        for s in _chunk_sizes(gT[g], cfg):
            chunks.append((t0, s))
            t0 += s
        gchunks.append(chunks)

    win = tile_c0s // WIN
    off = tile_c0s - win * WIN
    return dict(T=T, idx=idx_w, S=S_pk, perm=perm, gchunks=gchunks,
                gT=gT, gt0=gt0, win=win, off=off, tile_grp=tile_grp)


# ---------------------------------------------------------- program builder ---
def build_program(plan, cfg=FULL, name="gnn"):
    import concourse.bass as bass
    import concourse.mybir as mybir
    from concourse import bacc
    from concourse.tile import TileContext

    f32, i16, bf16 = mybir.dt.float32, mybir.dt.int16, mybir.dt.bfloat16
    C, W, WIN, NLOC = cfg.C, cfg.W, cfg.WIN, cfg.NLOC
    TBLW = cfg.TBLW
    G = cfg.NGRP
    NQ = 4  # SWDGE queues
    TCH = cfg.TCH

    nc = bacc.Bacc("TRN2", enable_partition_id=False,
                   target_bir_lowering=False, name=name,
                   num_swdge_queues=NQ)

    table = nc.dram_tensor("table", [cfg.NPAIR, TBLW], bf16, kind="ExternalInput")
    sxT_dr = nc.dram_tensor("sxT", [C, NLOC], f32, kind="ExternalInput")
    Wd = nc.dram_tensor("Wd", [C, C], bf16, kind="ExternalInput")
    bb_dr = nc.dram_tensor("bb", [128, C], f32, kind="ExternalInput")
    bdc = nc.dram_tensor("bdc", [C, 1], f32, kind="ExternalInput")
    lw = nc.dram_tensor("lw", [C, 1], bf16, kind="ExternalInput")
    lb = nc.dram_tensor("lb", [1, 1], f32, kind="ExternalInput")
    idx_dr = nc.dram_tensor("idx", list(plan["idx"].shape), i16, kind="ExternalInput")
    S_dr = nc.dram_tensor("S", list(plan["S"].shape), bf16, kind="ExternalInput")
    h_out = nc.dram_tensor("h_out", [NLOC, C], bf16, kind="ExternalOutput")
    outT = nc.dram_tensor("outT", [2, NLOC], f32, kind="ExternalOutput")

    # per-window tile lists: (g, t, off, weff)
    win_tiles = [[] for _ in range(cfg.NWIN)]
    for t in range(plan["T"]):
        w = int(plan["win"][t])
        off = int(plan["off"][t])
        wlen = min(WIN, NLOC - w * WIN)
        weff = min(W, wlen - off)
        win_tiles[w].append((int(plan["tile_grp"][t]), t, off, weff))

    # tile -> chunk per group
    tile_chunk = np.zeros(max(plan["T"], 1), np.int64)
    for g in range(G):
        for ci, (t0, ntl) in enumerate(plan["gchunks"][g]):
            tile_chunk[t0:t0 + ntl] = ci

    _stage = os.environ.get("GNN_STAGE", "all")

    with TileContext(nc) as tc, ExitStack() as ex:
        cpool = ex.enter_context(tc.tile_pool(name="consts", bufs=1))
        zpool = ex.enter_context(tc.tile_pool(name="z", bufs=1))
        gpools = [ex.enter_context(tc.tile_pool(name=f"gat{g}", bufs=4)) for g in range(G)]
        ipools = [ex.enter_context(tc.tile_pool(name=f"idx{g}", bufs=4)) for g in range(G)]
        spools = [ex.enter_context(tc.tile_pool(name=f"s{g}", bufs=4)) for g in range(G)]
        ppool = ex.enter_context(tc.tile_pool(name="psagg", bufs=2, space="PSUM"))
        pdpool = ex.enter_context(tc.tile_pool(name="psd", bufs=2, space="PSUM"))
        ptpool = ex.enter_context(tc.tile_pool(name="pst", bufs=2, space="PSUM"))
        plpool = ex.enter_context(tc.tile_pool(name="psl", bufs=2, space="PSUM"))
        hpool = ex.enter_context(tc.tile_pool(name="hrows", bufs=2))
        htpool = ex.enter_context(tc.tile_pool(name="ht", bufs=2))
        opool = ex.enter_context(tc.tile_pool(name="ot", bufs=2))

        # ---- aggregation chunk loader
        cur = [dict(ch=-1, gb=None, sb=None, t0=0) for _ in range(G)]
        q_counter = [0]

        def ensure_chunk(g, ch):
            st = cur[g]
            if st["ch"] == ch:
                return st
            t0, ntl = plan["gchunks"][g][ch]
            nid = ntl * 128
            ib = ipools[g].tile([128, TCH * 8], i16, tag="idx")
            nc.sync.dma_start(out=ib[:, : ntl * 8], in_=idx_dr[:, t0 * 8:(t0 + ntl) * 8])
            sb = spools[g].tile([128, TCH, 2, W], bf16, tag="s")
            nc.scalar.dma_start(out=sb[:, :ntl, :, :], in_=S_dr[:, t0:t0 + ntl, :, :])
            gb = gpools[g].tile([128, TCH, TBLW], bf16, tag="g")
            nc.gpsimd.dma_gather(
                gb[:, :ntl, :],
                table[g * cfg.PAIR_CHUNK:(g + 1) * cfg.PAIR_CHUNK, :],
                ib[:, : ntl * 8],
                nid, nid, TBLW,
                single_packet=False,
                queue_num=q_counter[0] % NQ,
            )
            q_counter[0] += 1
            st.update(ch=ch, gb=gb, sb=sb, t0=t0)
            return st

        # pre-warm the first gather round before the big sxT DMA queues up
        for g in range(G):
            if plan["gT"][g]:
                ensure_chunk(g, 0)

        # ---- constants
        zrow = cpool.tile([1, WIN], bf16)
        nc.vector.memset(zrow[:, :], 0.0)
        Wd_sb = cpool.tile([C, C], bf16)
        nc.sync.dma_start(out=Wd_sb[:, :], in_=Wd[:, :])
        bb = cpool.tile([128, C], f32)
        nc.sync.dma_start(out=bb[:, :], in_=bb_dr[:, :])
        bd_col = cpool.tile([C, 1], f32)
        nc.sync.dma_start(out=bd_col[:, :], in_=bdc[:, :])
        lw_sb = cpool.tile([C, 1], bf16)
        nc.sync.dma_start(out=lw_sb[:, :], in_=lw[:, :])
        lb_sb = cpool.tile([1, 1], f32)
        nc.sync.dma_start(out=lb_sb[:, :], in_=lb[:, :])
        nlb = cpool.tile([1, 1], f32)
        nc.scalar.mul(nlb[:, :], lb_sb[:, :], -1.0)

        zT = zpool.tile([C, NLOC], f32)
        nc.sync.dma_start(out=zT[:, :], in_=sxT_dr[:, :])  # self-loop term
        zb = zpool.tile([C, NLOC], bf16)

        HB = WIN // 128

        def dense_for_window(w):
            wlen = min(WIN, NLOC - w * WIN)
            nchunks = (wlen + 127) // 128
            hb = hpool.tile([128, HB, C], bf16, tag="h")
            for kk in range(nchunks):
                k = w * HB + kk
                mrow = min(128, NLOC - k * 128)
                psd = pdpool.tile([128, C], f32)
                nc.tensor.matmul(psd[:mrow, :], lhsT=zb[:, k * 128:k * 128 + mrow],
                                 rhs=Wd_sb[:, :], start=True, stop=True)
                nc.vector.tensor_tensor(out=hb[:mrow, kk, :], in0=psd[:mrow, :],
                                        in1=bb[:mrow, :], op=mybir.AluOpType.add)
                nc.scalar.activation(hb[:mrow, kk, :], hb[:mrow, kk, :],
                                     mybir.ActivationFunctionType.Relu)
            r0, r1 = w * WIN, w * WIN + wlen
            nfull = (r1 - r0) // 128
            if nfull:
                dst = h_out[r0:r0 + nfull * 128, :].rearrange(
                    "(t p) c -> p t c", p=128)
                nc.sync.dma_start(out=dst, in_=hb[:, :nfull, :])
            rem = (r1 - r0) - nfull * 128
            if rem:
                nc.sync.dma_start(out=h_out[r0 + nfull * 128:r1, :],
                                  in_=hb[:rem, nfull, :])

        def head_for_window(w):
            wlen = min(WIN, NLOC - w * WIN)
            pst = ptpool.tile([C, WIN], f32)
            nc.tensor.matmul(pst[:, :wlen], lhsT=Wd_sb[:, :],
                             rhs=zb[:, w * WIN:w * WIN + wlen],
                             start=True, stop=True)
            ht = htpool.tile([C, WIN], bf16, tag="ht")
            nc.scalar.activation(ht[:, :wlen], pst[:, :wlen],
                                 mybir.ActivationFunctionType.Relu, bias=bd_col[:, :])
            psl = plpool.tile([1, WIN], f32)
            nc.tensor.matmul(psl[:, :wlen], lhsT=lw_sb[:, :], rhs=ht[:, :wlen],
                             start=True, stop=True)
            otn = opool.tile([1, WIN], f32, tag="otn")
            otp = opool.tile([1, WIN], f32, tag="otp")
            nc.scalar.activation(otn[:, :wlen], psl[:, :wlen],
                                 mybir.ActivationFunctionType.Identity,
                                 bias=nlb[:, :], scale=-1.0)
            nc.scalar.activation(otp[:, :wlen], psl[:, :wlen],
                                 mybir.ActivationFunctionType.Identity,
                                 bias=lb_sb[:, :], scale=1.0)
            nc.sync.dma_start(out=outT[0:1, w * WIN:w * WIN + wlen], in_=otn[:, :wlen])
            nc.sync.dma_start(out=outT[1:2, w * WIN:w * WIN + wlen], in_=otp[:, :wlen])

        for w in range(cfg.NWIN):
            wlen = min(WIN, NLOC - w * WIN)
            ps = ppool.tile([C, WIN], f32)
            nc.tensor.matmul(ps[:, :wlen], lhsT=zrow[:, :C], rhs=zrow[:, :wlen],
                             start=True, stop=False)
            for g, t, off, weff in win_tiles[w]:
                st = ensure_chunk(g, int(tile_chunk[t]))
                tp = t - st["t0"]
                for hh in range(2):
                    nc.tensor.matmul(
                        ps[:, off:off + weff],
                        lhsT=st["gb"][:, tp, hh * C:hh * C + C],
                        rhs=st["sb"][:, tp, hh, :weff],
                        start=False, stop=False,
                        skip_group_check=True,
                    )
            nc.tensor.matmul(ps[:, :wlen], lhsT=zrow[:, :C], rhs=zrow[:, :wlen],
                             start=False, stop=True)
            zw = zT[:, w * WIN:w * WIN + wlen]
            nc.vector.tensor_tensor(out=zw, in0=ps[:, :wlen], in1=zw,
                                    op=mybir.AluOpType.add)
            nc.scalar.copy(out=zb[:, w * WIN:w * WIN + wlen], in_=zw)
            # dense/head run one window behind so the in-order PE stream
            # never waits on the cross-engine flush->cast chain
            if w > 0:
                if _stage in ("all", "dense"):
                    dense_for_window(w - 1)
                if _stage in ("all", "head"):
                    head_for_window(w - 1)
        if _stage in ("all", "dense"):
            dense_for_window(cfg.NWIN - 1)
        if _stage in ("all", "head"):
            head_for_window(cfg.NWIN - 1)

    nc.compile()
    return nc


# ------------------------------------------------------------------ runner ---
def make_runner(nc, device):
    """Single-core jit runner pinned to one device, reusable across calls."""
    import jax
    import concourse.mybir as mybir
    from concourse import bass2jax

    bass2jax.install_neuronx_cc_hook()

    in_names, out_names, out_avals, zero_shapes = [], [], [], []
    for alloc in nc.m.functions[0].allocations:
        if not isinstance(alloc, mybir.MemoryLocationSet):
            continue
        nm = alloc.memorylocations[0].name
        if alloc.kind == "ExternalInput":
            in_names.append(nm)
        elif alloc.kind == "ExternalOutput":
            shape = tuple(alloc.tensor_shape)
            dtype = mybir.dt.np(alloc.dtype)
            out_names.append(nm)
            out_avals.append(jax.core.ShapedArray(shape, dtype))
            zero_shapes.append((shape, dtype))
    n_params = len(in_names)
    all_in_names = in_names + out_names
    donate = tuple(range(n_params, n_params + len(out_names)))

    def _body(*args):
        outs = bass2jax._bass_exec_p.bind(
            *args,
            out_avals=tuple(out_avals),
            in_names=tuple(all_in_names),
            out_names=tuple(out_names),
            lowering_input_output_aliases=(),
            sim_require_finite=True,
            sim_require_nnan=True,
            nc=nc,
        )
        return tuple(outs)

    jitted = jax.jit(_body, donate_argnums=donate, keep_unused=True)

    def run(in_map):
        args = [jax.device_put(np.asarray(in_map[nm]), device) for nm in in_names]
        zeros = [jax.device_put(np.zeros(s, d), device) for s, d in zero_shapes]
        outs = jitted(*args, *zeros)
        return {nm: outs[i] for i, nm in enumerate(out_names)}

    return run


# ---------------------------------------------------------------- kernel() ---
_CACHE = {}


def _get_runners(plans, cfg):
    import jax
    key = "runners"
    if key in _CACHE:
        return _CACHE[key]
    devices = jax.devices()[:cfg.P]
    ncs = [build_program(plans[d], cfg, name=f"gnn_d{d}") for d in range(cfg.P)]
    runners = [make_runner(ncs[d], devices[d]) for d in range(cfg.P)]
    _CACHE[key] = runners
    return runners


def run_two_phase(inputs, cfg=FULL):
    import jax
    from concurrent.futures import ThreadPoolExecutor

    x = np.asarray(inputs["x"], np.float32)
    W1 = np.asarray(inputs["W1"], np.float32)
    b1 = np.asarray(inputs["b1"], np.float32)
    W2 = np.asarray(inputs["W2"], np.float32)
    b2 = np.asarray(inputs["b2"], np.float32)
    lin_w = np.asarray(inputs["lin_w"], np.float32)
    lin_b = np.asarray(inputs["lin_b"], np.float32)
    C, H2 = cfg.C, cfg.H2

    plans, dis = preprocess(inputs["edge_index"], inputs["edge_logits"], cfg)
    dis2 = (dis * dis).astype(np.float32)
    runners = _get_runners(plans, cfg)

    W2p = np.zeros((C, C), np.float32)
    W2p[:, :H2] = W2
    b2p = np.zeros(C, np.float32)
    b2p[:H2] = b2
    lwp = np.zeros((C, 1), np.float32)
    lwp[:H2, 0] = lin_w[:, 0]
    lbp = lin_b.reshape(1, 1)
    zconst = np.zeros((C, 1), BF16)

    def pair_table(d, f_bf16):
        return np.ascontiguousarray(
            f_bf16[plans[d]["perm"]].reshape(cfg.NPAIR, cfg.TBLW))

    def phase_inputs(d, tbl_bf16, sx32, Wdv, bdv, lwv, lbv):
        sh = slice(d * cfg.NLOC, (d + 1) * cfg.NLOC)
        sxT = np.ascontiguousarray((sx32[sh] * dis2[sh, None]).T)
        m = dict(table=pair_table(d, tbl_bf16), sxT=sxT, Wd=Wdv.astype(BF16),
                 bb=np.tile(bdv, (128, 1)).astype(np.float32),
                 bdc=bdv.reshape(C, 1).astype(np.float32),
                 lw=lwv.astype(BF16), lb=lbv.astype(np.float32),
                 idx=plans[d]["idx"], S=plans[d]["S"])
        return m

    x_bf16 = x.astype(BF16)

    # phase A: table=x pairs, dense=W1/b1 (head inputs zeroed)
    with ThreadPoolExecutor(cfg.P) as exe:
        resA = list(exe.map(
            lambda d: runners[d](phase_inputs(d, x_bf16, x, W1, b1, zconst,
                                              np.zeros((1, 1), np.float32))),
            range(cfg.P)))
    h_shards = [np.asarray(r["h_out"]) for r in resA]
    h1_bf16 = np.concatenate(h_shards, axis=0)          # [N, 64] bf16
    h1_f32 = h1_bf16.astype(np.float32)

    # phase B: table=h1 pairs, dense=padded W2/b2, head=lin
    with ThreadPoolExecutor(cfg.P) as exe:
        resB = list(exe.map(
            lambda d: runners[d](phase_inputs(d, h1_bf16, h1_f32, W2p, b2p, lwp, lbp)),
            range(cfg.P)))
    out = np.concatenate([np.asarray(r["outT"]).T for r in resB], axis=0)
    return out.astype(np.float32)


def kernel(x, edge_index, edge_logits, W1, b1, W2, b2, lin_w, lin_b):
    inputs = dict(x=x, edge_index=edge_index, edge_logits=edge_logits,
                  W1=W1, b1=b1, W2=W2, b2=b2, lin_w=lin_w, lin_b=lin_b)
    return run_two_phase(inputs, FULL)


# revision 10
# speedup vs baseline: 4.4425x; 1.2279x over previous
"""Trainium2 Bass kernel for a 2-layer edge-gated GCN (DiffGNNPlacement).

Math (reference, per layer):
    ew   = 0.5 + sigmoid(edge_logits)                  # [E]
    deg  = segsum(ew -> col) + 1                       # [N]
    dis  = deg^-1/2
    norm = dis[row] * ew * dis[col]                    # [E]
    out  = segsum(norm * (h@W)[row] -> col) + (h@W)*dis^2 + b

Key transform: aggregation commutes with the (linear) feature transform, so
    out = (segsum(norm * h[row] -> col) + h*dis^2) @ W + b
and the self-loop term is folded in host-side via the sxT input.

Device algorithm (per core, nodes sharded 12500/core):
  - SWDGE descriptor generation is the bottleneck (~9.5ns/descriptor): every
    gathered row costs one descriptor, so nodes are PAIRED per core (greedy
    matching of sources that co-occur in the same 16-target-col bucket) and
    the per-core feature table stores one 256B bf16 row per PAIR. One
    descriptor then feeds up to two edges.
  - edges partitioned by target shard, sorted by target col; slots are
    (pair, bucket) units; tiles pack 128 slots spanning <=32 target cols.
  - per tile: dma_gather 128 pair-rows -> SBUF [128, 128] bf16; host-built
    Su/Sv [128, 32] bf16 (norms of the u-half / v-half edges); two PE
    matmuls psum[64, off:off+w] += gathered[:, h*64:h*64+64]^T @ S_h
    accumulate the aggregation z^T for a 512-col PSUM window.
  - gathers spread over all 4 SWDGE queues (queue q = Q7 cores 2q/2q+1),
    queue_num = issue_index % 4 so the Tile DMASW lane (issue % 8) is fed by
    a single queue (completions stay FIFO per lane -> race-free). Early
    chunks are small for fast pipeline ramp.
  - dense h = relu(z @ W + b) and the classifier head are interleaved into
    the window loop (run in PE stall gaps while descriptors generate).

The same compiled program serves both layers; it is launched twice per core
with a host re-pairing of h1 shards in between.
"""

import os
import sys
import math
import numpy as np
import ml_dtypes
from contextlib import ExitStack

for _p in ("/opt/trn_rl_repo", "/root/.axon_site/_ro/trn_rl_repo"):
    if os.path.isdir(_p) and _p not in sys.path:
        sys.path.insert(0, _p)

BF16 = ml_dtypes.bfloat16


# ----------------------------------------------------------------- config ---
class Cfg:
    def __init__(self, N=100000, E=1600000, C=64, H2=32, P=8,
                 PAIR_CHUNK=25000, B=16, W=32, WIN=512, TCH=24, TBLW=128):
        self.N, self.E, self.C, self.H2, self.P = N, E, C, H2, P
        self.NLOC = N // P
        self.NPAIR = N // 2
        self.PAIR_CHUNK = PAIR_CHUNK          # pairs per gather group
        self.NGRP = (self.NPAIR + PAIR_CHUNK - 1) // PAIR_CHUNK
        self.B = B            # pairing bucket (target cols)
        self.W = W            # S tile width (target-col window per tile)
        self.WIN = WIN        # PSUM accumulation window (cols)
        self.TCH = TCH        # steady-state tiles per gather chunk
        self.RAMP = (6, 12, 24)  # tile counts for the first chunks
        self.TBLW = TBLW      # pair row width (bf16 elems; 256B rows)
        self.NWIN = (self.NLOC + WIN - 1) // WIN
        assert PAIR_CHUNK <= 32767
        assert TBLW * 2 == 256  # dma_gather elem constraint (256B rows)
        assert 2 * B <= W and WIN % B == 0


FULL = Cfg()


# --------------------------------------------------------- host preprocess ---
def _sigmoid(x):
    return 0.5 * (np.tanh(0.5 * x) + 1.0)


def _pair_nodes(r, c, cfg):
    """Greedy per-core pairing: sort sources by the first B-col bucket they
    appear in, pair adjacent. Returns pairs [NPAIR, 2] (a permutation of all
    nodes)."""
    N, B = cfg.N, cfg.B
    bucket = c // B
    o = np.lexsort((bucket, r))
    rs, bs = r[o], bucket[o]
    first = np.ones(len(rs), bool)
    if len(rs):
        first[1:] = rs[1:] != rs[:-1]
    src_f, buck_f = rs[first], bs[first]
    present = np.zeros(N, bool)
    present[src_f] = True
    absent = np.where(~present)[0]
    o3 = np.argsort(buck_f, kind="stable")
    allsrc = np.concatenate([src_f[o3], absent])
    pairs = allsrc.reshape(-1, 2)
    pairid = np.empty(N, np.int64)
    half = np.empty(N, np.int8)
    pairid[pairs[:, 0]] = np.arange(len(pairs))
    pairid[pairs[:, 1]] = np.arange(len(pairs))
    half[pairs[:, 0]] = 0
    half[pairs[:, 1]] = 1
    return pairs, pairid, half


def preprocess(edge_index, edge_logits, cfg=FULL):
    """Compute norms and per-device pairings + tile plans (pure numpy)."""
    N, NLOC = cfg.N, cfg.NLOC
    row = np.asarray(edge_index[0], dtype=np.int64)
    col = np.asarray(edge_index[1], dtype=np.int64)
    ew = (0.5 + _sigmoid(np.asarray(edge_logits, dtype=np.float32))).astype(np.float32)
    deg = np.bincount(col, weights=ew.astype(np.float64), minlength=N).astype(np.float32) + 1.0
    dis = deg ** -0.5
    norm = (dis[row] * ew * dis[col]).astype(np.float32)

    dev = col // NLOC
    order = np.argsort(dev, kind="stable")
    rs, cs, vs, ds = row[order], col[order] % NLOC, norm[order], dev[order]
    bounds = np.searchsorted(ds, np.arange(cfg.P + 1))
    plans = []
    for d in range(cfg.P):
        a, b = bounds[d], bounds[d + 1]
        plans.append(_plan_device(rs[a:b], cs[a:b], vs[a:b], cfg))
    return plans, dis


def _chunk_sizes(T, cfg):
    sizes = []
    for s in cfg.RAMP:
        if T - sum(sizes) <= 0:
            break
        sizes.append(min(s, T - sum(sizes)))
    rem = T - sum(sizes)
    while rem > 0:
        s = min(cfg.TCH, rem)
        sizes.append(s)
        rem -= s
    return sizes or [0]


def _plan_device(r, c, v, cfg):
    """Pair sources, build (pair, bucket) slots, pack 128-slot tiles."""
    B, W, WIN, G = cfg.B, cfg.W, cfg.WIN, cfg.NGRP
    pairs, pairid, half = _pair_nodes(r, c, cfg)
    perm = pairs.reshape(-1)                     # node order in the pair table

    p = pairid[r]
    h = half[r].astype(np.int64)
    grp = p // cfg.PAIR_CHUNK
    bucket = c // B

    # slots: distinct (grp, bucket, pair); edges sorted into slot-major order
    okey = ((grp * 800 + bucket) * np.int64(cfg.NPAIR)) + p
    o = np.argsort(okey, kind="stable")
    co, vo, ho, go, ko = c[o], v[o], h[o], grp[o], okey[o]
    po = p[o]
    m = len(ko)
    newslot = np.ones(max(m, 1), bool)
    if m:
        newslot[1:m] = ko[1:] != ko[:-1]
    slot_of_edge = np.cumsum(newslot[:m]) - 1 if m else np.zeros(0, np.int64)
    nslot = int(slot_of_edge[-1]) + 1 if m else 0
    slot_starts = np.where(newslot[:m])[0] if m else np.zeros(0, np.int64)
    slot_pair = po[slot_starts] if m else np.zeros(0, np.int64)
    slot_grp = go[slot_starts] if m else np.zeros(0, np.int64)
    slot_c0 = np.minimum.reduceat(co, slot_starts) if m else np.zeros(0, np.int64)
    slot_cmax = np.maximum.reduceat(co, slot_starts) if m else np.zeros(0, np.int64)

    # tiles: sequential pack per grp, <=128 slots, span < W, same WIN window
    tile_id = np.zeros(max(nslot, 1), np.int64)
    tile_c0s = []
    tile_grps = []
    t = -1
    cnt = 0
    cur_c0 = -10 ** 9
    cur_grp = -1
    for s in range(nslot):
        g = int(slot_grp[s])
        # align the tile base to the slot's bucket start: later slots in the
        # same bucket (ordered by pair id) may have smaller cols
        c0 = (int(slot_c0[s]) // B) * B
        cmax = int(slot_cmax[s])
        if (g != cur_grp or cnt >= 128 or cmax >= cur_c0 + W
                or (cmax // WIN) != (cur_c0 // WIN)):
            t += 1
            cnt = 0
            cur_c0 = c0
            cur_grp = g
            tile_c0s.append(c0)
            tile_grps.append(g)
        tile_id[s] = t
        cnt += 1
    T = t + 1 if nslot else 0
    tile_c0s = np.array(tile_c0s if T else [0], np.int32)
    tile_grp = np.array(tile_grps if T else [0], np.int64)
    tile_first_slot = np.searchsorted(tile_id[:max(nslot, 1)], np.arange(max(T, 1)))
    slot_in_tile = (np.arange(max(nslot, 1)) - tile_first_slot[tile_id]) if nslot else np.zeros(1, np.int64)

    Tm = max(T, 1)
    idx16 = np.zeros((Tm, 128), np.int16)
    if nslot:
        idx16[tile_id[:nslot], slot_in_tile[:nslot]] = (
            slot_pair - slot_grp * cfg.PAIR_CHUNK).astype(np.int16)
    S = np.zeros((Tm, 2, 128, cfg.W), np.float32)
    if m:
        e_tile = tile_id[slot_of_edge]
        e_slot = slot_in_tile[slot_of_edge]
        e_coff = co - tile_c0s[e_tile]
        np.add.at(S, (e_tile, ho, e_slot, e_coff), vo)

    # tile-major packing for the device
    wrapped = idx16.reshape(Tm, 8, 16).transpose(2, 0, 1)          # [16, T, 8]
    idx_w = np.ascontiguousarray(np.tile(wrapped.reshape(16, Tm * 8), (8, 1)))
    S_pk = np.ascontiguousarray(S.transpose(2, 0, 1, 3)).astype(BF16)  # [128, T, 2, W]

    # group tiles by grp for chunking (tiles are grp-ordered)
    gT = [int(np.sum(tile_grp[:T] == g)) for g in range(G)] if T else [0] * G
    gt0 = [int(np.searchsorted(tile_grp[:T], g)) for g in range(G)] if T else [0] * G
    gchunks = []
    for g in range(G):
        chunks = []
        t0 = gt0[g]
        for s in _chunk_sizes(gT[g], cfg):
            chunks.append((t0, s))
            t0 += s
        gchunks.append(chunks)

    win = tile_c0s // WIN
    off = tile_c0s - win * WIN
    return dict(T=T, idx=idx_w, S=S_pk, perm=perm, gchunks=gchunks,
                gT=gT, gt0=gt0, win=win, off=off, tile_grp=tile_grp)


# ---------------------------------------------------------- program builder ---
def build_program(plan, cfg=FULL, name="gnn"):
    import concourse.bass as bass
    import concourse.mybir as mybir
    from concourse import bacc
    from concourse.tile import TileContext

    f32, i16, bf16 = mybir.dt.float32, mybir.dt.int16, mybir.dt.bfloat16
    C, W, WIN, NLOC = cfg.C, cfg.W, cfg.WIN, cfg.NLOC
    TBLW = cfg.TBLW
    G = cfg.NGRP
    NQ = 4  # SWDGE queues
    TCH = cfg.TCH

    nc = bacc.Bacc("TRN2", enable_partition_id=False,
                   target_bir_lowering=False, name=name,
                   num_swdge_queues=NQ)

    table = nc.dram_tensor("table", [cfg.NPAIR, TBLW], bf16, kind="ExternalInput")
    sxT_dr = nc.dram_tensor("sxT", [C, NLOC], f32, kind="ExternalInput")
    Wd = nc.dram_tensor("Wd", [C, C], bf16, kind="ExternalInput")
    bb_dr = nc.dram_tensor("bb", [128, C], f32, kind="ExternalInput")
    bdc = nc.dram_tensor("bdc", [C, 1], f32, kind="ExternalInput")
    lw = nc.dram_tensor("lw", [C, 1], bf16, kind="ExternalInput")
    lb = nc.dram_tensor("lb", [1, 1], f32, kind="ExternalInput")
    idx_dr = nc.dram_tensor("idx", list(plan["idx"].shape), i16, kind="ExternalInput")
    S_dr = nc.dram_tensor("S", list(plan["S"].shape), bf16, kind="ExternalInput")
    h_out = nc.dram_tensor("h_out", [NLOC, C], bf16, kind="ExternalOutput")
    outT = nc.dram_tensor("outT", [2, NLOC], f32, kind="ExternalOutput")

    # per-window tile lists: (g, t, off, weff)
    win_tiles = [[] for _ in range(cfg.NWIN)]
    for t in range(plan["T"]):
        w = int(plan["win"][t])
        off = int(plan["off"][t])
        wlen = min(WIN, NLOC - w * WIN)
        weff = min(W, wlen - off)
        win_tiles[w].append((int(plan["tile_grp"][t]), t, off, weff))

    # tile -> chunk per group
    tile_chunk = np.zeros(max(plan["T"], 1), np.int64)
    for g in range(G):
        for ci, (t0, ntl) in enumerate(plan["gchunks"][g]):
            tile_chunk[t0:t0 + ntl] = ci

    _stage = os.environ.get("GNN_STAGE", "all")

    with TileContext(nc) as tc, ExitStack() as ex:
        cpool = ex.enter_context(tc.tile_pool(name="consts", bufs=1))
        zpool = ex.enter_context(tc.tile_pool(name="z", bufs=1))
        gpools = [ex.enter_context(tc.tile_pool(name=f"gat{g}", bufs=6)) for g in range(G)]
        ipools = [ex.enter_context(tc.tile_pool(name=f"idx{g}", bufs=6)) for g in range(G)]
        spools = [ex.enter_context(tc.tile_pool(name=f"s{g}", bufs=6)) for g in range(G)]
        ppool = ex.enter_context(tc.tile_pool(name="psagg", bufs=2, space="PSUM"))
        pdpool = ex.enter_context(tc.tile_pool(name="psd", bufs=2, space="PSUM"))
        ptpool = ex.enter_context(tc.tile_pool(name="pst", bufs=2, space="PSUM"))
        plpool = ex.enter_context(tc.tile_pool(name="psl", bufs=2, space="PSUM"))
        hpool = ex.enter_context(tc.tile_pool(name="hrows", bufs=2))
        htpool = ex.enter_context(tc.tile_pool(name="ht", bufs=2))
        opool = ex.enter_context(tc.tile_pool(name="ot", bufs=2))

        # ---- aggregation chunk loader
        cur = [dict(ch=-1, gb=None, sb=None, t0=0) for _ in range(G)]
        q_counter = [0]

        def ensure_chunk(g, ch):
            st = cur[g]
            if st["ch"] == ch:
                return st
            t0, ntl = plan["gchunks"][g][ch]
            nid = ntl * 128
            ib = ipools[g].tile([128, TCH * 8], i16, tag="idx")
            nc.sync.dma_start(out=ib[:, : ntl * 8], in_=idx_dr[:, t0 * 8:(t0 + ntl) * 8])
            sb = spools[g].tile([128, TCH, 2, W], bf16, tag="s")
            nc.scalar.dma_start(out=sb[:, :ntl, :, :], in_=S_dr[:, t0:t0 + ntl, :, :])
            gb = gpools[g].tile([128, TCH, TBLW], bf16, tag="g")
            nc.gpsimd.dma_gather(
                gb[:, :ntl, :],
                table[g * cfg.PAIR_CHUNK:(g + 1) * cfg.PAIR_CHUNK, :],
                ib[:, : ntl * 8],
                nid, nid, TBLW,
                single_packet=False,
                queue_num=q_counter[0] % NQ,
            )
            q_counter[0] += 1
            st.update(ch=ch, gb=gb, sb=sb, t0=t0)
            return st

        # pre-warm the first gather round before the big sxT DMA queues up
        for g in range(G):
            if plan["gT"][g]:
                ensure_chunk(g, 0)

        # ---- constants
        zrow = cpool.tile([1, WIN], bf16)
        nc.vector.memset(zrow[:, :], 0.0)
        Wd_sb = cpool.tile([C, C], bf16)
        nc.sync.dma_start(out=Wd_sb[:, :], in_=Wd[:, :])
        bb = cpool.tile([128, C], f32)
        nc.sync.dma_start(out=bb[:, :], in_=bb_dr[:, :])
        bd_col = cpool.tile([C, 1], f32)
        nc.sync.dma_start(out=bd_col[:, :], in_=bdc[:, :])
        lw_sb = cpool.tile([C, 1], bf16)
        nc.sync.dma_start(out=lw_sb[:, :], in_=lw[:, :])
        lb_sb = cpool.tile([1, 1], f32)
        nc.sync.dma_start(out=lb_sb[:, :], in_=lb[:, :])
        nlb = cpool.tile([1, 1], f32)
        nc.scalar.mul(nlb[:, :], lb_sb[:, :], -1.0)

        zT = zpool.tile([C, NLOC], f32)
        nc.sync.dma_start(out=zT[:, :], in_=sxT_dr[:, :])  # self-loop term
        zb = zpool.tile([C, NLOC], bf16)

        HB = WIN // 128

        def dense_for_window(w):
            wlen = min(WIN, NLOC - w * WIN)
            nchunks = (wlen + 127) // 128
            hb = hpool.tile([128, HB, C], bf16, tag="h")
            for kk in range(nchunks):
                k = w * HB + kk
                mrow = min(128, NLOC - k * 128)
                psd = pdpool.tile([128, C], f32)
                nc.tensor.matmul(psd[:mrow, :], lhsT=zb[:, k * 128:k * 128 + mrow],
                                 rhs=Wd_sb[:, :], start=True, stop=True)
                nc.vector.tensor_tensor(out=hb[:mrow, kk, :], in0=psd[:mrow, :],
                                        in1=bb[:mrow, :], op=mybir.AluOpType.add)
                nc.scalar.activation(hb[:mrow, kk, :], hb[:mrow, kk, :],
                                     mybir.ActivationFunctionType.Relu)
            r0, r1 = w * WIN, w * WIN + wlen
            nfull = (r1 - r0) // 128
            if nfull:
                dst = h_out[r0:r0 + nfull * 128, :].rearrange(
                    "(t p) c -> p t c", p=128)
                nc.sync.dma_start(out=dst, in_=hb[:, :nfull, :])
            rem = (r1 - r0) - nfull * 128
            if rem:
                nc.sync.dma_start(out=h_out[r0 + nfull * 128:r1, :],
                                  in_=hb[:rem, nfull, :])

        def head_for_window(w):
            wlen = min(WIN, NLOC - w * WIN)
            pst = ptpool.tile([C, WIN], f32)
            nc.tensor.matmul(pst[:, :wlen], lhsT=Wd_sb[:, :],
                             rhs=zb[:, w * WIN:w * WIN + wlen],
                             start=True, stop=True)
            ht = htpool.tile([C, WIN], bf16, tag="ht")
            nc.scalar.activation(ht[:, :wlen], pst[:, :wlen],
                                 mybir.ActivationFunctionType.Relu, bias=bd_col[:, :])
            psl = plpool.tile([1, WIN], f32)
            nc.tensor.matmul(psl[:, :wlen], lhsT=lw_sb[:, :], rhs=ht[:, :wlen],
                             start=True, stop=True)
            otn = opool.tile([1, WIN], f32, tag="otn")
            otp = opool.tile([1, WIN], f32, tag="otp")
            nc.scalar.activation(otn[:, :wlen], psl[:, :wlen],
                                 mybir.ActivationFunctionType.Identity,
                                 bias=nlb[:, :], scale=-1.0)
            nc.scalar.activation(otp[:, :wlen], psl[:, :wlen],
                                 mybir.ActivationFunctionType.Identity,
                                 bias=lb_sb[:, :], scale=1.0)
            nc.sync.dma_start(out=outT[0:1, w * WIN:w * WIN + wlen], in_=otn[:, :wlen])
            nc.sync.dma_start(out=outT[1:2, w * WIN:w * WIN + wlen], in_=otp[:, :wlen])

        for w in range(cfg.NWIN):
            wlen = min(WIN, NLOC - w * WIN)
            ps = ppool.tile([C, WIN], f32)
            nc.tensor.matmul(ps[:, :wlen], lhsT=zrow[:, :C], rhs=zrow[:, :wlen],
                             start=True, stop=False)
            for g, t, off, weff in win_tiles[w]:
                st = ensure_chunk(g, int(tile_chunk[t]))
                tp = t - st["t0"]
                for hh in range(2):
                    nc.tensor.matmul(
                        ps[:, off:off + weff],
                        lhsT=st["gb"][:, tp, hh * C:hh * C + C],
                        rhs=st["sb"][:, tp, hh, :weff],
                        start=False, stop=False,
                        skip_group_check=True,
                    )
            nc.tensor.matmul(ps[:, :wlen], lhsT=zrow[:, :C], rhs=zrow[:, :wlen],
                             start=False, stop=True)
            zw = zT[:, w * WIN:w * WIN + wlen]
            nc.vector.tensor_tensor(out=zw, in0=ps[:, :wlen], in1=zw,
                                    op=mybir.AluOpType.add)
            nc.scalar.copy(out=zb[:, w * WIN:w * WIN + wlen], in_=zw)
            # dense/head run one window behind so the in-order PE stream
            # never waits on the cross-engine flush->cast chain
            if w > 0:
                if _stage in ("all", "dense"):
                    dense_for_window(w - 1)
                if _stage in ("all", "head"):
                    head_for_window(w - 1)
        if _stage in ("all", "dense"):
            dense_for_window(cfg.NWIN - 1)
        if _stage in ("all", "head"):
            head_for_window(cfg.NWIN - 1)

    nc.compile()
    return nc


# ------------------------------------------------------------------ runner ---
def make_runner(nc, device):
    """Single-core jit runner pinned to one device, reusable across calls."""
    import jax
    import concourse.mybir as mybir
    from concourse import bass2jax

    bass2jax.install_neuronx_cc_hook()

    in_names, out_names, out_avals, zero_shapes = [], [], [], []
    for alloc in nc.m.functions[0].allocations:
        if not isinstance(alloc, mybir.MemoryLocationSet):
            continue
        nm = alloc.memorylocations[0].name
        if alloc.kind == "ExternalInput":
            in_names.append(nm)
        elif alloc.kind == "ExternalOutput":
            shape = tuple(alloc.tensor_shape)
            dtype = mybir.dt.np(alloc.dtype)
            out_names.append(nm)
            out_avals.append(jax.core.ShapedArray(shape, dtype))
            zero_shapes.append((shape, dtype))
    n_params = len(in_names)
    all_in_names = in_names + out_names
    donate = tuple(range(n_params, n_params + len(out_names)))

    def _body(*args):
        outs = bass2jax._bass_exec_p.bind(
            *args,
            out_avals=tuple(out_avals),
            in_names=tuple(all_in_names),
            out_names=tuple(out_names),
            lowering_input_output_aliases=(),
            sim_require_finite=True,
            sim_require_nnan=True,
            nc=nc,
        )
        return tuple(outs)

    jitted = jax.jit(_body, donate_argnums=donate, keep_unused=True)

    def run(in_map):
        args = [jax.device_put(np.asarray(in_map[nm]), device) for nm in in_names]
        zeros = [jax.device_put(np.zeros(s, d), device) for s, d in zero_shapes]
        outs = jitted(*args, *zeros)
        return {nm: outs[i] for i, nm in enumerate(out_names)}

    return run


# ---------------------------------------------------------------- kernel() ---
_CACHE = {}


def _get_runners(plans, cfg):
    import jax
    key = "runners"
    if key in _CACHE:
        return _CACHE[key]
    devices = jax.devices()[:cfg.P]
    ncs = [build_program(plans[d], cfg, name=f"gnn_d{d}") for d in range(cfg.P)]
    runners = [make_runner(ncs[d], devices[d]) for d in range(cfg.P)]
    _CACHE[key] = runners
    return runners


def run_two_phase(inputs, cfg=FULL):
    import jax
    from concurrent.futures import ThreadPoolExecutor

    x = np.asarray(inputs["x"], np.float32)
    W1 = np.asarray(inputs["W1"], np.float32)
    b1 = np.asarray(inputs["b1"], np.float32)
    W2 = np.asarray(inputs["W2"], np.float32)
    b2 = np.asarray(inputs["b2"], np.float32)
    lin_w = np.asarray(inputs["lin_w"], np.float32)
    lin_b = np.asarray(inputs["lin_b"], np.float32)
    C, H2 = cfg.C, cfg.H2

    plans, dis = preprocess(inputs["edge_index"], inputs["edge_logits"], cfg)
    dis2 = (dis * dis).astype(np.float32)
    runners = _get_runners(plans, cfg)

    W2p = np.zeros((C, C), np.float32)
    W2p[:, :H2] = W2
    b2p = np.zeros(C, np.float32)
    b2p[:H2] = b2
    lwp = np.zeros((C, 1), np.float32)
    lwp[:H2, 0] = lin_w[:, 0]
    lbp = lin_b.reshape(1, 1)
    zconst = np.zeros((C, 1), BF16)

    def pair_table(d, f_bf16):
        return np.ascontiguousarray(
            f_bf16[plans[d]["perm"]].reshape(cfg.NPAIR, cfg.TBLW))

    def phase_inputs(d, tbl_bf16, sx32, Wdv, bdv, lwv, lbv):
        sh = slice(d * cfg.NLOC, (d + 1) * cfg.NLOC)
        sxT = np.ascontiguousarray((sx32[sh] * dis2[sh, None]).T)
        m = dict(table=pair_table(d, tbl_bf16), sxT=sxT, Wd=Wdv.astype(BF16),
                 bb=np.tile(bdv, (128, 1)).astype(np.float32),
                 bdc=bdv.reshape(C, 1).astype(np.float32),
                 lw=lwv.astype(BF16), lb=lbv.astype(np.float32),
                 idx=plans[d]["idx"], S=plans[d]["S"])
        return m

    x_bf16 = x.astype(BF16)

    # phase A: table=x pairs, dense=W1/b1 (head inputs zeroed)
    with ThreadPoolExecutor(cfg.P) as exe:
        resA = list(exe.map(
            lambda d: runners[d](phase_inputs(d, x_bf16, x, W1, b1, zconst,
                                              np.zeros((1, 1), np.float32))),
            range(cfg.P)))
    h_shards = [np.asarray(r["h_out"]) for r in resA]
    h1_bf16 = np.concatenate(h_shards, axis=0)          # [N, 64] bf16
    h1_f32 = h1_bf16.astype(np.float32)

    # phase B: table=h1 pairs, dense=padded W2/b2, head=lin
    with ThreadPoolExecutor(cfg.P) as exe:
        resB = list(exe.map(
            lambda d: runners[d](phase_inputs(d, h1_bf16, h1_f32, W2p, b2p, lwp, lbp)),
            range(cfg.P)))
    out = np.concatenate([np.asarray(r["outT"]).T for r in resB], axis=0)
    return out.astype(np.float32)


def kernel(x, edge_index, edge_logits, W1, b1, W2, b2, lin_w, lin_b):
    inputs = dict(x=x, edge_index=edge_index, edge_logits=edge_logits,
                  W1=W1, b1=b1, W2=W2, b2=b2, lin_w=lin_w, lin_b=lin_b)
    return run_two_phase(inputs, FULL)


# revision 12
# speedup vs baseline: 4.5751x; 1.0299x over previous
"""Trainium2 Bass kernel for a 2-layer edge-gated GCN (DiffGNNPlacement).

Math (reference, per layer):
    ew   = 0.5 + sigmoid(edge_logits)                  # [E]
    deg  = segsum(ew -> col) + 1                       # [N]
    dis  = deg^-1/2
    norm = dis[row] * ew * dis[col]                    # [E]
    out  = segsum(norm * (h@W)[row] -> col) + (h@W)*dis^2 + b

Key transform: aggregation commutes with the (linear) feature transform, so
    out = (segsum(norm * h[row] -> col) + h*dis^2) @ W + b
and the self-loop term is folded in host-side via the sxT input.

Device algorithm (per core, nodes sharded 12500/core):
  - SWDGE descriptor generation is the bottleneck (~9.5ns/descriptor): every
    gathered row costs one descriptor, so nodes are PAIRED per core (greedy
    matching of sources that co-occur in the same 16-target-col bucket) and
    the per-core feature table stores one 256B bf16 row per PAIR. One
    descriptor then feeds up to two edges.
  - edges partitioned by target shard, sorted by target col; slots are
    (pair, bucket) units; tiles pack 128 slots spanning <=32 target cols.
  - per tile: dma_gather 128 pair-rows -> SBUF [128, 128] bf16; host-built
    Su/Sv [128, 32] bf16 (norms of the u-half / v-half edges); two PE
    matmuls psum[64, off:off+w] += gathered[:, h*64:h*64+64]^T @ S_h
    accumulate the aggregation z^T for a 512-col PSUM window.
  - gathers spread over all 4 SWDGE queues (queue q = Q7 cores 2q/2q+1),
    queue_num = issue_index % 4 so the Tile DMASW lane (issue % 8) is fed by
    a single queue (completions stay FIFO per lane -> race-free). Early
    chunks are small for fast pipeline ramp.
  - dense h = relu(z @ W + b) and the classifier head are interleaved into
    the window loop (run in PE stall gaps while descriptors generate).

The same compiled program serves both layers; it is launched twice per core
with a host re-pairing of h1 shards in between.
"""

import os
import sys
import math
import numpy as np
import ml_dtypes
from contextlib import ExitStack

for _p in ("/opt/trn_rl_repo", "/root/.axon_site/_ro/trn_rl_repo"):
    if os.path.isdir(_p) and _p not in sys.path:
        sys.path.insert(0, _p)

BF16 = ml_dtypes.bfloat16


# ----------------------------------------------------------------- config ---
class Cfg:
    def __init__(self, N=100000, E=1600000, C=64, H2=32, P=8,
                 PAIR_CHUNK=25000, B=16, W=32, WIN=512, TCH=16, TBLW=128):
        self.N, self.E, self.C, self.H2, self.P = N, E, C, H2, P
        self.NLOC = N // P
        self.NPAIR = N // 2
        self.PAIR_CHUNK = PAIR_CHUNK          # pairs per gather group
        self.NGRP = (self.NPAIR + PAIR_CHUNK - 1) // PAIR_CHUNK
        self.B = B            # pairing bucket (target cols)
        self.W = W            # S tile width (target-col window per tile)
        self.WIN = WIN        # PSUM accumulation window (cols)
        self.TCH = TCH        # steady-state tiles per gather chunk
        self.RAMP = (6, 12, 24)  # tile counts for the first chunks
        self.TBLW = TBLW      # pair row width (bf16 elems; 256B rows)
        self.NWIN = (self.NLOC + WIN - 1) // WIN
        assert PAIR_CHUNK <= 32767
        assert TBLW * 2 == 256  # dma_gather elem constraint (256B rows)
        assert 2 * B <= W and WIN % B == 0


FULL = Cfg()


# --------------------------------------------------------- host preprocess ---
def _sigmoid(x):
    return 0.5 * (np.tanh(0.5 * x) + 1.0)


def _pair_nodes(r, c, cfg):
    """Greedy per-core pairing: sort sources by the first B-col bucket they
    appear in, pair adjacent. Returns pairs [NPAIR, 2] (a permutation of all
    nodes)."""
    N, B = cfg.N, cfg.B
    bucket = c // B
    o = np.lexsort((bucket, r))
    rs, bs = r[o], bucket[o]
    first = np.ones(len(rs), bool)
    if len(rs):
        first[1:] = rs[1:] != rs[:-1]
    src_f, buck_f = rs[first], bs[first]
    present = np.zeros(N, bool)
    present[src_f] = True
    absent = np.where(~present)[0]
    o3 = np.argsort(buck_f, kind="stable")
    allsrc = np.concatenate([src_f[o3], absent])
    pairs = allsrc.reshape(-1, 2)
    pairid = np.empty(N, np.int64)
    half = np.empty(N, np.int8)
    pairid[pairs[:, 0]] = np.arange(len(pairs))
    pairid[pairs[:, 1]] = np.arange(len(pairs))
    half[pairs[:, 0]] = 0
    half[pairs[:, 1]] = 1
    return pairs, pairid, half


def preprocess(edge_index, edge_logits, cfg=FULL):
    """Compute norms and per-device pairings + tile plans (pure numpy)."""
    N, NLOC = cfg.N, cfg.NLOC
    row = np.asarray(edge_index[0], dtype=np.int64)
    col = np.asarray(edge_index[1], dtype=np.int64)
    ew = (0.5 + _sigmoid(np.asarray(edge_logits, dtype=np.float32))).astype(np.float32)
    deg = np.bincount(col, weights=ew.astype(np.float64), minlength=N).astype(np.float32) + 1.0
    dis = deg ** -0.5
    norm = (dis[row] * ew * dis[col]).astype(np.float32)

    dev = col // NLOC
    order = np.argsort(dev, kind="stable")
    rs, cs, vs, ds = row[order], col[order] % NLOC, norm[order], dev[order]
    bounds = np.searchsorted(ds, np.arange(cfg.P + 1))
    plans = []
    for d in range(cfg.P):
        a, b = bounds[d], bounds[d + 1]
        plans.append(_plan_device(rs[a:b], cs[a:b], vs[a:b], cfg))
    return plans, dis


def _chunk_sizes(T, cfg):
    sizes = []
    for s in cfg.RAMP:
        if T - sum(sizes) <= 0:
            break
        sizes.append(min(s, cfg.TCH, T - sum(sizes)))
    rem = T - sum(sizes)
    while rem > 0:
        s = min(cfg.TCH, rem)
        sizes.append(s)
        rem -= s
    return sizes or [0]


def _plan_device(r, c, v, cfg):
    """Pair sources, build (pair, bucket) slots, pack 128-slot tiles."""
    B, W, WIN, G = cfg.B, cfg.W, cfg.WIN, cfg.NGRP
    pairs, pairid, half = _pair_nodes(r, c, cfg)
    perm = pairs.reshape(-1)                     # node order in the pair table

    p = pairid[r]
    h = half[r].astype(np.int64)
    grp = p // cfg.PAIR_CHUNK
    bucket = c // B

    # slots: distinct (grp, bucket, pair); edges sorted into slot-major order
    okey = ((grp * 800 + bucket) * np.int64(cfg.NPAIR)) + p
    o = np.argsort(okey, kind="stable")
    co, vo, ho, go, ko = c[o], v[o], h[o], grp[o], okey[o]
    po = p[o]
    m = len(ko)
    newslot = np.ones(max(m, 1), bool)
    if m:
        newslot[1:m] = ko[1:] != ko[:-1]
    slot_of_edge = np.cumsum(newslot[:m]) - 1 if m else np.zeros(0, np.int64)
    nslot = int(slot_of_edge[-1]) + 1 if m else 0
    slot_starts = np.where(newslot[:m])[0] if m else np.zeros(0, np.int64)
    slot_pair = po[slot_starts] if m else np.zeros(0, np.int64)
    slot_grp = go[slot_starts] if m else np.zeros(0, np.int64)
    slot_c0 = np.minimum.reduceat(co, slot_starts) if m else np.zeros(0, np.int64)
    slot_cmax = np.maximum.reduceat(co, slot_starts) if m else np.zeros(0, np.int64)

    # tiles: sequential pack per grp, <=128 slots, span < W, same WIN window
    tile_id = np.zeros(max(nslot, 1), np.int64)
    tile_c0s = []
    tile_grps = []
    t = -1
    cnt = 0
    cur_c0 = -10 ** 9
    cur_grp = -1
    for s in range(nslot):
        g = int(slot_grp[s])
        # align the tile base to the slot's bucket start: later slots in the
        # same bucket (ordered by pair id) may have smaller cols
        c0 = (int(slot_c0[s]) // B) * B
        cmax = int(slot_cmax[s])
        if (g != cur_grp or cnt >= 128 or cmax >= cur_c0 + W
                or (cmax // WIN) != (cur_c0 // WIN)):
            t += 1
            cnt = 0
            cur_c0 = c0
            cur_grp = g
            tile_c0s.append(c0)
            tile_grps.append(g)
        tile_id[s] = t
        cnt += 1
    T = t + 1 if nslot else 0
    tile_c0s = np.array(tile_c0s if T else [0], np.int32)
    tile_grp = np.array(tile_grps if T else [0], np.int64)
    tile_first_slot = np.searchsorted(tile_id[:max(nslot, 1)], np.arange(max(T, 1)))
    slot_in_tile = (np.arange(max(nslot, 1)) - tile_first_slot[tile_id]) if nslot else np.zeros(1, np.int64)

    Tm = max(T, 1)
    idx16 = np.zeros((Tm, 128), np.int16)
    if nslot:
        idx16[tile_id[:nslot], slot_in_tile[:nslot]] = (
            slot_pair - slot_grp * cfg.PAIR_CHUNK).astype(np.int16)
    S = np.zeros((Tm, 2, 128, cfg.W), np.float32)
    if m:
        e_tile = tile_id[slot_of_edge]
        e_slot = slot_in_tile[slot_of_edge]
        e_coff = co - tile_c0s[e_tile]
        np.add.at(S, (e_tile, ho, e_slot, e_coff), vo)

    # tile-major packing for the device
    wrapped = idx16.reshape(Tm, 8, 16).transpose(2, 0, 1)          # [16, T, 8]
    idx_w = np.ascontiguousarray(np.tile(wrapped.reshape(16, Tm * 8), (8, 1)))
    S_pk = np.ascontiguousarray(S.transpose(2, 0, 1, 3)).astype(BF16)  # [128, T, 2, W]

    # group tiles by grp for chunking (tiles are grp-ordered)
    gT = [int(np.sum(tile_grp[:T] == g)) for g in range(G)] if T else [0] * G
    gt0 = [int(np.searchsorted(tile_grp[:T], g)) for g in range(G)] if T else [0] * G
    gchunks = []
    for g in range(G):
        chunks = []
        t0 = gt0[g]
        for s in _chunk_sizes(gT[g], cfg):
            chunks.append((t0, s))
            t0 += s
        gchunks.append(chunks)

    win = tile_c0s // WIN
    off = tile_c0s - win * WIN
    return dict(T=T, idx=idx_w, S=S_pk, perm=perm, gchunks=gchunks,
                gT=gT, gt0=gt0, win=win, off=off, tile_grp=tile_grp)


# ---------------------------------------------------------- program builder ---
def build_program(plan, cfg=FULL, name="gnn"):
    import concourse.bass as bass
    import concourse.mybir as mybir
    from concourse import bacc
    from concourse.tile import TileContext

    f32, i16, bf16 = mybir.dt.float32, mybir.dt.int16, mybir.dt.bfloat16
    C, W, WIN, NLOC = cfg.C, cfg.W, cfg.WIN, cfg.NLOC
    TBLW = cfg.TBLW
    G = cfg.NGRP
    NQ = 4  # SWDGE queues
    TCH = cfg.TCH

    nc = bacc.Bacc("TRN2", enable_partition_id=False,
                   target_bir_lowering=False, name=name,
                   num_swdge_queues=NQ)

    table = nc.dram_tensor("table", [cfg.NPAIR, TBLW], bf16, kind="ExternalInput")
    sxT_dr = nc.dram_tensor("sxT", [C, NLOC], f32, kind="ExternalInput")
    Wd = nc.dram_tensor("Wd", [C, C], bf16, kind="ExternalInput")
    bb_dr = nc.dram_tensor("bb", [128, C], f32, kind="ExternalInput")
    bdc = nc.dram_tensor("bdc", [C, 1], f32, kind="ExternalInput")
    lw = nc.dram_tensor("lw", [C, 1], bf16, kind="ExternalInput")
    lb = nc.dram_tensor("lb", [1, 1], f32, kind="ExternalInput")
    idx_dr = nc.dram_tensor("idx", list(plan["idx"].shape), i16, kind="ExternalInput")
    S_dr = nc.dram_tensor("S", list(plan["S"].shape), bf16, kind="ExternalInput")
    h_out = nc.dram_tensor("h_out", [NLOC, C], bf16, kind="ExternalOutput")
    outT = nc.dram_tensor("outT", [2, NLOC], f32, kind="ExternalOutput")

    # per-window tile lists: (g, t, off, weff)
    win_tiles = [[] for _ in range(cfg.NWIN)]
    for t in range(plan["T"]):
        w = int(plan["win"][t])
        off = int(plan["off"][t])
        wlen = min(WIN, NLOC - w * WIN)
        weff = min(W, wlen - off)
        win_tiles[w].append((int(plan["tile_grp"][t]), t, off, weff))

    # tile -> chunk per group
    tile_chunk = np.zeros(max(plan["T"], 1), np.int64)
    for g in range(G):
        for ci, (t0, ntl) in enumerate(plan["gchunks"][g]):
            tile_chunk[t0:t0 + ntl] = ci

    _stage = os.environ.get("GNN_STAGE", "all")

    with TileContext(nc) as tc, ExitStack() as ex:
        cpool = ex.enter_context(tc.tile_pool(name="consts", bufs=1))
        zpool = ex.enter_context(tc.tile_pool(name="z", bufs=1))
        gpools = [ex.enter_context(tc.tile_pool(name=f"gat{g}", bufs=8)) for g in range(G)]
        ipools = [ex.enter_context(tc.tile_pool(name=f"idx{g}", bufs=8)) for g in range(G)]
        spools = [ex.enter_context(tc.tile_pool(name=f"s{g}", bufs=8)) for g in range(G)]
        ppool = ex.enter_context(tc.tile_pool(name="psagg", bufs=2, space="PSUM"))
        pdpool = ex.enter_context(tc.tile_pool(name="psd", bufs=2, space="PSUM"))
        ptpool = ex.enter_context(tc.tile_pool(name="pst", bufs=2, space="PSUM"))
        plpool = ex.enter_context(tc.tile_pool(name="psl", bufs=2, space="PSUM"))
        hpool = ex.enter_context(tc.tile_pool(name="hrows", bufs=2))
        htpool = ex.enter_context(tc.tile_pool(name="ht", bufs=2))
        opool = ex.enter_context(tc.tile_pool(name="ot", bufs=2))

        # ---- aggregation chunk loader
        cur = [dict(ch=-1, gb=None, sb=None, t0=0) for _ in range(G)]
        q_counter = [0]

        def ensure_chunk(g, ch):
            st = cur[g]
            if st["ch"] == ch:
                return st
            t0, ntl = plan["gchunks"][g][ch]
            nid = ntl * 128
            ib = ipools[g].tile([128, TCH * 8], i16, tag="idx")
            nc.sync.dma_start(out=ib[:, : ntl * 8], in_=idx_dr[:, t0 * 8:(t0 + ntl) * 8])
            sb = spools[g].tile([128, TCH, 2, W], bf16, tag="s")
            nc.scalar.dma_start(out=sb[:, :ntl, :, :], in_=S_dr[:, t0:t0 + ntl, :, :])
            gb = gpools[g].tile([128, TCH, TBLW], bf16, tag="g")
            nc.gpsimd.dma_gather(
                gb[:, :ntl, :],
                table[g * cfg.PAIR_CHUNK:(g + 1) * cfg.PAIR_CHUNK, :],
                ib[:, : ntl * 8],
                nid, nid, TBLW,
                single_packet=False,
                queue_num=q_counter[0] % NQ,
            )
            q_counter[0] += 1
            st.update(ch=ch, gb=gb, sb=sb, t0=t0)
            return st

        # pre-warm the first gather round before the big sxT DMA queues up
        for g in range(G):
            if plan["gT"][g]:
                ensure_chunk(g, 0)

        # ---- constants
        zrow = cpool.tile([1, WIN], bf16)
        nc.vector.memset(zrow[:, :], 0.0)
        Wd_sb = cpool.tile([C, C], bf16)
        nc.sync.dma_start(out=Wd_sb[:, :], in_=Wd[:, :])
        bb = cpool.tile([128, C], f32)
        nc.sync.dma_start(out=bb[:, :], in_=bb_dr[:, :])
        bd_col = cpool.tile([C, 1], f32)
        nc.sync.dma_start(out=bd_col[:, :], in_=bdc[:, :])
        lw_sb = cpool.tile([C, 1], bf16)
        nc.sync.dma_start(out=lw_sb[:, :], in_=lw[:, :])
        lb_sb = cpool.tile([1, 1], f32)
        nc.sync.dma_start(out=lb_sb[:, :], in_=lb[:, :])
        nlb = cpool.tile([1, 1], f32)
        nc.scalar.mul(nlb[:, :], lb_sb[:, :], -1.0)

        zT = zpool.tile([C, NLOC], f32)
        nc.sync.dma_start(out=zT[:, :], in_=sxT_dr[:, :])  # self-loop term
        zb = zpool.tile([C, NLOC], bf16)

        HB = WIN // 128

        def dense_for_window(w):
            wlen = min(WIN, NLOC - w * WIN)
            nchunks = (wlen + 127) // 128
            hb = hpool.tile([128, HB, C], bf16, tag="h")
            for kk in range(nchunks):
                k = w * HB + kk
                mrow = min(128, NLOC - k * 128)
                psd = pdpool.tile([128, C], f32)
                nc.tensor.matmul(psd[:mrow, :], lhsT=zb[:, k * 128:k * 128 + mrow],
                                 rhs=Wd_sb[:, :], start=True, stop=True)
                nc.vector.tensor_tensor(out=hb[:mrow, kk, :], in0=psd[:mrow, :],
                                        in1=bb[:mrow, :], op=mybir.AluOpType.add)
                nc.scalar.activation(hb[:mrow, kk, :], hb[:mrow, kk, :],
                                     mybir.ActivationFunctionType.Relu)
            r0, r1 = w * WIN, w * WIN + wlen
            nfull = (r1 - r0) // 128
            if nfull:
                dst = h_out[r0:r0 + nfull * 128, :].rearrange(
                    "(t p) c -> p t c", p=128)
                nc.sync.dma_start(out=dst, in_=hb[:, :nfull, :])
            rem = (r1 - r0) - nfull * 128
            if rem:
                nc.sync.dma_start(out=h_out[r0 + nfull * 128:r1, :],
                                  in_=hb[:rem, nfull, :])

        def head_for_window(w):
            wlen = min(WIN, NLOC - w * WIN)
            pst = ptpool.tile([C, WIN], f32)
            nc.tensor.matmul(pst[:, :wlen], lhsT=Wd_sb[:, :],
                             rhs=zb[:, w * WIN:w * WIN + wlen],
                             start=True, stop=True)
            ht = htpool.tile([C, WIN], bf16, tag="ht")
            nc.scalar.activation(ht[:, :wlen], pst[:, :wlen],
                                 mybir.ActivationFunctionType.Relu, bias=bd_col[:, :])
            psl = plpool.tile([1, WIN], f32)
            nc.tensor.matmul(psl[:, :wlen], lhsT=lw_sb[:, :], rhs=ht[:, :wlen],
                             start=True, stop=True)
            otn = opool.tile([1, WIN], f32, tag="otn")
            otp = opool.tile([1, WIN], f32, tag="otp")
            nc.scalar.activation(otn[:, :wlen], psl[:, :wlen],
                                 mybir.ActivationFunctionType.Identity,
                                 bias=nlb[:, :], scale=-1.0)
            nc.scalar.activation(otp[:, :wlen], psl[:, :wlen],
                                 mybir.ActivationFunctionType.Identity,
                                 bias=lb_sb[:, :], scale=1.0)
            nc.sync.dma_start(out=outT[0:1, w * WIN:w * WIN + wlen], in_=otn[:, :wlen])
            nc.sync.dma_start(out=outT[1:2, w * WIN:w * WIN + wlen], in_=otp[:, :wlen])

        for w in range(cfg.NWIN):
            wlen = min(WIN, NLOC - w * WIN)
            ps = ppool.tile([C, WIN], f32)
            nc.tensor.matmul(ps[:, :wlen], lhsT=zrow[:, :C], rhs=zrow[:, :wlen],
                             start=True, stop=False)
            for g, t, off, weff in win_tiles[w]:
                st = ensure_chunk(g, int(tile_chunk[t]))
                tp = t - st["t0"]
                for hh in range(2):
                    nc.tensor.matmul(
                        ps[:, off:off + weff],
                        lhsT=st["gb"][:, tp, hh * C:hh * C + C],
                        rhs=st["sb"][:, tp, hh, :weff],
                        start=False, stop=False,
                        skip_group_check=True,
                    )
            nc.tensor.matmul(ps[:, :wlen], lhsT=zrow[:, :C], rhs=zrow[:, :wlen],
                             start=False, stop=True)
            zw = zT[:, w * WIN:w * WIN + wlen]
            nc.vector.tensor_tensor(out=zw, in0=ps[:, :wlen], in1=zw,
                                    op=mybir.AluOpType.add)
            nc.scalar.copy(out=zb[:, w * WIN:w * WIN + wlen], in_=zw)
            # dense/head run one window behind so the in-order PE stream
            # never waits on the cross-engine flush->cast chain
            if w > 0:
                if _stage in ("all", "dense"):
                    dense_for_window(w - 1)
                if _stage in ("all", "head"):
                    head_for_window(w - 1)
        if _stage in ("all", "dense"):
            dense_for_window(cfg.NWIN - 1)
        if _stage in ("all", "head"):
            head_for_window(cfg.NWIN - 1)

    nc.compile()
    return nc


# ------------------------------------------------------------------ runner ---
def make_runner(nc, device):
    """Single-core jit runner pinned to one device, reusable across calls."""
    import jax
    import concourse.mybir as mybir
    from concourse import bass2jax

    bass2jax.install_neuronx_cc_hook()

    in_names, out_names, out_avals, zero_shapes = [], [], [], []
    for alloc in nc.m.functions[0].allocations:
        if not isinstance(alloc, mybir.MemoryLocationSet):
            continue
        nm = alloc.memorylocations[0].name
        if alloc.kind == "ExternalInput":
            in_names.append(nm)
        elif alloc.kind == "ExternalOutput":
            shape = tuple(alloc.tensor_shape)
            dtype = mybir.dt.np(alloc.dtype)
            out_names.append(nm)
            out_avals.append(jax.core.ShapedArray(shape, dtype))
            zero_shapes.append((shape, dtype))
    n_params = len(in_names)
    all_in_names = in_names + out_names
    donate = tuple(range(n_params, n_params + len(out_names)))

    def _body(*args):
        outs = bass2jax._bass_exec_p.bind(
            *args,
            out_avals=tuple(out_avals),
            in_names=tuple(all_in_names),
            out_names=tuple(out_names),
            lowering_input_output_aliases=(),
            sim_require_finite=True,
            sim_require_nnan=True,
            nc=nc,
        )
        return tuple(outs)

    jitted = jax.jit(_body, donate_argnums=donate, keep_unused=True)

    def run(in_map):
        args = [jax.device_put(np.asarray(in_map[nm]), device) for nm in in_names]
        zeros = [jax.device_put(np.zeros(s, d), device) for s, d in zero_shapes]
        outs = jitted(*args, *zeros)
        return {nm: outs[i] for i, nm in enumerate(out_names)}

    return run


# ---------------------------------------------------------------- kernel() ---
_CACHE = {}


def _get_runners(plans, cfg):
    import jax
    key = "runners"
    if key in _CACHE:
        return _CACHE[key]
    devices = jax.devices()[:cfg.P]
    ncs = [build_program(plans[d], cfg, name=f"gnn_d{d}") for d in range(cfg.P)]
    runners = [make_runner(ncs[d], devices[d]) for d in range(cfg.P)]
    _CACHE[key] = runners
    return runners


def run_two_phase(inputs, cfg=FULL):
    import jax
    from concurrent.futures import ThreadPoolExecutor

    x = np.asarray(inputs["x"], np.float32)
    W1 = np.asarray(inputs["W1"], np.float32)
    b1 = np.asarray(inputs["b1"], np.float32)
    W2 = np.asarray(inputs["W2"], np.float32)
    b2 = np.asarray(inputs["b2"], np.float32)
    lin_w = np.asarray(inputs["lin_w"], np.float32)
    lin_b = np.asarray(inputs["lin_b"], np.float32)
    C, H2 = cfg.C, cfg.H2

    plans, dis = preprocess(inputs["edge_index"], inputs["edge_logits"], cfg)
    dis2 = (dis * dis).astype(np.float32)
    runners = _get_runners(plans, cfg)

    W2p = np.zeros((C, C), np.float32)
    W2p[:, :H2] = W2
    b2p = np.zeros(C, np.float32)
    b2p[:H2] = b2
    lwp = np.zeros((C, 1), np.float32)
    lwp[:H2, 0] = lin_w[:, 0]
    lbp = lin_b.reshape(1, 1)
    zconst = np.zeros((C, 1), BF16)

    def pair_table(d, f_bf16):
        return np.ascontiguousarray(
            f_bf16[plans[d]["perm"]].reshape(cfg.NPAIR, cfg.TBLW))

    def phase_inputs(d, tbl_bf16, sx32, Wdv, bdv, lwv, lbv):
        sh = slice(d * cfg.NLOC, (d + 1) * cfg.NLOC)
        sxT = np.ascontiguousarray((sx32[sh] * dis2[sh, None]).T)
        m = dict(table=pair_table(d, tbl_bf16), sxT=sxT, Wd=Wdv.astype(BF16),
                 bb=np.tile(bdv, (128, 1)).astype(np.float32),
                 bdc=bdv.reshape(C, 1).astype(np.float32),
                 lw=lwv.astype(BF16), lb=lbv.astype(np.float32),
                 idx=plans[d]["idx"], S=plans[d]["S"])
        return m

    x_bf16 = x.astype(BF16)

    # phase A: table=x pairs, dense=W1/b1 (head inputs zeroed)
    with ThreadPoolExecutor(cfg.P) as exe:
        resA = list(exe.map(
            lambda d: runners[d](phase_inputs(d, x_bf16, x, W1, b1, zconst,
                                              np.zeros((1, 1), np.float32))),
            range(cfg.P)))
    h_shards = [np.asarray(r["h_out"]) for r in resA]
    h1_bf16 = np.concatenate(h_shards, axis=0)          # [N, 64] bf16
    h1_f32 = h1_bf16.astype(np.float32)

    # phase B: table=h1 pairs, dense=padded W2/b2, head=lin
    with ThreadPoolExecutor(cfg.P) as exe:
        resB = list(exe.map(
            lambda d: runners[d](phase_inputs(d, h1_bf16, h1_f32, W2p, b2p, lwp, lbp)),
            range(cfg.P)))
    out = np.concatenate([np.asarray(r["outT"]).T for r in resB], axis=0)
    return out.astype(np.float32)


def kernel(x, edge_index, edge_logits, W1, b1, W2, b2, lin_w, lin_b):
    inputs = dict(x=x, edge_index=edge_index, edge_logits=edge_logits,
                  W1=W1, b1=b1, W2=W2, b2=b2, lin_w=lin_w, lin_b=lin_b)
    return run_two_phase(inputs, FULL)
